# revision 1
# baseline (speedup 1.0000x reference)
"""DeepseekV2 MLA attention on 8 Trainium2 NeuronCores.

Sharding: token-split A-projections -> AllGather(latents) -> head-split
(4 heads/core) B-projections + causal attention -> AllGather(attn out) ->
D-column-split output projection. Layouts are d-major (feature dim on the
SBUF partition axis) so no on-device transposes are needed; the host
pre-transposes h and re-orders weight columns instead.

Precision: bf16 matmul inputs with fp32 PSUM accumulation throughout;
rmsnorm statistics, softmax (exp, denominators, rescale) and all staging
run in fp32/fp32r. Measured end-to-end relative error vs the fp32 jax
reference: ~3.9e-3.
"""
import math

import numpy as np
import ml_dtypes

import concourse.bass as bass
import concourse.mybir as mybir
from concourse.tile import TileContext
from concourse import bass_utils

# ---------------------------------------------------------------------------
# Walrus workaround: this container's walrus accepts at most ONE sync-wait
# per TPB instruction, but Tile attaches several (tail Drain, LDWEIGHTS...).
# Split: keep the last wait, move the rest onto preceding same-engine NOPs.
# ---------------------------------------------------------------------------
import concourse.tile as _tile_mod

_orig_sched = _tile_mod.TileContext.schedule_and_allocate
_nopctr = [0]


def _split_multiwait(nc):
    for fn in nc.m.functions:
        for blk in fn.blocks:
            insts = blk.instructions
            if not any(
                i.sync_info and i.sync_info.on_wait and len(i.sync_info.on_wait) > 1
                for i in insts
            ):
                continue
            out = []
            for ins in insts:
                si = ins.sync_info
                if si and si.on_wait and len(si.on_wait) > 1:
                    waits = list(si.on_wait)
                    for w in waits[:-1]:
                        _nopctr[0] += 1
                        nop = mybir.InstNoOp(name=f"I-mws-{_nopctr[0]}", ins=[], outs=[])
                        nop.engine = ins.engine
                        nop.sync_info = mybir.SyncInfo(on_wait=[w], on_update=[])
                        out.append(nop)
                    ins.sync_info = mybir.SyncInfo(
                        on_wait=[waits[-1]], on_update=list(si.on_update or [])
                    )
                out.append(ins)
            blk.instructions = out


def _patched_sched(self, *a, **k):
    res = _orig_sched(self, *a, **k)
    _split_multiwait(self.nc)
    return res


if getattr(_tile_mod.TileContext.schedule_and_allocate, "__name__", "") != "_patched_sched":
    _tile_mod.TileContext.schedule_and_allocate = _patched_sched


# ---------------------------------------------------------------------------
T, D, H = 2048, 5120, 32
NOPE, ROPE, QK = 128, 64, 192
KVR, QR, VH = 512, 1536, 128
EPS, THETA = 1e-6, 10000.0
NCORES = 8
HL = H // NCORES          # 4 heads per core
TC = T // NCORES          # 256 tokens per core
LAT = KVR + ROPE          # 576
AGR = QR + LAT            # 2112 rows in allgather-1
DCOL = D // NCORES        # 640 output columns per core

F32 = mybir.dt.float32
F32R = mybir.dt.float32r
BF16 = mybir.dt.bfloat16
AF = mybir.ActivationFunctionType
MUL = mybir.AluOpType.mult
ADD = mybir.AluOpType.add
SUB = mybir.AluOpType.subtract

TRACE = [False]          # test.py sets TRACE[0]=True to profile
LAST_RESULT = [None]     # BassKernelResults stashed here for test.py

_cache = {}


def _phase_a(nc, tc, io, consts_t, ag1a_in, ag1b_in):
    """Token-split A projections (bf16), rmsnorms, k_pe rope -> ag1_in."""
    ones_c, ones_r = consts_t["ones_c"], consts_t["ones_r"]
    cosa_sb, sina_sb, bias_sb = (consts_t["cosa_sb"], consts_t["sina_sb"],
                                 consts_t["bias_sb"])
    with (
        tc.tile_pool(name="a_ht", bufs=1) as a_ht,
        tc.tile_pool(name="a_w", bufs=3) as a_w,
        tc.tile_pool(name="a_st", bufs=1) as a_st,
        tc.tile_pool(name="a_tmp", bufs=3) as a_tmp,
        tc.tile_pool(name="a_ps", bufs=2, space="PSUM") as a_ps,
        tc.tile_pool(name="a_ss", bufs=1, space="PSUM") as a_ss,
    ):
        ht_sb = a_ht.tile([128, 40 * TC], BF16, name="ht_sb")
        htv = ht_sb[:].rearrange("p (k t) -> p k t", k=40)
        nc.sync.dma_start(htv, io["hT"][:].rearrange("(k p) t -> p k t", p=128))
        stage = a_st.tile([128, 17 * TC], F32R, name="stage")
        ss_q = a_ss.tile([1, TC], F32, name="ss_q")
        ss_kv = a_ss.tile([1, TC], F32, name="ss_kv")

        for m in range(17):
            mrows = 64 if m == 16 else 128
            # one column-chunk DMA: all 40 k-chunks of this m column
            wt = a_w.tile([128, 40 * 128], BF16, name=f"a_w_{m}", tag="aw")
            wtv = wt[:].rearrange("p (k c) -> p k c", k=40)
            if m < 12:
                nc.sync.dma_start(
                    wtv[:, :, :mrows],
                    io["wqa"][:].rearrange("(k p) q -> p k q", p=128)[
                        :, :, m * 128:(m + 1) * 128])
            else:
                nc.sync.dma_start(
                    wtv[:, :, :mrows],
                    io["wkva"][:].rearrange("(k p) q -> p k q", p=128)[
                        :, :, (m - 12) * 128:(m - 12) * 128 + mrows])
            ps = a_ps.tile([128, TC], F32, name=f"a_ps_{m}", tag="aps")
            for k in range(40):
                nc.tensor.matmul(ps[:mrows, :], wtv[:, k, :mrows], htv[:, k, :],
                                 start=(k == 0), stop=(k == 39))
            st = stage[:, m * TC:(m + 1) * TC]
            if m < 12:
                nc.vector.tensor_copy(st, ps[:])
                sq = a_tmp.tile([128, TC], F32R, name=f"sq_{m}", tag="sq")
                nc.scalar.activation(sq[:], st, AF.Square)
                nc.tensor.matmul(ss_q[:], ones_c, sq[:],
                                 start=(m == 0), stop=(m == 11))
            elif m < 16:
                nc.vector.tensor_scalar(st, ps[:], bias_sb[:, m - 12:m - 11],
                                        None, op0=ADD)
                sq = a_tmp.tile([128, TC], F32R, name=f"sq_{m}", tag="sq")
                nc.scalar.activation(sq[:], st, AF.Square)
                nc.tensor.matmul(ss_kv[:], ones_c, sq[:],
                                 start=(m == 12), stop=(m == 15))
            else:
                nc.vector.tensor_scalar(st[:64, :], ps[:64, :],
                                        bias_sb[:64, 4:5], None, op0=ADD)

        # rms scales: 1/sqrt(mean(ss) + eps) broadcast to 128 partitions
        bcs = {}
        for key, ss, nfeat in (("q", ss_q, QR), ("kv", ss_kv, KVR)):
            ms = a_tmp.tile([1, TC], F32R, name=f"ms_{key}", tag="ms")
            nc.vector.tensor_scalar(ms[:], ss[:], 1.0 / nfeat, EPS,
                                    op0=MUL, op1=ADD)
            sq2 = a_tmp.tile([1, TC], F32R, name=f"sqr_{key}", tag="sqr")
            nc.scalar.activation(sq2[:], ms[:], AF.Sqrt)
            rs = a_tmp.tile([1, TC], F32R, name=f"rs_{key}", tag="rs")
            with nc.allow_low_precision(reason="f32r holds full fp32 bits"):
                nc.vector.reciprocal(rs[:], sq2[:])
            bps = a_ps.tile([128, TC], F32, name=f"bps_{key}", tag="bps")
            nc.tensor.matmul(bps[:], ones_r[:1, :], rs[:], start=True, stop=True)
            bc = a_tmp.tile([128, TC], F32R, name=f"bc_{key}", tag=f"bc{key}")
            nc.vector.tensor_copy(bc[:], bps[:])
            bcs[key] = bc

        for m in range(16):
            st = stage[:, m * TC:(m + 1) * TC]
            sc = a_tmp.tile([128, TC], BF16, name=f"sc_{m}", tag="sc")
            nc.vector.tensor_tensor(sc[:], st, bcs["q" if m < 12 else "kv"][:],
                                    op=MUL)
            if m < 8:
                nc.sync.dma_start(ag1a_in[m * 128:(m + 1) * 128, :], sc[:])
            else:
                nc.sync.dma_start(ag1b_in[(m - 8) * 128:(m - 7) * 128, :], sc[:])

        # k_pe rope (no norm) -> rows 2048:2112
        st = stage[:, 16 * TC:17 * TC]
        rp = a_tmp.tile([64, TC], BF16, name="rp_kpe")
        t1 = a_tmp.tile([32, TC], F32R, name="rt1", tag="rt1")
        t2 = a_tmp.tile([32, TC], F32R, name="rt2", tag="rt2")
        x1, x2 = st[0:32, :], st[32:64, :]
        nc.vector.tensor_tensor(t1[:], x1, cosa_sb[0:32, :], op=MUL)
        nc.vector.tensor_tensor(t2[:], x2, sina_sb[32:64, :], op=MUL)
        nc.vector.tensor_tensor(rp[0:32, :], t1[:], t2[:], op=SUB)
        nc.vector.tensor_tensor(t1[:], x1, sina_sb[0:32, :], op=MUL)
        nc.vector.tensor_tensor(t2[:], x2, cosa_sb[32:64, :], op=MUL)
        nc.vector.tensor_tensor(rp[32:64, :], t1[:], t2[:], op=ADD)
        nc.sync.dma_start(ag1b_in[1024:1088, :], rp[:])


def _phase_b(nc, tc, io, ag1bv, ktv, vv, kpe_sb):
    """Head-split k_nope^T and v projections from the gathered latents."""
    with (
        tc.tile_pool(name="b_kva", bufs=1) as b_kva,
        tc.tile_pool(name="b_w", bufs=4) as b_w,
        tc.tile_pool(name="b_ps", bufs=2, space="PSUM") as b_ps,
    ):
        kva_sb = b_kva.tile([128, 4 * T], BF16, name="kva_sb")
        kvav = kva_sb[:].rearrange("p (k t) -> p k t", k=4)
        for k in range(4):
            nc.sync.dma_start(
                kvav[:, k, :].rearrange("p (r t) -> p r t", r=NCORES),
                ag1bv[512 + k * 128:512 + (k + 1) * 128])
        nc.sync.dma_start(
            kpe_sb[:].rearrange("p (r t) -> p r t", r=NCORES),
            ag1bv[1024:1088])

        wk_sb = b_w.tile([128, 4 * 512], BF16, name="wk_sb", tag="wkw")
        wkv_ = wk_sb[:].rearrange("p (k c) -> p k c", k=4)
        nc.sync.dma_start(wkv_, io["wkvbk"][:].rearrange("(k p) c -> p k c", p=128))
        for j in range(HL):
            for qb in range(4):
                ps = b_ps.tile([128, 512], F32, name=f"psk_{j}_{qb}", tag="psk")
                for k in range(4):
                    nc.tensor.matmul(ps[:], wkv_[:, k, j * 128:(j + 1) * 128],
                                     kvav[:, k, qb * 512:(qb + 1) * 512],
                                     start=(k == 0), stop=(k == 3))
                nc.vector.tensor_copy(ktv[:, j, qb * 512:(qb + 1) * 512], ps[:])

        wv_sb = b_w.tile([128, 4 * 512], BF16, name="wv_sb", tag="wvw")
        wvv = wv_sb[:].rearrange("p (k c) -> p k c", k=4)
        nc.sync.dma_start(wvv, io["wkvbv"][:].rearrange("(k p) c -> p k c", p=128))
        for mt in range(16):
            ps = b_ps.tile([128, 512], F32, name=f"psv_{mt}", tag="psv")
            for k in range(4):
                nc.tensor.matmul(ps[:], kvav[:, k, mt * 128:(mt + 1) * 128],
                                 wvv[:, k, :], start=(k == 0), stop=(k == 3))
            nc.vector.tensor_copy(vv[:, mt, :], ps[:])


def _phase_q(nc, tc, io, ag1av, ag1bv, qt_spill):
    """Head-split q^T projection with rope, into the resident qT tile."""
    with (
        tc.tile_pool(name="c_qa", bufs=1) as c_qa,
        tc.tile_pool(name="c_tab", bufs=1) as c_tab,
        tc.tile_pool(name="c_w", bufs=3) as c_w,
        tc.tile_pool(name="c_tmp", bufs=3) as c_tmp,
        tc.tile_pool(name="c_ps", bufs=2, space="PSUM") as c_ps,
    ):
        cos_sb = c_tab.tile([128, T], F32R, name="cos_sb")
        sin_sb = c_tab.tile([128, T], F32R, name="sin_sb")
        nc.sync.dma_start(cos_sb[:], io["cosT"][:])
        nc.sync.dma_start(sin_sb[:], io["sinT"][:])
        qa_sb = c_qa.tile([128, 12 * T], BF16, name="qa_sb")
        qav = qa_sb[:].rearrange("p (k t) -> p k t", k=12)
        for k in range(12):
            srcv = (ag1av[k * 128:(k + 1) * 128] if k < 8 else
                    ag1bv[(k - 8) * 128:(k - 7) * 128])
            nc.sync.dma_start(
                qav[:, k, :].rearrange("p (r t) -> p r t", r=NCORES), srcv)
        for m in range(6):
            wt = c_w.tile([128, 12 * 128], BF16, name=f"cw_{m}", tag="cw")
            wtv = wt[:].rearrange("p (k c) -> p k c", k=12)
            nc.sync.dma_start(
                wtv, io["wqb"][:].rearrange("(k p) c -> p k c", p=128)[
                    :, :, m * 128:(m + 1) * 128])
            pss = [c_ps.tile([128, 512], F32, name=f"psq_{m}_{qb}", tag=f"psq{qb}")
                   for qb in range(4)]
            for k in range(12):
                for qb in range(4):
                    nc.tensor.matmul(pss[qb][:], wtv[:, k, :],
                                     qav[:, k, qb * 512:(qb + 1) * 512],
                                     start=(k == 0), stop=(k == 11))
            for qb in range(4):
                st = c_tmp.tile([128, 512], BF16, name=f"cst_{m}_{qb}", tag="cst")
                if m < 4:
                    nc.vector.tensor_copy(st[:], pss[qb][:])
                else:
                    cs = cos_sb[:, qb * 512:(qb + 1) * 512]
                    sn = sin_sb[:, qb * 512:(qb + 1) * 512]
                    for half in range(2):
                        r0 = 64 * half
                        x1 = pss[qb][r0:r0 + 32, :]
                        x2 = pss[qb][r0 + 32:r0 + 64, :]
                        t1 = c_tmp.tile([32, 512], F32R,
                                        name=f"ct1_{m}_{qb}_{half}", tag="ct1")
                        t2 = c_tmp.tile([32, 512], F32R,
                                        name=f"ct2_{m}_{qb}_{half}", tag="ct2")
                        nc.vector.tensor_tensor(t1[:], x1, cs[r0:r0 + 32, :],
                                                op=MUL)
                        nc.vector.tensor_tensor(t2[:], x2,
                                                sn[r0 + 32:r0 + 64, :], op=MUL)
                        nc.vector.tensor_tensor(st[r0:r0 + 32, :],
                                                t1[:], t2[:], op=SUB)
                        nc.vector.tensor_tensor(t1[:], x1, sn[r0:r0 + 32, :],
                                                op=MUL)
                        nc.vector.tensor_tensor(t2[:], x2,
                                                cs[r0 + 32:r0 + 64, :], op=MUL)
                        nc.vector.tensor_tensor(st[r0 + 32:r0 + 64, :],
                                                t1[:], t2[:], op=ADD)
                nc.sync.dma_start(
                    qt_spill[m * 128:(m + 1) * 128, qb * 512:(qb + 1) * 512],
                    st[:])


def _phase_attn(nc, tc, qt_spill, ag2_ins, ag2_outs, ktv, vv, kpe_sb, consts_t):
    """Causal attention, two heads interleaved per pass; bf16 out -> ag2_in."""
    ones_c, ones_r, tri_sb = (consts_t["ones_cb"], consts_t["ones_r"],
                              consts_t["tri_b"])
    with (
        tc.tile_pool(name="t_qf", bufs=3) as t_qf,
        tc.tile_pool(name="t_p", bufs=8) as t_p,
        tc.tile_pool(name="t_o", bufs=2) as t_o,
        tc.tile_pool(name="t_ps", bufs=3, space="PSUM") as t_ps,
        tc.tile_pool(name="t_bc", bufs=1, space="PSUM") as t_bc,
        tc.tile_pool(name="t_acc", bufs=1, space="PSUM") as t_acc,
    ):
        for qb in range(4):
            for jp in range(HL // 2):
                js = (2 * jp, 2 * jp + 1)
                qf = {}
                dens, ots = {}, {}
                for s, j in enumerate(js):
                    qfn = t_qf.tile([128, 512], BF16, name=f"qfn_{qb}_{j}",
                                    tag=f"qfn{s}")
                    qfp = t_qf.tile([64, 512], BF16, name=f"qfp_{qb}_{j}",
                                    tag=f"qfp{s}")
                    nc.sync.dma_start(
                        qfn[:], qt_spill[j * 128:(j + 1) * 128,
                                         qb * 512:(qb + 1) * 512])
                    pchunk, phalf = 4 + j // 2, j % 2
                    rr = pchunk * 128 + 64 * phalf
                    nc.sync.dma_start(
                        qfp[:], qt_spill[rr:rr + 64, qb * 512:(qb + 1) * 512])
                    qf[j] = (qfn, qfp)
                    dens[j] = t_acc.tile([1, 512], F32, name=f"den_{qb}_{j}",
                                         tag=f"den{s}")
                    ots[j] = t_acc.tile([128, 512], F32, name=f"ot_{qb}_{j}",
                                        tag=f"ot{s}")
                kmax = 4 * qb + 4
                for kk in range(kmax):
                    o = kk - 4 * qb
                    c0 = max(0, o) * 128
                    pts = {}
                    for s, j in enumerate(js):
                        qfn, qfp = qf[j]
                        sT = t_ps.tile([128, 512], F32,
                                       name=f"sT_{qb}_{j}_{kk}", tag="sT")
                        nc.tensor.matmul(sT[:, c0:512],
                                         ktv[:, j, kk * 128:(kk + 1) * 128],
                                         qfn[:, c0:512], start=True, stop=False)
                        nc.tensor.matmul(sT[:, c0:512],
                                         kpe_sb[:, kk * 128:(kk + 1) * 128],
                                         qfp[:, c0:512], start=False, stop=True)
                        pT = t_p.tile([128, 512], BF16,
                                      name=f"pT_{qb}_{j}_{kk}", tag="pT")
                        nc.scalar.activation(pT[:, c0:512], sT[:, c0:512],
                                             AF.Exp)
                        if o >= 0:
                            nc.vector.tensor_tensor(pT[:, c0:c0 + 128],
                                                    pT[:, c0:c0 + 128],
                                                    tri_sb[:], op=MUL)
                        pts[j] = pT
                    for j in js:
                        pT = pts[j]
                        nc.tensor.matmul(dens[j][:, c0:512], ones_c,
                                         pT[:, c0:512],
                                         start=(kk == 0), stop=(kk == kmax - 1))
                        nc.tensor.matmul(ots[j][:, c0:512],
                                         vv[:, kk, j * 128:(j + 1) * 128],
                                         pT[:, c0:512],
                                         start=(kk == 0), stop=(kk == kmax - 1))
                for s, j in enumerate(js):
                    den, ot = dens[j], ots[j]
                    rden = t_o.tile([1, 512], F32R, name=f"rden_{qb}_{j}",
                                    tag=f"rden{s}")
                    with nc.allow_low_precision(reason="f32r = fp32 bits"):
                        nc.vector.reciprocal(rden[:], den[:])
                    bcp = t_bc.tile([128, 512], F32, name=f"bcp_{qb}_{j}",
                                    tag="bcp")
                    nc.tensor.matmul(bcp[:], ones_r[:1, :], rden[:],
                                     start=True, stop=True)
                    bcs = t_o.tile([128, 512], F32R, name=f"bcs_{qb}_{j}",
                                   tag=f"bcs{s}")
                    nc.vector.tensor_copy(bcs[:], bcp[:])
                    obf = t_o.tile([128, 512], BF16, name=f"obf_{qb}_{j}",
                                   tag=f"obf{s}")
                    nc.vector.tensor_tensor(obf[:], ots[j][:], bcs[:], op=MUL)
                    nc.sync.dma_start(
                        ag2_ins[qb][j * 128:(j + 1) * 128, :], obf[:])
            nc.gpsimd.collective_compute(
                "AllGather", mybir.AluOpType.bypass,
                ins=[ag2_ins[qb][:]], outs=[ag2_outs[qb][:]],
                replica_groups=[list(range(NCORES))],
            )


def _phase_out(nc, tc, io, ag2_outs, wov):
    """D-column-split output projection (bf16); wo preloaded upstream."""
    with (
        tc.tile_pool(name="o_a", bufs=2) as o_a,
        tc.tile_pool(name="o_st", bufs=3) as o_st,
        tc.tile_pool(name="o_ps", bufs=3, space="PSUM") as o_ps,
    ):
        for tq in range(4):
            oa = o_a.tile([128, 32 * 512], BF16, name=f"oa_{tq}", tag="oa")
            oav = oa[:].rearrange("p (k t) -> p k t", k=32)
            nc.sync.dma_start(
                oav, ag2_outs[tq][:].rearrange("(k p) t -> p k t", p=128))
            for d in range(5):
                ps = o_ps.tile([128, 512], F32, name=f"ops_{tq}_{d}", tag="ops")
                for k in range(32):
                    nc.tensor.matmul(ps[:], wov[:, k, d * 128:(d + 1) * 128],
                                     oav[:, k, :], start=(k == 0), stop=(k == 31))
                st = o_st.tile([128, 512], F32, name=f"ost_{tq}_{d}", tag="ost")
                nc.vector.tensor_copy(st[:], ps[:])
                nc.sync.dma_start(
                    io["outT"][d * 128:(d + 1) * 128,
                               tq * 512:(tq + 1) * 512], st[:])


def _build():
    nc = bass.Bass("TRN2", target_bir_lowering=False, debug=False,
                   num_devices=NCORES)
    io = {
        "hT": nc.dram_tensor("hT", [D, TC], BF16, kind="ExternalInput"),
        "wqa": nc.dram_tensor("wqa", [D, QR], BF16, kind="ExternalInput"),
        "wkva": nc.dram_tensor("wkva", [D, LAT], BF16, kind="ExternalInput"),
        "biask": nc.dram_tensor("biask", [128, 5], F32, kind="ExternalInput"),
        "wqb": nc.dram_tensor("wqb", [QR, 6 * 128], BF16, kind="ExternalInput"),
        "wkvbk": nc.dram_tensor("wkvbk", [KVR, HL * NOPE], BF16,
                                kind="ExternalInput"),
        "wkvbv": nc.dram_tensor("wkvbv", [KVR, HL * VH], BF16,
                                kind="ExternalInput"),
        "wo": nc.dram_tensor("wo", [H * VH, DCOL], BF16, kind="ExternalInput"),
        "cosT": nc.dram_tensor("cosT", [128, T], F32R, kind="ExternalInput"),
        "sinT": nc.dram_tensor("sinT", [128, T], F32R, kind="ExternalInput"),
        "cosA": nc.dram_tensor("cosA", [128, TC], F32R, kind="ExternalInput"),
        "sinA": nc.dram_tensor("sinA", [128, TC], F32R, kind="ExternalInput"),
        "tri": nc.dram_tensor("tri", [128, 128], F32R, kind="ExternalInput"),
        "onesin": nc.dram_tensor("onesin", [128, 128], F32R, kind="ExternalInput"),
        "outT": nc.dram_tensor("outT", [DCOL, T], F32, kind="ExternalOutput"),
    }

    with TileContext(nc) as tc:
        with (
            tc.tile_pool(name="dram", bufs=1, space="DRAM") as dram,
            tc.tile_pool(name="consts", bufs=1) as consts,
        ):
            ag1a_in = dram.tile([1024, TC], BF16, name="ag1a_in")
            ag1a_out = dram.tile([NCORES * 1024, TC], BF16, addr_space="Shared",
                                 name="ag1a_out")
            ag1b_in = dram.tile([1088, TC], BF16, name="ag1b_in")
            ag1b_out = dram.tile([NCORES * 1088, TC], BF16, addr_space="Shared",
                                 name="ag1b_out")
            ag2_ins = [dram.tile([HL * VH, 512], BF16, name=f"ag2_in_{qb}")
                       for qb in range(4)]
            ag2_outs = [dram.tile([H * VH, 512], BF16, addr_space="Shared",
                                  name=f"ag2_out_{qb}") for qb in range(4)]

            consts_t = {}
            ones_sb = consts.tile([128, 128], F32R, name="ones_sb")
            nc.sync.dma_start(ones_sb[:], io["onesin"][:])
            consts_t["ones_c"] = ones_sb[:, 0:1]
            consts_t["ones_r"] = ones_sb
            ones_b = consts.tile([128, 1], BF16, name="ones_b")
            nc.vector.tensor_copy(ones_b[:], ones_sb[:, 0:1])
            consts_t["ones_cb"] = ones_b[:]
            trib = consts.tile([128, 128], BF16, name="trib")
            consts_t["tri_b"] = trib
            for nm, srcn, shp in (("tri_sb", "tri", [128, 128]),
                                  ("cosa_sb", "cosA", [128, TC]),
                                  ("sina_sb", "sinA", [128, TC]),
                                  ):
                consts_t[nm] = consts.tile(shp, F32R, name=nm)
                nc.sync.dma_start(consts_t[nm][:], io[srcn][:])
            consts_t["bias_sb"] = consts.tile([128, 5], F32, name="bias_sb")
            nc.sync.dma_start(consts_t["bias_sb"][:], io["biask"][:])
            nc.vector.tensor_copy(trib[:], consts_t["tri_sb"][:])

            with nc.named_scope("phase_a"):
                _phase_a(nc, tc, io, consts_t, ag1a_in, ag1b_in)

            with nc.named_scope("ag1"):
                nc.gpsimd.collective_compute(
                    "AllGather", mybir.AluOpType.bypass,
                    ins=[ag1a_in[:]], outs=[ag1a_out[:]],
                    replica_groups=[list(range(NCORES))],
                )
                nc.gpsimd.collective_compute(
                    "AllGather", mybir.AluOpType.bypass,
                    ins=[ag1b_in[:]], outs=[ag1b_out[:]],
                    replica_groups=[list(range(NCORES))],
                )
            ag1av = ag1a_out[:].rearrange("(r a) t -> a r t", a=1024)
            ag1bv = ag1b_out[:].rearrange("(r a) t -> a r t", a=1088)

            qt_spill = dram.tile([6 * 128, T], BF16, name="qt_spill")
            with nc.named_scope("phase_q"):
                _phase_q(nc, tc, io, ag1av, ag1bv, qt_spill)
            with tc.tile_pool(name="persist", bufs=1) as persist:
                kt_sb = persist.tile([128, HL * T], BF16, name="kt_sb")
                ktv = kt_sb[:].rearrange("p (j t) -> p j t", j=HL)
                v_sb = persist.tile([128, 16 * 512], BF16, name="v_sb")
                vv = v_sb[:].rearrange("p (mt c) -> p mt c", mt=16)
                kpe_sb = persist.tile([64, T], BF16, name="kpe_sb")
                with nc.named_scope("phase_b"):
                    _phase_b(nc, tc, io, ag1bv, ktv, vv, kpe_sb)
                wo_sb = persist.tile([128, 32 * DCOL], BF16, name="wo_sb")
                wov = wo_sb[:].rearrange("p (k c) -> p k c", k=32)
                nc.sync.dma_start(
                    wov, io["wo"][:].rearrange("(k p) c -> p k c", p=128))
                with nc.named_scope("phase_attn"):
                    _phase_attn(nc, tc, qt_spill, ag2_ins, ag2_outs,
                                ktv, vv, kpe_sb, consts_t)

                with nc.named_scope("phase_out"):
                    _phase_out(nc, tc, io, ag2_outs, wov)
    return nc


def _get_nc():
    if "nc" not in _cache:
        _cache["nc"] = _build()
    return _cache["nc"]


def _prep(inputs):
    h = np.asarray(inputs["h"], np.float32)
    pos = np.asarray(inputs["position_ids"], np.int32)
    Wq_a = np.asarray(inputs["Wq_a"], np.float32)
    gq = np.asarray(inputs["gq"], np.float32)
    Wq_b = np.asarray(inputs["Wq_b"], np.float32)
    Wkv_a = np.asarray(inputs["Wkv_a"], np.float32)
    bkv_a = np.asarray(inputs["bkv_a"], np.float32)
    gkv = np.asarray(inputs["gkv"], np.float32)
    Wkv_b = np.asarray(inputs["Wkv_b"], np.float32)
    Wo = np.asarray(inputs["Wo"], np.float32)

    dperm = np.concatenate([np.arange(0, ROPE, 2), np.arange(1, ROPE, 2)])
    scale = np.float32(1.0 / math.sqrt(QK))

    hT = np.ascontiguousarray(h.T)                      # [D, T]
    wkva = Wkv_a.copy()
    wkva[:, KVR:] = Wkv_a[:, KVR + dperm]
    bias = bkv_a.copy()
    bias[KVR:] = bkv_a[KVR + dperm]
    bm = np.zeros((5, 128), np.float32)
    bm.reshape(-1)[:LAT] = bias
    biask = np.ascontiguousarray(bm.T)                  # [128, 5]

    wqb_eff = (Wq_b * gq[:, None]) * scale              # [QR, H*QK]
    wkvb_eff = Wkv_b * gkv[:, None]                     # [KVR, H*(NOPE+VH)]

    inv = THETA ** (-np.arange(0, ROPE, 2, dtype=np.float32) / ROPE)
    fr = pos.astype(np.float32)[:, None] * inv[None, :]  # [T, 32]
    cosT = np.ascontiguousarray(np.tile(np.cos(fr).T, (4, 1)))  # [128, T]
    sinT = np.ascontiguousarray(np.tile(np.sin(fr).T, (4, 1)))
    tri = np.triu(np.ones((128, 128), np.float32))
    wqa_b = Wq_a.astype(ml_dtypes.bfloat16)
    wkva_b = wkva.astype(ml_dtypes.bfloat16)

    bf16 = ml_dtypes.bfloat16
    in_maps = []
    for c in range(NCORES):
        heads = list(range(HL * c, HL * (c + 1)))
        qcols = [np.arange(hh * QK, hh * QK + NOPE) for hh in heads]
        for pair in range(2):
            for hh in heads[2 * pair:2 * pair + 2]:
                qcols.append(hh * QK + NOPE + dperm)
        kcols = np.concatenate(
            [np.arange(hh * (NOPE + VH), hh * (NOPE + VH) + NOPE)
             for hh in heads])
        vcols = np.concatenate(
            [np.arange(hh * (NOPE + VH) + NOPE, (hh + 1) * (NOPE + VH))
             for hh in heads])
        in_maps.append({
            "hT": np.ascontiguousarray(hT[:, c * TC:(c + 1) * TC]).astype(bf16),
            "wqa": wqa_b,
            "wkva": wkva_b,
            "biask": biask,
            "wqb": np.ascontiguousarray(wqb_eff[:, np.concatenate(qcols)]).astype(bf16),
            "wkvbk": np.ascontiguousarray(wkvb_eff[:, kcols]).astype(bf16),
            "wkvbv": np.ascontiguousarray(wkvb_eff[:, vcols]).astype(bf16),
            "wo": np.ascontiguousarray(Wo[:, c * DCOL:(c + 1) * DCOL]).astype(bf16),
            "cosT": cosT,
            "sinT": sinT,
            "cosA": np.ascontiguousarray(cosT[:, c * TC:(c + 1) * TC]),
            "sinA": np.ascontiguousarray(sinT[:, c * TC:(c + 1) * TC]),
            "tri": tri,
            "onesin": np.ones((128, 128), np.float32),
        })
    return in_maps


def kernel(**inputs):
    nc = _get_nc()
    in_maps = _prep(inputs)
    res = bass_utils.run_bass_kernel_spmd(
        nc, in_maps, core_ids=list(range(NCORES)), trace=TRACE[0])
    LAST_RESULT[0] = res
    out = np.empty((T, D), np.float32)
    for c in range(NCORES):
        out[:, c * DCOL:(c + 1) * DCOL] = res.results[c]["outT"].T
    return out



# revision 7
# speedup vs baseline: 1.0253x; 1.0253x over previous
"""DeepseekV2 MLA attention on 8 Trainium2 NeuronCores.

Sharding: token-split A projections -> AllGather(kv latents) early +
AllGather(q latents) -> head-split (4 heads/core) B projections + causal
attention -> per-qb AllGather(attn out) with the D-column-split output
projection interleaved one block behind the attention loop.

Layouts are d-major (feature dim on the SBUF partition axis); the host
pre-transposes h and pre-tiles every weight so each SBUF weight chunk is
one contiguous DMA. q tiles stay SBUF-resident (no DRAM spill).

Precision: bf16 matmul inputs with fp32 PSUM accumulation; rmsnorm
statistics and softmax run in fp32/f32r; softmax reciprocal via
reciprocal_approx_fast (~18 correct bits).
"""
import math

import numpy as np
import ml_dtypes

import concourse.bass as bass
import concourse.mybir as mybir
from concourse.tile import TileContext
from concourse import bass_utils

# ---------------------------------------------------------------------------
# Walrus workaround: this container's walrus accepts at most ONE sync-wait
# per TPB instruction, but Tile attaches several (tail Drain, LDWEIGHTS...).
# Split: keep the last wait, move the rest onto preceding same-engine NOPs.
# ---------------------------------------------------------------------------
import concourse.tile as _tile_mod

_orig_sched = _tile_mod.TileContext.schedule_and_allocate
_nopctr = [0]


def _split_multiwait(nc):
    for fn in nc.m.functions:
        for blk in fn.blocks:
            insts = blk.instructions
            if not any(
                i.sync_info and i.sync_info.on_wait and len(i.sync_info.on_wait) > 1
                for i in insts
            ):
                continue
            out = []
            for ins in insts:
                si = ins.sync_info
                if si and si.on_wait and len(si.on_wait) > 1:
                    waits = list(si.on_wait)
                    for w in waits[:-1]:
                        _nopctr[0] += 1
                        nop = mybir.InstNoOp(name=f"I-mws-{_nopctr[0]}", ins=[], outs=[])
                        nop.engine = ins.engine
                        nop.sync_info = mybir.SyncInfo(on_wait=[w], on_update=[])
                        out.append(nop)
                    ins.sync_info = mybir.SyncInfo(
                        on_wait=[waits[-1]], on_update=list(si.on_update or [])
                    )
                out.append(ins)
            blk.instructions = out


def _patched_sched(self, *a, **k):
    res = _orig_sched(self, *a, **k)
    _split_multiwait(self.nc)
    return res


if getattr(_tile_mod.TileContext.schedule_and_allocate, "__name__", "") != "_patched_sched":
    _tile_mod.TileContext.schedule_and_allocate = _patched_sched


# ---------------------------------------------------------------------------
T, D, H = 2048, 5120, 32
NOPE, ROPE, QK = 128, 64, 192
KVR, QR, VH = 512, 1536, 128
EPS, THETA = 1e-6, 10000.0
NCORES = 8
HL = H // NCORES          # 4 heads per core
TC = T // NCORES          # 256 tokens per core
LAT = KVR + ROPE          # 576
DCOL = D // NCORES        # 640 output columns per core

F32 = mybir.dt.float32
F32R = mybir.dt.float32r
BF16 = mybir.dt.bfloat16
AF = mybir.ActivationFunctionType
MUL = mybir.AluOpType.mult
ADD = mybir.AluOpType.add
SUB = mybir.AluOpType.subtract

TRACE = [False]          # test.py sets TRACE[0]=True to profile
LAST_RESULT = [None]     # BassKernelResults stashed here for test.py

_cache = {}


def _rms_scale(nc, a_tmp, a_ps, ss, nfeat, ones_r, key):
    """1/sqrt(mean(ss)+eps) broadcast to [128, TC] f32r."""
    ms = a_tmp.tile([1, TC], F32, name=f"ms_{key}", tag="ms")
    nc.vector.tensor_scalar(ms[:], ss[:], 1.0 / nfeat, EPS, op0=MUL, op1=ADD)
    sm = a_tmp.tile([1, TC], F32R, name=f"sm_{key}", tag="sm")
    nc.scalar.activation(sm[:], ms[:], AF.Sqrt)
    rs = a_tmp.tile([1, TC], F32R, name=f"rs_{key}", tag="rs")
    with nc.allow_low_precision(reason="f32r holds full fp32 bits"):
        nc.vector.reciprocal(rs[:], sm[:])
    bps = a_ps.tile([128, TC], F32, name=f"bps_{key}", tag="bps")
    nc.tensor.matmul(bps[:], ones_r[:1, :], rs[:], start=True, stop=True)
    bc = a_tmp.tile([128, TC], F32R, name=f"bc_{key}", tag=f"bc{key}")
    nc.vector.tensor_copy(bc[:], bps[:])
    return bc


def _phase_a(nc, tc, io, consts_t, ag1kv_in, ag1q_in, ag1kv_out, ag1q_out):
    """Token-split A projections, kv-first so its allgather ships early."""
    ones_c, ones_r = consts_t["ones_c"], consts_t["ones_r"]
    cosa_sb, sina_sb, bias_sb = (consts_t["cosa_sb"], consts_t["sina_sb"],
                                 consts_t["bias_sb"])
    with (
        tc.tile_pool(name="a_ht", bufs=1) as a_ht,
        tc.tile_pool(name="a_w", bufs=3) as a_w,
        tc.tile_pool(name="a_st", bufs=1) as a_st,
        tc.tile_pool(name="a_tmp", bufs=3) as a_tmp,
        tc.tile_pool(name="a_ps", bufs=2, space="PSUM") as a_ps,
        tc.tile_pool(name="a_ss", bufs=1, space="PSUM") as a_ss,
    ):
        ht_sb = a_ht.tile([128, 40 * TC], BF16, name="ht_sb")
        htv = ht_sb[:].rearrange("p (k t) -> p k t", k=40)
        nc.sync.dma_start(htv, io["hT"][:].rearrange("(k p) t -> p k t", p=128))
        stage = a_st.tile([128, 17 * TC], F32R, name="stage")
        ss_q = a_ss.tile([1, TC], F32, name="ss_q")
        ss_kv = a_ss.tile([1, TC], F32, name="ss_kv")

        def mchunk(m):
            mrows = 64 if m == 16 else 128
            wt = a_w.tile([128, 40 * 128], BF16, name=f"a_w_{m}", tag="aw")
            wtv = wt[:].rearrange("p (k c) -> p k c", k=40)
            nc.sync.dma_start(wtv, io["wa"][m])
            ps = a_ps.tile([128, TC], F32, name=f"a_ps_{m}", tag="aps")
            for k in range(40):
                nc.tensor.matmul(ps[:mrows, :], wtv[:, k, :mrows], htv[:, k, :],
                                 start=(k == 0), stop=(k == 39))
            st = stage[:, m * TC:(m + 1) * TC]
            if m < 12:
                nc.vector.tensor_copy(st, ps[:])
                sq = a_tmp.tile([128, TC], F32R, name=f"sq_{m}", tag="sq")
                nc.scalar.activation(sq[:], st, AF.Square)
                nc.tensor.matmul(ss_q[:], ones_c, sq[:],
                                 start=(m == 0), stop=(m == 11))
            elif m < 16:
                nc.vector.tensor_scalar(st, ps[:], bias_sb[:, m - 12:m - 11],
                                        None, op0=ADD)
                sq = a_tmp.tile([128, TC], F32R, name=f"sq_{m}", tag="sq")
                nc.scalar.activation(sq[:], st, AF.Square)
                nc.tensor.matmul(ss_kv[:], ones_c, sq[:],
                                 start=(m == 12), stop=(m == 15))
            else:
                nc.vector.tensor_scalar(st[:64, :], ps[:64, :],
                                        bias_sb[:64, 4:5], None, op0=ADD)

        # --- kv chunks first ---
        for m in range(12, 17):
            mchunk(m)
        bc_kv = _rms_scale(nc, a_tmp, a_ps, ss_kv, KVR, ones_r, "kv")
        for i in range(4):
            st = stage[:, (12 + i) * TC:(13 + i) * TC]
            sc = a_tmp.tile([128, TC], BF16, name=f"sckv_{i}", tag="sc")
            nc.vector.tensor_tensor(sc[:], st, bc_kv[:], op=MUL)
            nc.sync.dma_start(ag1kv_in[i * 128:(i + 1) * 128, :], sc[:])
        # k_pe rope (no norm) -> rows 512:576
        st = stage[:, 16 * TC:17 * TC]
        rp = a_tmp.tile([64, TC], BF16, name="rp_kpe")
        t1 = a_tmp.tile([32, TC], F32R, name="rt1", tag="rt1")
        t2 = a_tmp.tile([32, TC], F32R, name="rt2", tag="rt2")
        x1, x2 = st[0:32, :], st[32:64, :]
        nc.vector.tensor_tensor(t1[:], x1, cosa_sb[0:32, :], op=MUL)
        nc.vector.tensor_tensor(t2[:], x2, sina_sb[32:64, :], op=MUL)
        nc.vector.tensor_tensor(rp[0:32, :], t1[:], t2[:], op=SUB)
        nc.vector.tensor_tensor(t1[:], x1, sina_sb[0:32, :], op=MUL)
        nc.vector.tensor_tensor(t2[:], x2, cosa_sb[32:64, :], op=MUL)
        nc.vector.tensor_tensor(rp[32:64, :], t1[:], t2[:], op=ADD)
        nc.sync.dma_start(ag1kv_in[512:576, :], rp[:])

        nc.gpsimd.collective_compute(
            "AllGather", mybir.AluOpType.bypass,
            ins=[ag1kv_in[:]], outs=[ag1kv_out[:]],
            replica_groups=[list(range(NCORES))],
        )

        # --- q chunks ---
        for m in range(12):
            mchunk(m)
        bc_q = _rms_scale(nc, a_tmp, a_ps, ss_q, QR, ones_r, "q")
        for m in range(12):
            st = stage[:, m * TC:(m + 1) * TC]
            sc = a_tmp.tile([128, TC], BF16, name=f"scq_{m}", tag="sc")
            nc.vector.tensor_tensor(sc[:], st, bc_q[:], op=MUL)
            nc.sync.dma_start(ag1q_in[m * 128:(m + 1) * 128, :], sc[:])

        nc.gpsimd.collective_compute(
            "AllGather", mybir.AluOpType.bypass,
            ins=[ag1q_in[:]], outs=[ag1q_out[:]],
            replica_groups=[list(range(NCORES))],
        )


def _phase_b(nc, tc, io, ag1kv_out, ktv, vv, kpe_sb):
    """Head-split k_nope^T and v projections from the gathered kv latents."""
    ag1kv_v = ag1kv_out[:].rearrange("(r a) t -> a r t", a=LAT)
    with (
        tc.tile_pool(name="b_kva", bufs=1) as b_kva,
        tc.tile_pool(name="b_w", bufs=2) as b_w,
        tc.tile_pool(name="b_ps", bufs=3, space="PSUM") as b_ps,
    ):
        kva_sb = b_kva.tile([128, 4 * T], BF16, name="kva_sb")
        kvav = kva_sb[:].rearrange("p (k t) -> p k t", k=4)
        for k in range(4):
            nc.sync.dma_start(
                kvav[:, k, :].rearrange("p (r t) -> p r t", r=NCORES),
                ag1kv_v[k * 128:(k + 1) * 128])
        nc.sync.dma_start(
            kpe_sb[:].rearrange("p (r t) -> p r t", r=NCORES),
            ag1kv_v[512:576])

        wk_sb = b_w.tile([128, 4 * 512], BF16, name="wk_sb", tag="wkw")
        wkv_ = wk_sb[:].rearrange("p (k c) -> p k c", k=4)
        nc.sync.dma_start(wkv_, io["wkvbk"][:])
        for j in range(HL):
            for qb in range(4):
                ps = b_ps.tile([128, 512], F32, name=f"psk_{j}_{qb}", tag="psk")
                for k in range(4):
                    nc.tensor.matmul(ps[:], wkv_[:, k, j * 128:(j + 1) * 128],
                                     kvav[:, k, qb * 512:(qb + 1) * 512],
                                     start=(k == 0), stop=(k == 3))
                nc.scalar.activation(ktv[:, j, qb * 512:(qb + 1) * 512], ps[:],
                                     AF.Copy)

        wv_sb = b_w.tile([128, 4 * 512], BF16, name="wv_sb", tag="wvw")
        wvv = wv_sb[:].rearrange("p (k c) -> p k c", k=4)
        nc.sync.dma_start(wvv, io["wkvbv"][:])
        for mt in range(16):
            ps = b_ps.tile([128, 512], F32, name=f"psv_{mt}", tag="psv")
            for k in range(4):
                nc.tensor.matmul(ps[:], kvav[:, k, mt * 128:(mt + 1) * 128],
                                 wvv[:, k, :], start=(k == 0), stop=(k == 3))
            nc.scalar.activation(vv[:, mt, :], ps[:], AF.Copy)


def _phase_q(nc, tc, io, ag1q_out, qtn, qpe, consts_t):
    """Head-split q^T projection; rope via full-tile ops + PE half-swap."""
    pswap_b = consts_t["pswap_b"]
    with (
        tc.tile_pool(name="c_qa", bufs=1) as c_qa,
        tc.tile_pool(name="c_tab", bufs=1) as c_tab,
        tc.tile_pool(name="c_w", bufs=1) as c_w,
        tc.tile_pool(name="c_tmp", bufs=2) as c_tmp,
        tc.tile_pool(name="c_ps", bufs=3, space="PSUM") as c_ps,
        tc.tile_pool(name="c_sw", bufs=2, space="PSUM") as c_sw,
    ):
        cos_sb = c_tab.tile([128, T], F32R, name="cos_sb")
        sin_sb = c_tab.tile([128, T], F32R, name="sin_sb")
        nc.sync.dma_start(cos_sb[:], io["cosC"][:])
        nc.sync.dma_start(sin_sb[:], io["sinS"][:])
        qa_sb = c_qa.tile([128, 12 * T], BF16, name="qa_sb")
        qav = qa_sb[:].rearrange("p (k t) -> p k t", k=12)
        ag1q_v = ag1q_out[:].rearrange("(r a) t -> a r t", a=QR)
        for k in range(12):
            nc.sync.dma_start(
                qav[:, k, :].rearrange("p (r t) -> p r t", r=NCORES),
                ag1q_v[k * 128:(k + 1) * 128])
        wq_sb = c_w.tile([128, 6 * 12 * 128], BF16, name="wq_sb")
        wqv = wq_sb[:].rearrange("p (m k c) -> p m k c", m=6, k=12)
        for m in range(6):
            nc.sync.dma_start(wqv[:, m], io["wqb"][m])

        for qb in range(4):
            cols = slice(qb * 512, (qb + 1) * 512)
            for m in range(6):
                ps = c_ps.tile([128, 512], F32, name=f"psq_{qb}_{m}", tag="psq")
                for k in range(12):
                    nc.tensor.matmul(ps[:], wqv[:, m, k, :],
                                     qav[:, k, cols],
                                     start=(k == 0), stop=(k == 11))
                if m < 4:
                    nc.scalar.activation(qtn[m][:, cols], ps[:], AF.Copy)
                else:
                    cs = c_tmp.tile([128, 512], BF16, name=f"cs_{qb}_{m}",
                                    tag="cs")
                    nc.scalar.activation(cs[:], ps[:], AF.Copy)
                    sw = c_sw.tile([128, 512], F32, name=f"sw_{qb}_{m}",
                                   tag="sw")
                    nc.tensor.matmul(sw[:], pswap_b[:], cs[:],
                                     start=True, stop=True)
                    t1 = c_tmp.tile([128, 512], F32R, name=f"t1_{qb}_{m}",
                                    tag="t1")
                    t2 = c_tmp.tile([128, 512], F32R, name=f"t2_{qb}_{m}",
                                    tag="t2")
                    nc.vector.tensor_tensor(t1[:], ps[:], cos_sb[:, cols],
                                            op=MUL)
                    nc.vector.tensor_tensor(t2[:], sw[:], sin_sb[:, cols],
                                            op=MUL)
                    jj = 2 * (m - 4)
                    nc.vector.tensor_tensor(qpe[jj][:, cols], t1[0:64, :],
                                            t2[0:64, :], op=ADD)
                    nc.vector.tensor_tensor(qpe[jj + 1][:, cols], t1[64:128, :],
                                            t2[64:128, :], op=ADD)


def _attn_out(nc, tc, io, ag2_ins, ag2_outs, qtn, qpe, ktv, vv, kpe_sb,
              consts_t, wov):
    """Causal attention per qb + interleaved output projection (tq=qb-1)."""
    ones_cb, ones_r, tri_b = (consts_t["ones_cb"], consts_t["ones_r"],
                              consts_t["tri_b"])
    with (
        tc.tile_pool(name="t_pt", bufs=2) as t_pt,
        tc.tile_pool(name="t_fin", bufs=2) as t_fin,
        tc.tile_pool(name="t_ring", bufs=1, space="PSUM") as t_ring,
        tc.tile_pool(name="t_ots", bufs=2, space="PSUM") as t_ots,
        tc.tile_pool(name="t_dpo", bufs=2, space="PSUM") as t_dpo,
        tc.tile_pool(name="o_oa", bufs=1) as o_oa,
        tc.tile_pool(name="o_st", bufs=2) as o_st,
    ):
        ring = t_ring.tile([128, 2048], F32, name="ring")

        def out_block(tq):
            oa = o_oa.tile([128, 32 * 512], BF16, name=f"oa_{tq}", tag="oa")
            oav = oa[:].rearrange("p (k t) -> p k t", k=32)
            nc.sync.dma_start(
                oav, ag2_outs[tq][:].rearrange("(k p) t -> p k t", p=128))
            for d in range(5):
                ps = t_dpo.tile([128, 512], F32, name=f"ops_{tq}_{d}",
                                tag="dpo")
                for k in range(32):
                    nc.tensor.matmul(ps[:], wov[:, d, k, :], oav[:, k, :],
                                     start=(k == 0), stop=(k == 31))
                st = o_st.tile([128, 512], F32, name=f"ost_{tq}_{d}",
                               tag="ost")
                nc.vector.tensor_copy(st[:], ps[:])
                nc.sync.dma_start(
                    io["outT"][d * 128:(d + 1) * 128,
                               tq * 512:(tq + 1) * 512], st[:])

        for qb in range(4):
            kmax = 4 * qb + 4
            cols = slice(qb * 512, (qb + 1) * 512)
            for j in range(HL):
                qfn = qtn[j][:, cols]
                qfp = qpe[j][:, cols]
                ots = t_ots.tile([128, 512], F32, name=f"ot_{qb}_{j}",
                                 tag="ots")
                den = t_dpo.tile([1, 512], F32, name=f"den_{qb}_{j}",
                                 tag="dpo")
                for p in range(kmax // 2):
                    kk0 = 2 * p
                    off = (p % 2) * 1024
                    for kk in (kk0, kk0 + 1):
                        c0 = max(0, kk - 4 * qb) * 128
                        sl = ring[:, off + (kk % 2) * 512 + c0:
                                  off + (kk % 2) * 512 + 512]
                        nc.tensor.matmul(sl,
                                         ktv[:, j, kk * 128:(kk + 1) * 128],
                                         qfn[:, c0:512], start=True, stop=False)
                        nc.tensor.matmul(sl,
                                         kpe_sb[:, kk * 128:(kk + 1) * 128],
                                         qfp[:, c0:512], start=False, stop=True)
                    pt = t_pt.tile([128, 1024], BF16, name=f"pt_{qb}_{j}_{p}",
                                   tag="pt")
                    c0f = max(0, kk0 - 4 * qb) * 128
                    nc.scalar.activation(pt[:, c0f:1024],
                                         ring[:, off + c0f:off + 1024], AF.Exp)
                    for kk in (kk0, kk0 + 1):
                        o = kk - 4 * qb
                        if o >= 0:
                            d0 = (kk % 2) * 512 + o * 128
                            nc.vector.tensor_tensor(pt[:, d0:d0 + 128],
                                                    pt[:, d0:d0 + 128],
                                                    tri_b[:], op=MUL)
                    for kk in (kk0, kk0 + 1):
                        c0 = max(0, kk - 4 * qb) * 128
                        psl = pt[:, (kk % 2) * 512 + c0:(kk % 2) * 512 + 512]
                        nc.tensor.matmul(den[:, c0:512], ones_cb, psl,
                                         start=(kk == 0), stop=(kk == kmax - 1))
                        nc.tensor.matmul(ots[:, c0:512],
                                         vv[:, kk, j * 128:(j + 1) * 128],
                                         psl,
                                         start=(kk == 0), stop=(kk == kmax - 1))
                rden = t_fin.tile([1, 512], F32R, name=f"rd_{qb}_{j}", tag="rd")
                with nc.allow_low_precision(reason="f32r = fp32 bits"):
                    nc.vector.reciprocal(rden[:], den[:])
                bcp = t_dpo.tile([128, 512], F32, name=f"bcp_{qb}_{j}",
                                 tag="dpo")
                nc.tensor.matmul(bcp[:], ones_r[:1, :], rden[:],
                                 start=True, stop=True)
                bcs = t_fin.tile([128, 512], F32R, name=f"bcs_{qb}_{j}",
                                 tag="bcs")
                nc.scalar.activation(bcs[:], bcp[:], AF.Copy)
                obf = t_fin.tile([128, 512], BF16, name=f"obf_{qb}_{j}",
                                 tag="obf")
                nc.vector.tensor_tensor(obf[:], ots[:], bcs[:], op=MUL)
                nc.sync.dma_start(
                    ag2_ins[qb][j * 128:(j + 1) * 128, :], obf[:])
            nc.gpsimd.collective_compute(
                "AllGather", mybir.AluOpType.bypass,
                ins=[ag2_ins[qb][:]], outs=[ag2_outs[qb][:]],
                replica_groups=[list(range(NCORES))],
            )
            if qb >= 1:
                out_block(qb - 1)
        out_block(3)


def _build():
    nc = bass.Bass("TRN2", target_bir_lowering=False, debug=False,
                   num_devices=NCORES)
    io = {
        "hT": nc.dram_tensor("hT", [D, TC], BF16, kind="ExternalInput"),
        "wa": nc.dram_tensor("wa", [17, 128, 40, 128], BF16,
                             kind="ExternalInput"),
        "biask": nc.dram_tensor("biask", [128, 5], F32, kind="ExternalInput"),
        "wqb": nc.dram_tensor("wqb", [6, 128, 12, 128], BF16,
                              kind="ExternalInput"),
        "wkvbk": nc.dram_tensor("wkvbk", [128, 4, 512], BF16,
                                kind="ExternalInput"),
        "wkvbv": nc.dram_tensor("wkvbv", [128, 4, 512], BF16,
                                kind="ExternalInput"),
        "wo": nc.dram_tensor("wo", [5, 128, 32, 128], BF16,
                             kind="ExternalInput"),
        "cosC": nc.dram_tensor("cosC", [128, T], F32R, kind="ExternalInput"),
        "sinS": nc.dram_tensor("sinS", [128, T], F32R, kind="ExternalInput"),
        "cosA": nc.dram_tensor("cosA", [128, TC], F32R, kind="ExternalInput"),
        "sinA": nc.dram_tensor("sinA", [128, TC], F32R, kind="ExternalInput"),
        "tri": nc.dram_tensor("tri", [128, 128], F32R, kind="ExternalInput"),
        "onesin": nc.dram_tensor("onesin", [128, 128], F32R,
                                 kind="ExternalInput"),
        "pswap": nc.dram_tensor("pswap", [128, 128], BF16,
                                kind="ExternalInput"),
        "outT": nc.dram_tensor("outT", [DCOL, T], F32, kind="ExternalOutput"),
    }

    with TileContext(nc) as tc:
        with (
            tc.tile_pool(name="dram", bufs=1, space="DRAM") as dram,
            tc.tile_pool(name="consts", bufs=1) as consts,
        ):
            ag1kv_in = dram.tile([LAT, TC], BF16, name="ag1kv_in")
            ag1kv_out = dram.tile([NCORES * LAT, TC], BF16, addr_space="Shared",
                                  name="ag1kv_out")
            ag1q_in = dram.tile([QR, TC], BF16, name="ag1q_in")
            ag1q_out = dram.tile([NCORES * QR, TC], BF16, addr_space="Shared",
                                 name="ag1q_out")
            ag2_ins = [dram.tile([HL * VH, 512], BF16, name=f"ag2_in_{qb}")
                       for qb in range(4)]
            ag2_outs = [dram.tile([H * VH, 512], BF16, addr_space="Shared",
                                  name=f"ag2_out_{qb}") for qb in range(4)]

            consts_t = {}
            ones_sb = consts.tile([128, 128], F32R, name="ones_sb")
            nc.sync.dma_start(ones_sb[:], io["onesin"][:])
            consts_t["ones_c"] = ones_sb[:, 0:1]
            consts_t["ones_r"] = ones_sb
            ones_b = consts.tile([128, 1], BF16, name="ones_b")
            nc.vector.tensor_copy(ones_b[:], ones_sb[:, 0:1])
            consts_t["ones_cb"] = ones_b[:]
            trib = consts.tile([128, 128], BF16, name="trib")
            consts_t["tri_b"] = trib
            for nm, srcn, shp in (("tri_sb", "tri", [128, 128]),
                                  ("cosa_sb", "cosA", [128, TC]),
                                  ("sina_sb", "sinA", [128, TC]),
                                  ):
                consts_t[nm] = consts.tile(shp, F32R, name=nm)
                nc.sync.dma_start(consts_t[nm][:], io[srcn][:])
            consts_t["bias_sb"] = consts.tile([128, 5], F32, name="bias_sb")
            nc.sync.dma_start(consts_t["bias_sb"][:], io["biask"][:])
            nc.vector.tensor_copy(trib[:], consts_t["tri_sb"][:])
            pswap_b = consts.tile([128, 128], BF16, name="pswap_b")
            nc.sync.dma_start(pswap_b[:], io["pswap"][:])
            consts_t["pswap_b"] = pswap_b

            with nc.named_scope("phase_a"):
                _phase_a(nc, tc, io, consts_t, ag1kv_in, ag1q_in,
                         ag1kv_out, ag1q_out)

            with tc.tile_pool(name="persist", bufs=1) as persist:
                kt_sb = persist.tile([128, HL * T], BF16, name="kt_sb")
                ktv = kt_sb[:].rearrange("p (j t) -> p j t", j=HL)
                v_sb = persist.tile([128, 16 * 512], BF16, name="v_sb")
                vv = v_sb[:].rearrange("p (mt c) -> p mt c", mt=16)
                kpe_sb = persist.tile([64, T], BF16, name="kpe_sb")
                qtn = [persist.tile([128, T], BF16, name=f"qtn_{m}")
                       for m in range(4)]
                qpe = [persist.tile([64, T], BF16, name=f"qpe_{j}")
                       for j in range(4)]
                with nc.named_scope("phase_b"):
                    _phase_b(nc, tc, io, ag1kv_out, ktv, vv, kpe_sb)
                with nc.named_scope("phase_q"):
                    _phase_q(nc, tc, io, ag1q_out, qtn, qpe, consts_t)
                with tc.tile_pool(name="wo_pool", bufs=1) as wo_pool:
                    wo_sb = wo_pool.tile([128, 5 * 32 * 128], BF16,
                                         name="wo_sb")
                    wov = wo_sb[:].rearrange("p (d k c) -> p d k c", d=5, k=32)
                    for d in range(5):
                        nc.sync.dma_start(wov[:, d], io["wo"][d])
                    with nc.named_scope("phase_attn"):
                        _attn_out(nc, tc, io, ag2_ins, ag2_outs, qtn, qpe,
                                  ktv, vv, kpe_sb, consts_t, wov)
    return nc


def _get_nc():
    if "nc" not in _cache:
        _cache["nc"] = _build()
    return _cache["nc"]


def _prep(inputs):
    h = np.asarray(inputs["h"], np.float32)
    pos = np.asarray(inputs["position_ids"], np.int32)
    Wq_a = np.asarray(inputs["Wq_a"], np.float32)
    gq = np.asarray(inputs["gq"], np.float32)
    Wq_b = np.asarray(inputs["Wq_b"], np.float32)
    Wkv_a = np.asarray(inputs["Wkv_a"], np.float32)
    bkv_a = np.asarray(inputs["bkv_a"], np.float32)
    gkv = np.asarray(inputs["gkv"], np.float32)
    Wkv_b = np.asarray(inputs["Wkv_b"], np.float32)
    Wo = np.asarray(inputs["Wo"], np.float32)

    bf16 = ml_dtypes.bfloat16
    dperm = np.concatenate([np.arange(0, ROPE, 2), np.arange(1, ROPE, 2)])
    scale = np.float32(1.0 / math.sqrt(QK))

    hT = np.ascontiguousarray(h.T)                      # [D, T]
    wkva = Wkv_a.copy()
    wkva[:, KVR:] = Wkv_a[:, KVR + dperm]
    bias = bkv_a.copy()
    bias[KVR:] = bkv_a[KVR + dperm]
    bm = np.zeros((5, 128), np.float32)
    bm.reshape(-1)[:LAT] = bias
    biask = np.ascontiguousarray(bm.T)                  # [128, 5]

    wqb_eff = (Wq_b * gq[:, None]) * scale              # [QR, H*QK]
    wkvb_eff = Wkv_b * gkv[:, None]                     # [KVR, H*(NOPE+VH)]

    # A weights pre-tiled: wa[m, p, k, c] = Wcat[k*128+p, m*128+c]
    Wcat = np.concatenate([Wq_a, wkva], axis=1)          # [D, 2112]
    A = Wcat.reshape(40, 128, QR + LAT)
    wa = np.zeros((17, 128, 40, 128), np.float32)
    for m in range(17):
        cw = min(128, QR + LAT - m * 128)
        wa[m, :, :, :cw] = A[:, :, m * 128:m * 128 + cw].transpose(1, 0, 2)
    wa = wa.astype(bf16)

    inv = THETA ** (-np.arange(0, ROPE, 2, dtype=np.float32) / ROPE)
    fr = pos.astype(np.float32)[:, None] * inv[None, :]  # [T, 32]
    cosT = np.ascontiguousarray(np.tile(np.cos(fr).T, (4, 1)))  # [128, T]
    sinT = np.ascontiguousarray(np.tile(np.sin(fr).T, (4, 1)))
    sgn = np.repeat(np.array([-1.0, 1.0, -1.0, 1.0], np.float32), 32)[:, None]
    sinS = np.ascontiguousarray(sinT * sgn)
    tri = np.triu(np.ones((128, 128), np.float32))
    pswap = np.zeros((128, 128), np.float32)
    for i in range(128):
        pswap[i ^ 32, i] = 1.0
    pswap = np.ascontiguousarray(pswap).astype(bf16)

    in_maps = []
    for c in range(NCORES):
        heads = list(range(HL * c, HL * (c + 1)))
        qcols = [np.arange(hh * QK, hh * QK + NOPE) for hh in heads]
        for pair in range(2):
            for hh in heads[2 * pair:2 * pair + 2]:
                qcols.append(hh * QK + NOPE + dperm)
        kcols = np.concatenate(
            [np.arange(hh * (NOPE + VH), hh * (NOPE + VH) + NOPE)
             for hh in heads])
        vcols = np.concatenate(
            [np.arange(hh * (NOPE + VH) + NOPE, (hh + 1) * (NOPE + VH))
             for hh in heads])

        wqb_c = wqb_eff[:, np.concatenate(qcols)]        # [1536, 768]
        wqb_t = np.ascontiguousarray(
            wqb_c.reshape(12, 128, 6, 128).transpose(2, 1, 0, 3)).astype(bf16)
        wkvbk_t = np.ascontiguousarray(
            wkvb_eff[:, kcols].reshape(4, 128, 512).transpose(1, 0, 2)
        ).astype(bf16)
        wkvbv_t = np.ascontiguousarray(
            wkvb_eff[:, vcols].reshape(4, 128, 512).transpose(1, 0, 2)
        ).astype(bf16)
        wo_c = Wo[:, c * DCOL:(c + 1) * DCOL]            # [4096, 640]
        wo_t = np.ascontiguousarray(
            wo_c.reshape(32, 128, 5, 128).transpose(2, 1, 0, 3)).astype(bf16)

        in_maps.append({
            "hT": np.ascontiguousarray(hT[:, c * TC:(c + 1) * TC]).astype(bf16),
            "wa": wa,
            "biask": biask,
            "wqb": wqb_t,
            "wkvbk": wkvbk_t,
            "wkvbv": wkvbv_t,
            "wo": wo_t,
            "cosC": cosT,
            "sinS": sinS,
            "cosA": np.ascontiguousarray(cosT[:, c * TC:(c + 1) * TC]),
            "sinA": np.ascontiguousarray(sinT[:, c * TC:(c + 1) * TC]),
            "tri": tri,
            "onesin": np.ones((128, 128), np.float32),
            "pswap": pswap,
        })
    return in_maps


def kernel(**inputs):
    nc = _get_nc()
    in_maps = _prep(inputs)
    res = bass_utils.run_bass_kernel_spmd(
        nc, in_maps, core_ids=list(range(NCORES)), trace=TRACE[0])
    LAST_RESULT[0] = res
    out = np.empty((T, D), np.float32)
    for c in range(NCORES):
        out[:, c * DCOL:(c + 1) * DCOL] = res.results[c]["outT"].T
    return out


# revision 15
# speedup vs baseline: 1.1165x; 1.0890x over previous
"""DeepseekV2 MLA attention on 8 Trainium2 NeuronCores.

Sharding: token-split A projections -> AllGather(kv latents) early +
AllGather(q latents) -> head-split (4 heads/core) B projections + causal
attention -> per-qb AllGather(attn out) with the D-column-split output
projection interleaved one block behind the attention loop.

Layouts are d-major (feature dim on the SBUF partition axis); the host
pre-transposes h and pre-tiles every weight so each SBUF weight chunk is
one contiguous DMA. q tiles stay SBUF-resident (no DRAM spill).

Precision: bf16 matmul inputs with fp32 PSUM accumulation; rmsnorm
statistics and softmax run in fp32/f32r; softmax reciprocal via
reciprocal_approx_fast (~18 correct bits).
"""
import math

import numpy as np
import ml_dtypes

import concourse.bass as bass
import concourse.mybir as mybir
from concourse.tile import TileContext
from concourse import bass_utils

# ---------------------------------------------------------------------------
# Walrus workaround: this container's walrus accepts at most ONE sync-wait
# per TPB instruction, but Tile attaches several (tail Drain, LDWEIGHTS...).
# Split: keep the last wait, move the rest onto preceding same-engine NOPs.
# ---------------------------------------------------------------------------
import concourse.tile as _tile_mod

_orig_sched = _tile_mod.TileContext.schedule_and_allocate
_nopctr = [0]


def _split_multiwait(nc):
    for fn in nc.m.functions:
        for blk in fn.blocks:
            insts = blk.instructions
            if not any(
                i.sync_info and i.sync_info.on_wait and len(i.sync_info.on_wait) > 1
                for i in insts
            ):
                continue
            out = []
            for ins in insts:
                si = ins.sync_info
                if si and si.on_wait and len(si.on_wait) > 1:
                    waits = list(si.on_wait)
                    for w in waits[:-1]:
                        _nopctr[0] += 1
                        nop = mybir.InstNoOp(name=f"I-mws-{_nopctr[0]}", ins=[], outs=[])
                        nop.engine = ins.engine
                        nop.sync_info = mybir.SyncInfo(on_wait=[w], on_update=[])
                        out.append(nop)
                    ins.sync_info = mybir.SyncInfo(
                        on_wait=[waits[-1]], on_update=list(si.on_update or [])
                    )
                out.append(ins)
            blk.instructions = out


def _patched_sched(self, *a, **k):
    res = _orig_sched(self, *a, **k)
    _split_multiwait(self.nc)
    return res


if getattr(_tile_mod.TileContext.schedule_and_allocate, "__name__", "") != "_patched_sched":
    _tile_mod.TileContext.schedule_and_allocate = _patched_sched


# ---------------------------------------------------------------------------
T, D, H = 2048, 5120, 32
NOPE, ROPE, QK = 128, 64, 192
KVR, QR, VH = 512, 1536, 128
EPS, THETA = 1e-6, 10000.0
NCORES = 8
HL = H // NCORES          # 4 heads per core
TC = T // NCORES          # 256 tokens per core
LAT = KVR + ROPE          # 576
DCOL = D // NCORES        # 640 output columns per core

F32 = mybir.dt.float32
F32R = mybir.dt.float32r
BF16 = mybir.dt.bfloat16
AF = mybir.ActivationFunctionType
MUL = mybir.AluOpType.mult
ADD = mybir.AluOpType.add
SUB = mybir.AluOpType.subtract

TRACE = [False]          # test.py sets TRACE[0]=True to profile
LAST_RESULT = [None]     # BassKernelResults stashed here for test.py

_cache = {}


def _rms_scale(nc, a_tmp, a_ps, ss, nfeat, ones_r, key):
    """1/sqrt(mean(ss)+eps) broadcast to [128, TC] f32r."""
    ms = a_tmp.tile([1, TC], F32, name=f"ms_{key}", tag="ms")
    nc.vector.tensor_scalar(ms[:], ss[:], 1.0 / nfeat, EPS, op0=MUL, op1=ADD)
    sm = a_tmp.tile([1, TC], F32R, name=f"sm_{key}", tag="sm")
    nc.scalar.activation(sm[:], ms[:], AF.Sqrt)
    rs = a_tmp.tile([1, TC], F32R, name=f"rs_{key}", tag="rs")
    with nc.allow_low_precision(reason="f32r holds full fp32 bits"):
        nc.vector.reciprocal(rs[:], sm[:])
    bps = a_ps.tile([128, TC], F32, name=f"bps_{key}", tag="bps")
    nc.tensor.matmul(bps[:], ones_r[:1, :], rs[:], start=True, stop=True)
    bc = a_tmp.tile([128, TC], F32R, name=f"bc_{key}", tag=f"bc{key}")
    nc.vector.tensor_copy(bc[:], bps[:])
    return bc


def _phase_a(nc, tc, io, consts_t, ag1kv_in, ag1q_in, ag1kv_out, ag1q_out,
             ht_sb):
    """Token-split A projections, kv-first so its allgather ships early."""
    ones_c, ones_r = consts_t["ones_c"], consts_t["ones_r"]
    cosa_sb, sina_sb, bias_sb = (consts_t["cosa_sb"], consts_t["sina_sb"],
                                 consts_t["bias_sb"])
    with (
        tc.tile_pool(name="a_w", bufs=3) as a_w,
        tc.tile_pool(name="a_st", bufs=1) as a_st,
        tc.tile_pool(name="a_tmp", bufs=3) as a_tmp,
        tc.tile_pool(name="a_ps", bufs=2, space="PSUM") as a_ps,
        tc.tile_pool(name="a_ss", bufs=1, space="PSUM") as a_ss,
    ):
        htv = ht_sb[:].rearrange("p (k t) -> p k t", k=40)
        stage = a_st.tile([128, 17 * TC], F32R, name="stage")
        ss_q = a_ss.tile([1, TC], F32, name="ss_q")
        ss_kv = a_ss.tile([1, TC], F32, name="ss_kv")

        def mchunk(m):
            mrows = 64 if m == 16 else 128
            wt = a_w.tile([128, 40 * 128], BF16, name=f"a_w_{m}", tag="aw")
            wtv = wt[:].rearrange("p (k c) -> p k c", k=40)
            nc.sync.dma_start(wtv, io["wa"][m])
            ps = a_ps.tile([128, TC], F32, name=f"a_ps_{m}", tag="aps")
            for k in range(40):
                nc.tensor.matmul(ps[:mrows, :], wtv[:, k, :mrows], htv[:, k, :],
                                 start=(k == 0), stop=(k == 39))
            st = stage[:, m * TC:(m + 1) * TC]
            if m < 12:
                nc.vector.tensor_copy(st, ps[:])
                sq = a_tmp.tile([128, TC], F32R, name=f"sq_{m}", tag="sq")
                nc.scalar.activation(sq[:], st, AF.Square)
                nc.tensor.matmul(ss_q[:], ones_c, sq[:],
                                 start=(m == 0), stop=(m == 11))
            elif m < 16:
                nc.vector.tensor_scalar(st, ps[:], bias_sb[:, m - 12:m - 11],
                                        None, op0=ADD)
                sq = a_tmp.tile([128, TC], F32R, name=f"sq_{m}", tag="sq")
                nc.scalar.activation(sq[:], st, AF.Square)
                nc.tensor.matmul(ss_kv[:], ones_c, sq[:],
                                 start=(m == 12), stop=(m == 15))
            else:
                nc.vector.tensor_scalar(st[:64, :], ps[:64, :],
                                        bias_sb[:64, 4:5], None, op0=ADD)

        # --- kv chunks first ---
        for m in range(12, 17):
            mchunk(m)
        bc_kv = _rms_scale(nc, a_tmp, a_ps, ss_kv, KVR, ones_r, "kv")
        stg_kv = a_st.tile([128, 4 * TC], BF16, name="stg_kv")
        for i in range(4):
            st = stage[:, (12 + i) * TC:(13 + i) * TC]
            nc.vector.tensor_tensor(stg_kv[:, i * TC:(i + 1) * TC], st,
                                    bc_kv[:], op=MUL)
        # k_pe rope (no norm) -> rows 512:576
        st = stage[:, 16 * TC:17 * TC]
        rp = a_tmp.tile([64, TC], BF16, name="rp_kpe")
        t1 = a_tmp.tile([32, TC], F32R, name="rt1", tag="rt1")
        t2 = a_tmp.tile([32, TC], F32R, name="rt2", tag="rt2")
        x1, x2 = st[0:32, :], st[32:64, :]
        nc.vector.tensor_tensor(t1[:], x1, cosa_sb[0:32, :], op=MUL)
        nc.vector.tensor_tensor(t2[:], x2, sina_sb[32:64, :], op=MUL)
        nc.vector.tensor_tensor(rp[0:32, :], t1[:], t2[:], op=SUB)
        nc.vector.tensor_tensor(t1[:], x1, sina_sb[0:32, :], op=MUL)
        nc.vector.tensor_tensor(t2[:], x2, cosa_sb[32:64, :], op=MUL)
        nc.vector.tensor_tensor(rp[32:64, :], t1[:], t2[:], op=ADD)
        nc.sync.dma_start(
            ag1kv_in[0:512, :].rearrange("(k p) t -> p k t", p=128),
            stg_kv[:].rearrange("p (k t) -> p k t", k=4))
        nc.sync.dma_start(ag1kv_in[512:576, :], rp[:])

        nc.gpsimd.collective_compute(
            "AllGather", mybir.AluOpType.bypass,
            ins=[ag1kv_in[:]], outs=[ag1kv_out[:]],
            replica_groups=[list(range(NCORES))],
        )

        # --- q chunks ---
        for m in range(12):
            mchunk(m)
        bc_q = _rms_scale(nc, a_tmp, a_ps, ss_q, QR, ones_r, "q")
        stg_q = a_st.tile([128, 12 * TC], BF16, name="stg_q")
        for m in range(12):
            st = stage[:, m * TC:(m + 1) * TC]
            nc.vector.tensor_tensor(stg_q[:, m * TC:(m + 1) * TC], st,
                                    bc_q[:], op=MUL)
        nc.sync.dma_start(
            ag1q_in[:].rearrange("(k p) t -> p k t", p=128),
            stg_q[:].rearrange("p (k t) -> p k t", k=12))

        nc.gpsimd.collective_compute(
            "AllGather", mybir.AluOpType.bypass,
            ins=[ag1q_in[:]], outs=[ag1q_out[:]],
            replica_groups=[list(range(NCORES))],
        )


def _phase_b(nc, tc, ag1kv_out, ktv, vv, kpe_sb, wkv_, wvv):
    """Head-split k_nope^T and v projections from the gathered kv latents."""
    ag1kv_v = ag1kv_out[:].rearrange("(r a) t -> a r t", a=LAT)
    with (
        tc.tile_pool(name="b_kva", bufs=1) as b_kva,
        tc.tile_pool(name="b_ps", bufs=3, space="PSUM") as b_ps,
    ):
        kva_sb = b_kva.tile([128, 4 * T], BF16, name="kva_sb")
        kvav = kva_sb[:].rearrange("p (k t) -> p k t", k=4)
        for k in range(4):
            nc.sync.dma_start(
                kvav[:, k, :].rearrange("p (r t) -> p r t", r=NCORES),
                ag1kv_v[k * 128:(k + 1) * 128])
        nc.sync.dma_start(
            kpe_sb[:].rearrange("p (r t) -> p r t", r=NCORES),
            ag1kv_v[512:576])

        for j in range(HL):
            for qb in range(4):
                ps = b_ps.tile([128, 512], F32, name=f"psk_{j}_{qb}", tag="psk")
                for k in range(4):
                    nc.tensor.matmul(ps[:], wkv_[:, k, j * 128:(j + 1) * 128],
                                     kvav[:, k, qb * 512:(qb + 1) * 512],
                                     start=(k == 0), stop=(k == 3))
                nc.scalar.activation(ktv[:, j, qb * 512:(qb + 1) * 512], ps[:],
                                     AF.Copy)

        for mt in range(16):
            ps = b_ps.tile([128, 512], F32, name=f"psv_{mt}", tag="psv")
            for k in range(4):
                nc.tensor.matmul(ps[:], kvav[:, k, mt * 128:(mt + 1) * 128],
                                 wvv[:, k, :], start=(k == 0), stop=(k == 3))
            nc.scalar.activation(vv[:, mt, :], ps[:], AF.Copy)


def _phase_q(nc, tc, io, ag1q_out, qtn, qpe, consts_t):
    """Head-split q^T projection; rope via full-tile ops + PE half-swap."""
    pswap_b = consts_t["pswap_b"]
    with (
        tc.tile_pool(name="c_qa", bufs=1) as c_qa,
        tc.tile_pool(name="c_tab", bufs=1) as c_tab,
        tc.tile_pool(name="c_w", bufs=1) as c_w,
        tc.tile_pool(name="c_tmp", bufs=2) as c_tmp,
        tc.tile_pool(name="c_ps", bufs=3, space="PSUM") as c_ps,
        tc.tile_pool(name="c_sw", bufs=2, space="PSUM") as c_sw,
    ):
        cos_sb = c_tab.tile([128, T], F32R, name="cos_sb")
        sin_sb = c_tab.tile([128, T], F32R, name="sin_sb")
        nc.sync.dma_start(cos_sb[:], io["cosC"][:])
        nc.sync.dma_start(sin_sb[:], io["sinS"][:])
        qa_sb = c_qa.tile([128, 12 * T], BF16, name="qa_sb")
        qav = qa_sb[:].rearrange("p (k t) -> p k t", k=12)
        ag1q_v = ag1q_out[:].rearrange("(r a) t -> a r t", a=QR)
        for k in range(12):
            nc.sync.dma_start(
                qav[:, k, :].rearrange("p (r t) -> p r t", r=NCORES),
                ag1q_v[k * 128:(k + 1) * 128])
        wq_sb = c_w.tile([128, 6 * 12 * 128], BF16, name="wq_sb")
        wqv = wq_sb[:].rearrange("p (m k c) -> p m k c", m=6, k=12)
        for m in range(6):
            nc.sync.dma_start(wqv[:, m], io["wqb"][m])

        for qb in range(4):
            cols = slice(qb * 512, (qb + 1) * 512)
            for m in range(6):
                ps = c_ps.tile([128, 512], F32, name=f"psq_{qb}_{m}", tag="psq")
                for k in range(12):
                    nc.tensor.matmul(ps[:], wqv[:, m, k, :],
                                     qav[:, k, cols],
                                     start=(k == 0), stop=(k == 11))
                if m < 4:
                    nc.scalar.activation(qtn[m][:, cols], ps[:], AF.Copy)
                else:
                    cs = c_tmp.tile([128, 512], BF16, name=f"cs_{qb}_{m}",
                                    tag="cs")
                    nc.scalar.activation(cs[:], ps[:], AF.Copy)
                    sw = c_sw.tile([128, 512], F32, name=f"sw_{qb}_{m}",
                                   tag="sw")
                    nc.tensor.matmul(sw[:], pswap_b[:], cs[:],
                                     start=True, stop=True)
                    t1 = c_tmp.tile([128, 512], F32R, name=f"t1_{qb}_{m}",
                                    tag="t1")
                    t2 = c_tmp.tile([128, 512], F32R, name=f"t2_{qb}_{m}",
                                    tag="t2")
                    nc.vector.tensor_tensor(t1[:], ps[:], cos_sb[:, cols],
                                            op=MUL)
                    nc.vector.tensor_tensor(t2[:], sw[:], sin_sb[:, cols],
                                            op=MUL)
                    jj = 2 * (m - 4)
                    nc.vector.tensor_tensor(qpe[jj][:, cols], t1[0:64, :],
                                            t2[0:64, :], op=ADD)
                    nc.vector.tensor_tensor(qpe[jj + 1][:, cols], t1[64:128, :],
                                            t2[64:128, :], op=ADD)


def _attn_out(nc, tc, io, ag2_ins, ag2_outs, qtn, qpe, ktv, vv, kpe_sb,
              consts_t, wov):
    """Causal attention per qb + interleaved output projection (tq=qb-1)."""
    ones_cb, ones_r, tri_b = (consts_t["ones_cb"], consts_t["ones_r"],
                              consts_t["tri_b"])
    with (
        tc.tile_pool(name="t_pt", bufs=3) as t_pt,
        tc.tile_pool(name="t_fin", bufs=2) as t_fin,
        tc.tile_pool(name="t_ring", bufs=1, space="PSUM") as t_ring,
        tc.tile_pool(name="t_ots", bufs=2, space="PSUM") as t_ots,
        tc.tile_pool(name="t_dpo", bufs=2, space="PSUM") as t_dpo,
        tc.tile_pool(name="o_oa", bufs=1) as o_oa,
        tc.tile_pool(name="o_st", bufs=2) as o_st,
    ):
        ring = t_ring.tile([128, 2048], F32, name="ring")

        def out_block(tq):
            oa = o_oa.tile([128, 32 * 512], BF16, name=f"oa_{tq}", tag="oa")
            oav = oa[:].rearrange("p (k t) -> p k t", k=32)
            nc.sync.dma_start(
                oav, ag2_outs[tq][:].rearrange("(k p) t -> p k t", p=128))
            for d in range(5):
                ps = t_dpo.tile([128, 512], F32, name=f"ops_{tq}_{d}",
                                tag="dpo")
                for k in range(32):
                    nc.tensor.matmul(ps[:], wov[:, d, k, :], oav[:, k, :],
                                     start=(k == 0), stop=(k == 31))
                st = o_st.tile([128, 512], F32, name=f"ost_{tq}_{d}",
                               tag="ost")
                nc.vector.tensor_copy(st[:], ps[:])
                nc.sync.dma_start(
                    io["outT"][d * 128:(d + 1) * 128,
                               tq * 512:(tq + 1) * 512], st[:])

        gctr = [0]

        def finals(qb, j, ots, den):
            rden = t_fin.tile([1, 512], F32R, name=f"rd_{qb}_{j}", tag="rd")
            with nc.allow_low_precision(reason="f32r = fp32 bits"):
                nc.vector.reciprocal(rden[:], den[:])
            bcp = t_dpo.tile([128, 512], F32, name=f"bcp_{qb}_{j}", tag="dpo")
            nc.tensor.matmul(bcp[:], ones_r[:1, :], rden[:],
                             start=True, stop=True)
            bcs = t_fin.tile([128, 512], F32R, name=f"bcs_{qb}_{j}", tag="bcs")
            nc.scalar.activation(bcs[:], bcp[:], AF.Copy)
            obf = t_fin.tile([128, 512], BF16, name=f"obf_{qb}_{j}", tag="obf")
            nc.vector.tensor_tensor(obf[:], ots[:], bcs[:], op=MUL)
            nc.sync.dma_start(ag2_ins[qb][j * 128:(j + 1) * 128, :], obf[:])

        for qb in range(4):
            kmax = 4 * qb + 4
            npairs = kmax // 2
            cols = slice(qb * 512, (qb + 1) * 512)
            state = {}           # j -> (ots, den)
            pend = None          # (j, p, pt, off)

            def denpv(j, p, pt, off):
                if p == 0:
                    state[j] = (
                        t_ots.tile([128, 512], F32, name=f"ot_{qb}_{j}",
                                   tag="ots"),
                        t_dpo.tile([1, 512], F32, name=f"den_{qb}_{j}",
                                   tag="dpo"),
                    )
                ots, den = state[j]
                for kk in (2 * p, 2 * p + 1):
                    c0 = max(0, kk - 4 * qb) * 128
                    psl = pt[:, (kk % 2) * 512 + c0:(kk % 2) * 512 + 512]
                    nc.tensor.matmul(den[:, c0:512], ones_cb, psl,
                                     start=(kk == 0), stop=(kk == kmax - 1))
                    nc.tensor.matmul(ots[:, c0:512],
                                     vv[:, kk, j * 128:(j + 1) * 128], psl,
                                     start=(kk == 0), stop=(kk == kmax - 1))
                if p == npairs - 1:
                    finals(qb, j, ots, den)

            for j in range(HL):
                qfn = qtn[j][:, cols]
                qfp = qpe[j][:, cols]
                for p in range(npairs):
                    kk0 = 2 * p
                    off = (gctr[0] % 2) * 1024
                    gctr[0] += 1
                    for kk in (kk0, kk0 + 1):
                        c0 = max(0, kk - 4 * qb) * 128
                        sl = ring[:, off + (kk % 2) * 512 + c0:
                                  off + (kk % 2) * 512 + 512]
                        nc.tensor.matmul(sl,
                                         ktv[:, j, kk * 128:(kk + 1) * 128],
                                         qfn[:, c0:512], start=True, stop=False)
                        nc.tensor.matmul(sl,
                                         kpe_sb[:, kk * 128:(kk + 1) * 128],
                                         qfp[:, c0:512], start=False, stop=True)
                    pt = t_pt.tile([128, 1024], BF16, name=f"pt_{qb}_{j}_{p}",
                                   tag="pt")
                    c0f = max(0, kk0 - 4 * qb) * 128
                    nc.scalar.activation(pt[:, c0f:1024],
                                         ring[:, off + c0f:off + 1024], AF.Exp)
                    for kk in (kk0, kk0 + 1):
                        o = kk - 4 * qb
                        if o >= 0:
                            d0 = (kk % 2) * 512 + o * 128
                            nc.vector.tensor_tensor(pt[:, d0:d0 + 128],
                                                    pt[:, d0:d0 + 128],
                                                    tri_b[:], op=MUL)
                    if pend is not None:
                        denpv(*pend)
                    pend = (j, p, pt, off)
            denpv(*pend)
            nc.gpsimd.collective_compute(
                "AllGather", mybir.AluOpType.bypass,
                ins=[ag2_ins[qb][:]], outs=[ag2_outs[qb][:]],
                replica_groups=[list(range(NCORES))],
            )
            if qb >= 1:
                out_block(qb - 1)
        out_block(3)


def _build():
    nc = bass.Bass("TRN2", target_bir_lowering=False, debug=False,
                   num_devices=NCORES)
    io = {
        "hT": nc.dram_tensor("hT", [D, TC], BF16, kind="ExternalInput"),
        "wa": nc.dram_tensor("wa", [17, 128, 40, 128], BF16,
                             kind="ExternalInput"),
        "biask": nc.dram_tensor("biask", [128, 5], F32, kind="ExternalInput"),
        "wqb": nc.dram_tensor("wqb", [6, 128, 12, 128], BF16,
                              kind="ExternalInput"),
        "wkvbk": nc.dram_tensor("wkvbk", [128, 4, 512], BF16,
                                kind="ExternalInput"),
        "wkvbv": nc.dram_tensor("wkvbv", [128, 4, 512], BF16,
                                kind="ExternalInput"),
        "wo": nc.dram_tensor("wo", [5, 128, 32, 128], BF16,
                             kind="ExternalInput"),
        "cosC": nc.dram_tensor("cosC", [128, T], F32R, kind="ExternalInput"),
        "sinS": nc.dram_tensor("sinS", [128, T], F32R, kind="ExternalInput"),
        "cosA": nc.dram_tensor("cosA", [128, TC], F32R, kind="ExternalInput"),
        "sinA": nc.dram_tensor("sinA", [128, TC], F32R, kind="ExternalInput"),
        "tri": nc.dram_tensor("tri", [128, 128], F32R, kind="ExternalInput"),
        "onesin": nc.dram_tensor("onesin", [128, 128], F32R,
                                 kind="ExternalInput"),
        "pswap": nc.dram_tensor("pswap", [128, 128], BF16,
                                kind="ExternalInput"),
        "outT": nc.dram_tensor("outT", [DCOL, T], F32, kind="ExternalOutput"),
    }

    with TileContext(nc) as tc:
        with (
            tc.tile_pool(name="dram", bufs=1, space="DRAM") as dram,
            tc.tile_pool(name="consts", bufs=1) as consts,
            tc.tile_pool(name="a_ht", bufs=1) as a_ht,
            tc.tile_pool(name="b_w", bufs=1) as b_w,
        ):
            # earliest DMAs: h transpose chunks + phase_b weights (no deps)
            ht_sb = a_ht.tile([128, 40 * TC], BF16, name="ht_sb")
            htv = ht_sb[:].rearrange("p (k t) -> p k t", k=40)
            hsrc = io["hT"][:].rearrange("(k p) t -> p k t", p=128)
            for k0 in range(0, 40, 10):
                nc.sync.dma_start(htv[:, k0:k0 + 10, :], hsrc[:, k0:k0 + 10, :])
            wk_sb = b_w.tile([128, 4 * 512], BF16, name="wk_sb")
            wkv_ = wk_sb[:].rearrange("p (k c) -> p k c", k=4)
            nc.sync.dma_start(wkv_, io["wkvbk"][:])
            wv_sb = b_w.tile([128, 4 * 512], BF16, name="wv_sb")
            wvv = wv_sb[:].rearrange("p (k c) -> p k c", k=4)
            nc.sync.dma_start(wvv, io["wkvbv"][:])
            ag1kv_in = dram.tile([LAT, TC], BF16, name="ag1kv_in")
            ag1kv_out = dram.tile([NCORES * LAT, TC], BF16, addr_space="Shared",
                                  name="ag1kv_out")
            ag1q_in = dram.tile([QR, TC], BF16, name="ag1q_in")
            ag1q_out = dram.tile([NCORES * QR, TC], BF16, addr_space="Shared",
                                 name="ag1q_out")
            ag2_ins = [dram.tile([HL * VH, 512], BF16, name=f"ag2_in_{qb}")
                       for qb in range(4)]
            ag2_outs = [dram.tile([H * VH, 512], BF16, addr_space="Shared",
                                  name=f"ag2_out_{qb}") for qb in range(4)]

            consts_t = {}
            ones_sb = consts.tile([128, 128], F32R, name="ones_sb")
            nc.sync.dma_start(ones_sb[:], io["onesin"][:])
            consts_t["ones_c"] = ones_sb[:, 0:1]
            consts_t["ones_r"] = ones_sb
            ones_b = consts.tile([128, 1], BF16, name="ones_b")
            nc.vector.tensor_copy(ones_b[:], ones_sb[:, 0:1])
            consts_t["ones_cb"] = ones_b[:]
            trib = consts.tile([128, 128], BF16, name="trib")
            consts_t["tri_b"] = trib
            for nm, srcn, shp in (("tri_sb", "tri", [128, 128]),
                                  ("cosa_sb", "cosA", [128, TC]),
                                  ("sina_sb", "sinA", [128, TC]),
                                  ):
                consts_t[nm] = consts.tile(shp, F32R, name=nm)
                nc.sync.dma_start(consts_t[nm][:], io[srcn][:])
            consts_t["bias_sb"] = consts.tile([128, 5], F32, name="bias_sb")
            nc.sync.dma_start(consts_t["bias_sb"][:], io["biask"][:])
            nc.vector.tensor_copy(trib[:], consts_t["tri_sb"][:])
            pswap_b = consts.tile([128, 128], BF16, name="pswap_b")
            nc.sync.dma_start(pswap_b[:], io["pswap"][:])
            consts_t["pswap_b"] = pswap_b

            with nc.named_scope("phase_a"):
                _phase_a(nc, tc, io, consts_t, ag1kv_in, ag1q_in,
                         ag1kv_out, ag1q_out, ht_sb)

            with tc.tile_pool(name="persist", bufs=1) as persist:
                kt_sb = persist.tile([128, HL * T], BF16, name="kt_sb")
                ktv = kt_sb[:].rearrange("p (j t) -> p j t", j=HL)
                v_sb = persist.tile([128, 16 * 512], BF16, name="v_sb")
                vv = v_sb[:].rearrange("p (mt c) -> p mt c", mt=16)
                kpe_sb = persist.tile([64, T], BF16, name="kpe_sb")
                qtn = [persist.tile([128, T], BF16, name=f"qtn_{m}")
                       for m in range(4)]
                qpe = [persist.tile([64, T], BF16, name=f"qpe_{j}")
                       for j in range(4)]
                with nc.named_scope("phase_b"):
                    _phase_b(nc, tc, ag1kv_out, ktv, vv, kpe_sb, wkv_, wvv)
                with nc.named_scope("phase_q"):
                    _phase_q(nc, tc, io, ag1q_out, qtn, qpe, consts_t)
                with tc.tile_pool(name="wo_pool", bufs=1) as wo_pool:
                    wo_sb = wo_pool.tile([128, 5 * 32 * 128], BF16,
                                         name="wo_sb")
                    wov = wo_sb[:].rearrange("p (d k c) -> p d k c", d=5, k=32)
                    for d in range(5):
                        nc.sync.dma_start(wov[:, d], io["wo"][d])
                    with nc.named_scope("phase_attn"):
                        _attn_out(nc, tc, io, ag2_ins, ag2_outs, qtn, qpe,
                                  ktv, vv, kpe_sb, consts_t, wov)
    return nc


def _get_nc():
    if "nc" not in _cache:
        _cache["nc"] = _build()
    return _cache["nc"]


def _prep(inputs):
    h = np.asarray(inputs["h"], np.float32)
    pos = np.asarray(inputs["position_ids"], np.int32)
    Wq_a = np.asarray(inputs["Wq_a"], np.float32)
    gq = np.asarray(inputs["gq"], np.float32)
    Wq_b = np.asarray(inputs["Wq_b"], np.float32)
    Wkv_a = np.asarray(inputs["Wkv_a"], np.float32)
    bkv_a = np.asarray(inputs["bkv_a"], np.float32)
    gkv = np.asarray(inputs["gkv"], np.float32)
    Wkv_b = np.asarray(inputs["Wkv_b"], np.float32)
    Wo = np.asarray(inputs["Wo"], np.float32)

    bf16 = ml_dtypes.bfloat16
    dperm = np.concatenate([np.arange(0, ROPE, 2), np.arange(1, ROPE, 2)])
    scale = np.float32(1.0 / math.sqrt(QK))

    hT = np.ascontiguousarray(h.T)                      # [D, T]
    wkva = Wkv_a.copy()
    wkva[:, KVR:] = Wkv_a[:, KVR + dperm]
    bias = bkv_a.copy()
    bias[KVR:] = bkv_a[KVR + dperm]
    bm = np.zeros((5, 128), np.float32)
    bm.reshape(-1)[:LAT] = bias
    biask = np.ascontiguousarray(bm.T)                  # [128, 5]

    wqb_eff = (Wq_b * gq[:, None]) * scale              # [QR, H*QK]
    wkvb_eff = Wkv_b * gkv[:, None]                     # [KVR, H*(NOPE+VH)]

    # A weights pre-tiled: wa[m, p, k, c] = Wcat[k*128+p, m*128+c]
    Wcat = np.concatenate([Wq_a, wkva], axis=1)          # [D, 2112]
    A = Wcat.reshape(40, 128, QR + LAT)
    wa = np.zeros((17, 128, 40, 128), np.float32)
    for m in range(17):
        cw = min(128, QR + LAT - m * 128)
        wa[m, :, :, :cw] = A[:, :, m * 128:m * 128 + cw].transpose(1, 0, 2)
    wa = wa.astype(bf16)

    inv = THETA ** (-np.arange(0, ROPE, 2, dtype=np.float32) / ROPE)
    fr = pos.astype(np.float32)[:, None] * inv[None, :]  # [T, 32]
    cosT = np.ascontiguousarray(np.tile(np.cos(fr).T, (4, 1)))  # [128, T]
    sinT = np.ascontiguousarray(np.tile(np.sin(fr).T, (4, 1)))
    sgn = np.repeat(np.array([-1.0, 1.0, -1.0, 1.0], np.float32), 32)[:, None]
    sinS = np.ascontiguousarray(sinT * sgn)
    tri = np.triu(np.ones((128, 128), np.float32))
    pswap = np.zeros((128, 128), np.float32)
    for i in range(128):
        pswap[i ^ 32, i] = 1.0
    pswap = np.ascontiguousarray(pswap).astype(bf16)

    in_maps = []
    for c in range(NCORES):
        heads = list(range(HL * c, HL * (c + 1)))
        qcols = [np.arange(hh * QK, hh * QK + NOPE) for hh in heads]
        for pair in range(2):
            for hh in heads[2 * pair:2 * pair + 2]:
                qcols.append(hh * QK + NOPE + dperm)
        kcols = np.concatenate(
            [np.arange(hh * (NOPE + VH), hh * (NOPE + VH) + NOPE)
             for hh in heads])
        vcols = np.concatenate(
            [np.arange(hh * (NOPE + VH) + NOPE, (hh + 1) * (NOPE + VH))
             for hh in heads])

        wqb_c = wqb_eff[:, np.concatenate(qcols)]        # [1536, 768]
        wqb_t = np.ascontiguousarray(
            wqb_c.reshape(12, 128, 6, 128).transpose(2, 1, 0, 3)).astype(bf16)
        wkvbk_t = np.ascontiguousarray(
            wkvb_eff[:, kcols].reshape(4, 128, 512).transpose(1, 0, 2)
        ).astype(bf16)
        wkvbv_t = np.ascontiguousarray(
            wkvb_eff[:, vcols].reshape(4, 128, 512).transpose(1, 0, 2)
        ).astype(bf16)
        wo_c = Wo[:, c * DCOL:(c + 1) * DCOL]            # [4096, 640]
        wo_t = np.ascontiguousarray(
            wo_c.reshape(32, 128, 5, 128).transpose(2, 1, 0, 3)).astype(bf16)

        in_maps.append({
            "hT": np.ascontiguousarray(hT[:, c * TC:(c + 1) * TC]).astype(bf16),
            "wa": wa,
            "biask": biask,
            "wqb": wqb_t,
            "wkvbk": wkvbk_t,
            "wkvbv": wkvbv_t,
            "wo": wo_t,
            "cosC": cosT,
            "sinS": sinS,
            "cosA": np.ascontiguousarray(cosT[:, c * TC:(c + 1) * TC]),
            "sinA": np.ascontiguousarray(sinT[:, c * TC:(c + 1) * TC]),
            "tri": tri,
            "onesin": np.ones((128, 128), np.float32),
            "pswap": pswap,
        })
    return in_maps


def kernel(**inputs):
    nc = _get_nc()
    in_maps = _prep(inputs)
    res = bass_utils.run_bass_kernel_spmd(
        nc, in_maps, core_ids=list(range(NCORES)), trace=TRACE[0])
    LAST_RESULT[0] = res
    out = np.empty((T, D), np.float32)
    for c in range(NCORES):
        out[:, c * DCOL:(c + 1) * DCOL] = res.results[c]["outT"].T
    return out


# revision 27
# speedup vs baseline: 1.2412x; 1.1116x over previous
"""DeepseekV2 MLA attention on 8 Trainium2 NeuronCores.

Sharding: token-split A projections -> AllGather(kv latents) early +
AllGather(q latents) -> head-split (4 heads/core) B projections + causal
attention -> per-qb AllGather(attn out) with the D-column-split output
projection interleaved one block behind the attention loop.

Layouts are d-major (feature dim on the SBUF partition axis); the host
pre-transposes h and pre-tiles every weight so each SBUF weight chunk is
one contiguous DMA. q tiles stay SBUF-resident (no DRAM spill).

Precision: bf16 matmul inputs with fp32 PSUM accumulation; rmsnorm
statistics and softmax run in fp32/f32r; softmax reciprocal via
reciprocal_approx_fast (~18 correct bits).
"""
import math

import numpy as np
import ml_dtypes

import concourse.bass as bass
import concourse.mybir as mybir
from concourse.tile import TileContext
from concourse import bass_utils

# ---------------------------------------------------------------------------
# Walrus workaround: this container's walrus accepts at most ONE sync-wait
# per TPB instruction, but Tile attaches several (tail Drain, LDWEIGHTS...).
# Split: keep the last wait, move the rest onto preceding same-engine NOPs.
# ---------------------------------------------------------------------------
import concourse.tile as _tile_mod

_orig_sched = _tile_mod.TileContext.schedule_and_allocate
_nopctr = [0]


def _split_multiwait(nc):
    for fn in nc.m.functions:
        for blk in fn.blocks:
            insts = blk.instructions
            if not any(
                i.sync_info and i.sync_info.on_wait and len(i.sync_info.on_wait) > 1
                for i in insts
            ):
                continue
            out = []
            for ins in insts:
                si = ins.sync_info
                if si and si.on_wait and len(si.on_wait) > 1:
                    waits = list(si.on_wait)
                    for w in waits[:-1]:
                        _nopctr[0] += 1
                        nop = mybir.InstNoOp(name=f"I-mws-{_nopctr[0]}", ins=[], outs=[])
                        nop.engine = ins.engine
                        nop.sync_info = mybir.SyncInfo(on_wait=[w], on_update=[])
                        out.append(nop)
                    ins.sync_info = mybir.SyncInfo(
                        on_wait=[waits[-1]], on_update=list(si.on_update or [])
                    )
                out.append(ins)
            blk.instructions = out


def _patched_sched(self, *a, **k):
    res = _orig_sched(self, *a, **k)
    _split_multiwait(self.nc)
    return res


if getattr(_tile_mod.TileContext.schedule_and_allocate, "__name__", "") != "_patched_sched":
    _tile_mod.TileContext.schedule_and_allocate = _patched_sched


# ---------------------------------------------------------------------------
T, D, H = 2048, 5120, 32
NOPE, ROPE, QK = 128, 64, 192
KVR, QR, VH = 512, 1536, 128
EPS, THETA = 1e-6, 10000.0
NCORES = 8
HL = H // NCORES          # 4 heads per core
TC = T // NCORES          # 256 tokens per core
LAT = KVR + ROPE          # 576
DCOL = D // NCORES        # 640 output columns per core

F32 = mybir.dt.float32
F32R = mybir.dt.float32r
BF16 = mybir.dt.bfloat16
AF = mybir.ActivationFunctionType
MUL = mybir.AluOpType.mult
ADD = mybir.AluOpType.add
SUB = mybir.AluOpType.subtract

TRACE = [False]          # test.py sets TRACE[0]=True to profile
LAST_RESULT = [None]     # BassKernelResults stashed here for test.py

_cache = {}


def _rms_scale(nc, a_tmp, a_ps, ss, nfeat, ones_r, key):
    """1/sqrt(mean(ss)+eps) broadcast to [128, TC] f32r."""
    ms = a_tmp.tile([1, TC], F32, name=f"ms_{key}", tag="ms")
    nc.vector.tensor_scalar(ms[:], ss[:], 1.0 / nfeat, EPS, op0=MUL, op1=ADD)
    sm = a_tmp.tile([1, TC], F32R, name=f"sm_{key}", tag="sm")
    nc.scalar.activation(sm[:], ms[:], AF.Sqrt)
    rs = a_tmp.tile([1, TC], F32R, name=f"rs_{key}", tag="rs")
    with nc.allow_low_precision(reason="f32r holds full fp32 bits"):
        nc.vector.reciprocal(rs[:], sm[:])
    bps = a_ps.tile([128, TC], F32, name=f"bps_{key}", tag="bps")
    nc.tensor.matmul(bps[:], ones_r[:1, :], rs[:], start=True, stop=True)
    bc = a_tmp.tile([128, TC], F32R, name=f"bc_{key}", tag=f"bc{key}")
    nc.vector.tensor_copy(bc[:], bps[:])
    return bc


def _phase_a(nc, tc, io, consts_t, ag1kv_in, ag1kv_out, agq, ht_sb):
    """Token-split A projections, kv-first so its allgather ships early.

    q latents ship RAW (unnormalized) in two halves as soon as computed;
    the per-token 1/rms factor follows in a tiny third allgather and is
    folded into phase_q's consumption of the gathered latents.
    """
    ag1qa_in, ag1qa_out, ag1qb_in, ag1qb_out, ag1rs_in, ag1rs_out = agq
    ones_c, ones_r = consts_t["ones_c"], consts_t["ones_r"]
    cosa_sb, sina_sb, bias_sb = (consts_t["cosa_sb"], consts_t["sina_sb"],
                                 consts_t["bias_sb"])
    with (
        tc.tile_pool(name="a_w", bufs=3) as a_w,
        tc.tile_pool(name="a_st", bufs=1) as a_st,
        tc.tile_pool(name="a_tmp", bufs=3) as a_tmp,
        tc.tile_pool(name="a_ps", bufs=2, space="PSUM") as a_ps,
        tc.tile_pool(name="a_ss", bufs=1, space="PSUM") as a_ss,
    ):
        htv = ht_sb[:].rearrange("p (k t) -> p k t", k=40)
        stage = a_st.tile([128, 17 * TC], F32R, name="stage")
        stg_q = a_st.tile([128, 12 * TC], BF16, name="stg_q")
        ss_q = a_ss.tile([1, TC], F32, name="ss_q")
        ss_kv = a_ss.tile([1, TC], F32, name="ss_kv")

        def mchunk(m):
            mrows = 64 if m == 16 else 128
            wt = a_w.tile([128, 40 * 128], BF16, name=f"a_w_{m}", tag="aw")
            wtv = wt[:].rearrange("p (k c) -> p k c", k=40)
            nc.sync.dma_start(wtv, io["wa"][m])
            ps = a_ps.tile([128, TC], F32, name=f"a_ps_{m}", tag="aps")
            for k in range(40):
                nc.tensor.matmul(ps[:mrows, :], wtv[:, k, :mrows], htv[:, k, :],
                                 start=(k == 0), stop=(k == 39))
            st = stage[:, m * TC:(m + 1) * TC]
            if m < 12:
                sq = a_tmp.tile([128, TC], F32R, name=f"sq_{m}", tag="sq")
                nc.scalar.activation(sq[:], ps[:], AF.Square)
                nc.tensor.matmul(ss_q[:], ones_c, sq[:],
                                 start=(m == 0), stop=(m == 11))
                nc.vector.tensor_copy(stg_q[:, m * TC:(m + 1) * TC], ps[:])
            elif m < 16:
                nc.vector.tensor_scalar(st, ps[:], bias_sb[:, m - 12:m - 11],
                                        None, op0=ADD)
                sq = a_tmp.tile([128, TC], F32R, name=f"sq_{m}", tag="sq")
                nc.scalar.activation(sq[:], st, AF.Square)
                nc.tensor.matmul(ss_kv[:], ones_c, sq[:],
                                 start=(m == 12), stop=(m == 15))
            else:
                nc.vector.tensor_scalar(st[:64, :], ps[:64, :],
                                        bias_sb[:64, 4:5], None, op0=ADD)

        # --- kv chunks first ---
        for m in range(12, 17):
            mchunk(m)
        bc_kv = _rms_scale(nc, a_tmp, a_ps, ss_kv, KVR, ones_r, "kv")
        stg_kv = a_st.tile([128, 4 * TC], BF16, name="stg_kv")
        for i in range(4):
            st = stage[:, (12 + i) * TC:(13 + i) * TC]
            nc.vector.tensor_tensor(stg_kv[:, i * TC:(i + 1) * TC], st,
                                    bc_kv[:], op=MUL)
        # k_pe rope (no norm) -> rows 512:576
        st = stage[:, 16 * TC:17 * TC]
        rp = a_tmp.tile([64, TC], BF16, name="rp_kpe")
        t1 = a_tmp.tile([32, TC], F32R, name="rt1", tag="rt1")
        t2 = a_tmp.tile([32, TC], F32R, name="rt2", tag="rt2")
        x1, x2 = st[0:32, :], st[32:64, :]
        nc.vector.tensor_tensor(t1[:], x1, cosa_sb[0:32, :], op=MUL)
        nc.vector.tensor_tensor(t2[:], x2, sina_sb[32:64, :], op=MUL)
        nc.vector.tensor_tensor(rp[0:32, :], t1[:], t2[:], op=SUB)
        nc.vector.tensor_tensor(t1[:], x1, sina_sb[0:32, :], op=MUL)
        nc.vector.tensor_tensor(t2[:], x2, cosa_sb[32:64, :], op=MUL)
        nc.vector.tensor_tensor(rp[32:64, :], t1[:], t2[:], op=ADD)
        nc.sync.dma_start(
            ag1kv_in[0:512, :].rearrange("(k p) t -> p k t", p=128),
            stg_kv[:].rearrange("p (k t) -> p k t", k=4))
        nc.sync.dma_start(ag1kv_in[512:576, :], rp[:])

        nc.gpsimd.collective_compute(
            "AllGather", mybir.AluOpType.bypass,
            ins=[ag1kv_in[:]], outs=[ag1kv_out[:]],
            replica_groups=[list(range(NCORES))],
        )

        # --- q chunks: raw latents ship in halves as soon as computed ---
        for m in range(6):
            mchunk(m)
        nc.sync.dma_start(
            ag1qa_in[:].rearrange("(k p) t -> p k t", p=128),
            stg_q[:, 0:6 * TC].rearrange("p (k t) -> p k t", k=6))
        nc.gpsimd.collective_compute(
            "AllGather", mybir.AluOpType.bypass,
            ins=[ag1qa_in[:]], outs=[ag1qa_out[:]],
            replica_groups=[list(range(NCORES))],
        )
        for m in range(6, 12):
            mchunk(m)
        nc.sync.dma_start(
            ag1qb_in[:].rearrange("(k p) t -> p k t", p=128),
            stg_q[:, 6 * TC:12 * TC].rearrange("p (k t) -> p k t", k=6))
        nc.gpsimd.collective_compute(
            "AllGather", mybir.AluOpType.bypass,
            ins=[ag1qb_in[:]], outs=[ag1qb_out[:]],
            replica_groups=[list(range(NCORES))],
        )
        # rs_q = 1/sqrt(mean(ss_q)+eps) row, gathered separately
        ms = a_tmp.tile([1, TC], F32, name="ms_q", tag="ms")
        nc.vector.tensor_scalar(ms[:], ss_q[:], 1.0 / QR, EPS,
                                op0=MUL, op1=ADD)
        sm = a_tmp.tile([1, TC], F32R, name="sm_q", tag="sm")
        nc.scalar.activation(sm[:], ms[:], AF.Sqrt)
        rsq = a_tmp.tile([1, TC], F32R, name="rs_q", tag="rs")
        with nc.allow_low_precision(reason="f32r holds full fp32 bits"):
            nc.vector.reciprocal(rsq[:], sm[:])
        nc.sync.dma_start(ag1rs_in[:], rsq[:])
        nc.gpsimd.collective_compute(
            "AllGather", mybir.AluOpType.bypass,
            ins=[ag1rs_in[:]], outs=[ag1rs_out[:]],
            replica_groups=[list(range(NCORES))],
        )


def _phase_b(nc, tc, ag1kv_out, ktv, vv, kpe_sb, wkv_, wvv):
    """Head-split k_nope^T and v projections from the gathered kv latents."""
    ag1kv_v = ag1kv_out[:].rearrange("(r a) t -> a r t", a=LAT)
    with (
        tc.tile_pool(name="b_kva", bufs=1) as b_kva,
        tc.tile_pool(name="b_ps", bufs=3, space="PSUM") as b_ps,
    ):
        kva_sb = b_kva.tile([128, 4 * T], BF16, name="kva_sb")
        kvav = kva_sb[:].rearrange("p (k t) -> p k t", k=4)
        for k in range(4):
            nc.sync.dma_start(
                kvav[:, k, :].rearrange("p (r t) -> p r t", r=NCORES),
                ag1kv_v[k * 128:(k + 1) * 128])
        nc.sync.dma_start(
            kpe_sb[:].rearrange("p (r t) -> p r t", r=NCORES),
            ag1kv_v[512:576])

        for j in range(HL):
            for qb in range(4):
                ps = b_ps.tile([128, 512], F32, name=f"psk_{j}_{qb}", tag="psk")
                for k in range(4):
                    nc.tensor.matmul(ps[:], wkv_[:, k, j * 128:(j + 1) * 128],
                                     kvav[:, k, qb * 512:(qb + 1) * 512],
                                     start=(k == 0), stop=(k == 3))
                nc.scalar.activation(ktv[:, j, qb * 512:(qb + 1) * 512], ps[:],
                                     AF.Copy)

        for mt in range(16):
            ps = b_ps.tile([128, 512], F32, name=f"psv_{mt}", tag="psv")
            for k in range(4):
                nc.tensor.matmul(ps[:], kvav[:, k, mt * 128:(mt + 1) * 128],
                                 wvv[:, k, :], start=(k == 0), stop=(k == 3))
            nc.scalar.activation(vv[:, mt, :], ps[:], AF.Copy)


def _phase_q(nc, tc, io, agq, qtn, qpe, consts_t):
    """Head-split q^T projection from raw gathered latents.

    The per-token rms factor (gathered separately) is folded into the
    PSUM evacuation for the nope chunks and pre-multiplied into the
    cos/sin tables for the rope chunks. Rope itself is full-tile ops
    plus a PE half-swap via a permutation matmul.
    """
    _, ag1qa_out, _, ag1qb_out, _, ag1rs_out = agq
    pswap_b, ones_r = consts_t["pswap_b"], consts_t["ones_r"]
    with (
        tc.tile_pool(name="c_qa", bufs=1) as c_qa,
        tc.tile_pool(name="c_tab", bufs=1) as c_tab,
        tc.tile_pool(name="c_w", bufs=1) as c_w,
        tc.tile_pool(name="c_tmp", bufs=2) as c_tmp,
        tc.tile_pool(name="c_ps", bufs=6, space="PSUM") as c_ps,
        tc.tile_pool(name="c_sw", bufs=2, space="PSUM") as c_sw,
    ):
        cos_sb = c_tab.tile([128, T], F32R, name="cos_sb")
        sin_sb = c_tab.tile([128, T], F32R, name="sin_sb")
        nc.sync.dma_start(cos_sb[:], io["cosC"][:])
        nc.sync.dma_start(sin_sb[:], io["sinS"][:])
        qa_sb = c_qa.tile([128, 12 * T], BF16, name="qa_sb")
        qav = qa_sb[:].rearrange("p (k t) -> p k t", k=12)
        qa_v = ag1qa_out[:].rearrange("(r a) t -> a r t", a=6 * 128)
        qb_v = ag1qb_out[:].rearrange("(r a) t -> a r t", a=6 * 128)
        for k in range(6):
            nc.sync.dma_start(
                qav[:, k, :].rearrange("p (r t) -> p r t", r=NCORES),
                qa_v[k * 128:(k + 1) * 128])
        for k in range(6, 12):
            nc.sync.dma_start(
                qav[:, k, :].rearrange("p (r t) -> p r t", r=NCORES),
                qb_v[(k - 6) * 128:(k - 5) * 128])
        wq_sb = c_w.tile([128, 6 * 12 * 128], BF16, name="wq_sb")
        wqv = wq_sb[:].rearrange("p (m k c) -> p m k c", m=6, k=12)
        for m in range(6):
            nc.sync.dma_start(wqv[:, m], io["wqb"][m])

        # rs broadcast [128, T] f32r, then fold into the trig tables
        rsg = c_tab.tile([1, T], F32R, name="rsg")
        nc.sync.dma_start(
            rsg[:], ag1rs_out[:].rearrange("(o r) t -> o (r t)", o=1))
        rsb = c_tab.tile([128, T], F32R, name="rsb")
        for r in range(0, NCORES, 2):
            bp = c_sw.tile([128, 2 * TC], F32, name=f"rsb_{r}", tag="sw")
            nc.tensor.matmul(bp[:], ones_r[:1, :],
                             rsg[:, r * TC:(r + 2) * TC],
                             start=True, stop=True)
            nc.scalar.activation(rsb[:, r * TC:(r + 2) * TC], bp[:], AF.Copy)
        nc.vector.tensor_tensor(cos_sb[:], cos_sb[:], rsb[:], op=MUL)
        nc.vector.tensor_tensor(sin_sb[:], sin_sb[:], rsb[:], op=MUL)

        for qb in range(4):
            cols = slice(qb * 512, (qb + 1) * 512)
            pss = []
            for m in range(6):
                ps = c_ps.tile([128, 512], F32, name=f"psq_{qb}_{m}",
                               tag="psq")
                for k in range(6):
                    nc.tensor.matmul(ps[:], wqv[:, m, k, :], qav[:, k, cols],
                                     start=(k == 0), stop=False)
                pss.append(ps)
            for m in range(6):
                ps = pss[m]
                for k in range(6, 12):
                    nc.tensor.matmul(ps[:], wqv[:, m, k, :], qav[:, k, cols],
                                     start=False, stop=(k == 11))
                if m < 4:
                    nc.vector.tensor_tensor(qtn[m][:, cols], ps[:],
                                            rsb[:, cols], op=MUL)
                else:
                    cs = c_tmp.tile([128, 512], BF16, name=f"cs_{qb}_{m}",
                                    tag="cs")
                    nc.scalar.activation(cs[:], ps[:], AF.Copy)
                    sw = c_sw.tile([128, 512], F32, name=f"sw_{qb}_{m}",
                                   tag="sw")
                    nc.tensor.matmul(sw[:], pswap_b[:], cs[:],
                                     start=True, stop=True)
                    t1 = c_tmp.tile([128, 512], F32R, name=f"t1_{qb}_{m}",
                                    tag="t1")
                    t2 = c_tmp.tile([128, 512], F32R, name=f"t2_{qb}_{m}",
                                    tag="t2")
                    nc.vector.tensor_tensor(t1[:], ps[:], cos_sb[:, cols],
                                            op=MUL)
                    nc.vector.tensor_tensor(t2[:], sw[:], sin_sb[:, cols],
                                            op=MUL)
                    jj = 2 * (m - 4)
                    nc.vector.tensor_tensor(qpe[jj][:, cols], t1[0:64, :],
                                            t2[0:64, :], op=ADD)
                    nc.vector.tensor_tensor(qpe[jj + 1][:, cols], t1[64:128, :],
                                            t2[64:128, :], op=ADD)


def _attn_out(nc, tc, io, ag2a_ins, ag2a_outs, ag2b_ins, ag2b_outs,
              qtn, qpe, ktv, vv, kpe_sb, consts_t, wov):
    """Causal attention per qb + interleaved output projection.

    Software-pipelined: den/PV matmuls trail the score matmuls by one
    kk-pair in the PE FIFO (so exp never stalls PE); per-head finals are
    split (reciprocal early, broadcast/rescale two pairs later). The
    attention-output allgather is split per head-pair (a: heads 0-1,
    b: heads 2-3) so the output projection of a token block can start
    on the a-half while the b-half is still in flight; each out half is
    accumulated in PSUM, evacuated, and the halves summed on the DVE.
    """
    ones_cb, ones_r, tri_b = (consts_t["ones_cb"], consts_t["ones_r"],
                              consts_t["tri_b"])
    with (
        tc.tile_pool(name="t_pt", bufs=3) as t_pt,
        tc.tile_pool(name="t_fin", bufs=2) as t_fin,
        tc.tile_pool(name="t_ring", bufs=1, space="PSUM") as t_ring,
        tc.tile_pool(name="t_ots", bufs=2, space="PSUM") as t_ots,
        tc.tile_pool(name="t_dpo", bufs=2, space="PSUM") as t_dpo,
        tc.tile_pool(name="o_oa", bufs=1) as o_oa,
        tc.tile_pool(name="o_part", bufs=5) as o_part,
        tc.tile_pool(name="o_st", bufs=2) as o_st,
    ):
        ring = t_ring.tile([128, 2048], F32, name="ring")
        oa_t = {}
        parts = {}

        def out_dma(tq):
            oaa = o_oa.tile([128, 16 * 512], BF16, name=f"oaa_{tq}", tag="oaa")
            nc.sync.dma_start(
                oaa[:].rearrange("p (k t) -> p k t", k=16),
                ag2a_outs[tq][:].rearrange("(k p) t -> p k t", p=128))
            oab = o_oa.tile([128, 16 * 512], BF16, name=f"oab_{tq}", tag="oab")
            nc.sync.dma_start(
                oab[:].rearrange("p (k t) -> p k t", k=16),
                ag2b_outs[tq][:].rearrange("(k p) t -> p k t", p=128))
            oa_t[tq] = (oaa[:].rearrange("p (k t) -> p k t", k=16),
                        oab[:].rearrange("p (k t) -> p k t", k=16))

        def out_half(tq, d, half):
            oav = oa_t[tq][half]
            ps = t_dpo.tile([128, 512], F32, name=f"ops_{tq}_{d}_{half}",
                            tag="dpo")
            for i in range(16):
                kg = 4 * (i // 2) + 2 * half + (i % 2)
                nc.tensor.matmul(ps[:], wov[:, d, kg, :], oav[:, i, :],
                                 start=(i == 0), stop=(i == 15))
            if half == 0:
                pt = o_part.tile([128, 512], F32, name=f"part_{tq}_{d}",
                                 tag="part")
                nc.vector.tensor_copy(pt[:], ps[:])
                parts[(tq, d)] = pt
            else:
                st = o_st.tile([128, 512], F32, name=f"ost_{tq}_{d}",
                               tag="ost")
                nc.vector.tensor_tensor(st[:], ps[:], parts.pop((tq, d))[:],
                                        op=ADD)
                nc.sync.dma_start(
                    io["outT"][d * 128:(d + 1) * 128,
                               tq * 512:(tq + 1) * 512], st[:])

        def out_rest(tq, d_from_a):
            for d in range(d_from_a, 5):
                out_half(tq, d, 0)
                if d >= 2:
                    out_half(tq, d - 2, 1)
            out_half(tq, 3, 1)
            out_half(tq, 4, 1)

        gctr = [0]

        def finals_a(qb, j, den):
            den_s = t_fin.tile([1, 512], F32R, name=f"dns_{qb}_{j}",
                               tag="dns")
            nc.scalar.activation(den_s[:], den[:], AF.Copy)
            rden = t_fin.tile([1, 512], F32R, name=f"rd_{qb}_{j}", tag="rd")
            with nc.allow_low_precision(reason="f32r = fp32 bits"):
                nc.vector.reciprocal(rden[:], den_s[:])
            return rden

        def finals_b(qb, j, ots, rden):
            bcp = t_dpo.tile([128, 512], F32, name=f"bcp_{qb}_{j}", tag="dpo")
            nc.tensor.matmul(bcp[:], ones_r[:1, :], rden[:],
                             start=True, stop=True)
            bcs = t_fin.tile([128, 512], F32R, name=f"bcs_{qb}_{j}", tag="bcs")
            nc.scalar.activation(bcs[:], bcp[:], AF.Copy)
            obf = t_fin.tile([128, 512], BF16, name=f"obf_{qb}_{j}", tag="obf")
            nc.vector.tensor_tensor(obf[:], ots[:], bcs[:], op=MUL)
            if j < 2:
                nc.sync.dma_start(ag2a_ins[qb][j * 128:(j + 1) * 128, :],
                                  obf[:])
            else:
                nc.sync.dma_start(ag2b_ins[qb][(j - 2) * 128:(j - 1) * 128, :],
                                  obf[:])
            if j == 1:
                nc.gpsimd.collective_compute(
                    "AllGather", mybir.AluOpType.bypass,
                    ins=[ag2a_ins[qb][:]], outs=[ag2a_outs[qb][:]],
                    replica_groups=[list(range(NCORES))],
                )

        for qb in range(4):
            kmax = 4 * qb + 4
            npairs = kmax // 2
            cols = slice(qb * 512, (qb + 1) * 512)
            state = {}           # j -> (ots, den)
            pend = None          # (j, p, pt)
            finq = []            # [(emit_iter, j, ots, rden)]
            it = [0]

            def denpv(j, p, pt):
                if p == 0:
                    state[j] = (
                        t_ots.tile([128, 512], F32, name=f"ot_{qb}_{j}",
                                   tag="ots"),
                        t_dpo.tile([1, 512], F32, name=f"den_{qb}_{j}",
                                   tag="dpo"),
                    )
                ots, den = state[j]
                for kk in (2 * p, 2 * p + 1):
                    c0 = max(0, kk - 4 * qb) * 128
                    psl = pt[:, (kk % 2) * 512 + c0:(kk % 2) * 512 + 512]
                    nc.tensor.matmul(den[:, c0:512], ones_cb, psl,
                                     start=(kk == 0), stop=(kk == kmax - 1))
                    nc.tensor.matmul(ots[:, c0:512],
                                     vv[:, kk, j * 128:(j + 1) * 128], psl,
                                     start=(kk == 0), stop=(kk == kmax - 1))
                if p == npairs - 1:
                    finq.append((it[0], j, ots, finals_a(qb, j, den)))

            def flush_finals(min_age):
                while finq and it[0] - finq[0][0] >= min_age:
                    _, j, ots, rden = finq.pop(0)
                    finals_b(qb, j, ots, rden)

            for j in range(HL):
                qfn = qtn[j][:, cols]
                qfp = qpe[j][:, cols]
                for p in range(npairs):
                    kk0 = 2 * p
                    off = (gctr[0] % 2) * 1024
                    gctr[0] += 1
                    for kk in (kk0, kk0 + 1):
                        c0 = max(0, kk - 4 * qb) * 128
                        sl = ring[:, off + (kk % 2) * 512 + c0:
                                  off + (kk % 2) * 512 + 512]
                        nc.tensor.matmul(sl,
                                         ktv[:, j, kk * 128:(kk + 1) * 128],
                                         qfn[:, c0:512], start=True, stop=False)
                        nc.tensor.matmul(sl,
                                         kpe_sb[:, kk * 128:(kk + 1) * 128],
                                         qfp[:, c0:512], start=False, stop=True)
                    pt = t_pt.tile([128, 1024], BF16, name=f"pt_{qb}_{j}_{p}",
                                   tag="pt")
                    c0f = max(0, kk0 - 4 * qb) * 128
                    nc.scalar.activation(pt[:, c0f:1024],
                                         ring[:, off + c0f:off + 1024], AF.Exp)
                    for kk in (kk0, kk0 + 1):
                        o = kk - 4 * qb
                        if o >= 0:
                            d0 = (kk % 2) * 512 + o * 128
                            nc.vector.tensor_tensor(pt[:, d0:d0 + 128],
                                                    pt[:, d0:d0 + 128],
                                                    tri_b[:], op=MUL)
                    flush_finals(2)
                    if pend is not None:
                        denpv(*pend)
                    pend = (j, p, pt)
                    it[0] += 1
            denpv(*pend)
            # PE filler (a-halves of the previous out block) while the
            # last heads' reciprocals run
            if qb >= 1:
                out_half(qb - 1, 0, 0)
                out_half(qb - 1, 1, 0)
            flush_finals(0)
            nc.gpsimd.collective_compute(
                "AllGather", mybir.AluOpType.bypass,
                ins=[ag2b_ins[qb][:]], outs=[ag2b_outs[qb][:]],
                replica_groups=[list(range(NCORES))],
            )
            out_dma(qb)
            if qb >= 1:
                out_rest(qb - 1, 2)
        out_rest(3, 0)


def _build():
    nc = bass.Bass("TRN2", target_bir_lowering=False, debug=False,
                   num_devices=NCORES)
    io = {
        "hT": nc.dram_tensor("hT", [D, TC], BF16, kind="ExternalInput"),
        "wa": nc.dram_tensor("wa", [17, 128, 40, 128], BF16,
                             kind="ExternalInput"),
        "biask": nc.dram_tensor("biask", [128, 5], F32, kind="ExternalInput"),
        "wqb": nc.dram_tensor("wqb", [6, 128, 12, 128], BF16,
                              kind="ExternalInput"),
        "wkvbk": nc.dram_tensor("wkvbk", [128, 4, 512], BF16,
                                kind="ExternalInput"),
        "wkvbv": nc.dram_tensor("wkvbv", [128, 4, 512], BF16,
                                kind="ExternalInput"),
        "wo": nc.dram_tensor("wo", [5, 128, 32, 128], BF16,
                             kind="ExternalInput"),
        "cosC": nc.dram_tensor("cosC", [128, T], F32R, kind="ExternalInput"),
        "sinS": nc.dram_tensor("sinS", [128, T], F32R, kind="ExternalInput"),
        "cosA": nc.dram_tensor("cosA", [128, TC], F32R, kind="ExternalInput"),
        "sinA": nc.dram_tensor("sinA", [128, TC], F32R, kind="ExternalInput"),
        "tri": nc.dram_tensor("tri", [128, 128], F32R, kind="ExternalInput"),
        "onesin": nc.dram_tensor("onesin", [128, 128], F32R,
                                 kind="ExternalInput"),
        "pswap": nc.dram_tensor("pswap", [128, 128], BF16,
                                kind="ExternalInput"),
        "outT": nc.dram_tensor("outT", [DCOL, T], F32, kind="ExternalOutput"),
    }

    with TileContext(nc) as tc:
        with (
            tc.tile_pool(name="dram", bufs=1, space="DRAM") as dram,
            tc.tile_pool(name="consts", bufs=1) as consts,
            tc.tile_pool(name="a_ht", bufs=1) as a_ht,
            tc.tile_pool(name="b_w", bufs=1) as b_w,
        ):
            # earliest DMAs: h transpose chunks + phase_b weights (no deps)
            ht_sb = a_ht.tile([128, 40 * TC], BF16, name="ht_sb")
            htv = ht_sb[:].rearrange("p (k t) -> p k t", k=40)
            hsrc = io["hT"][:].rearrange("(k p) t -> p k t", p=128)
            for k0 in range(0, 40, 10):
                nc.sync.dma_start(htv[:, k0:k0 + 10, :], hsrc[:, k0:k0 + 10, :])
            wk_sb = b_w.tile([128, 4 * 512], BF16, name="wk_sb")
            wkv_ = wk_sb[:].rearrange("p (k c) -> p k c", k=4)
            nc.sync.dma_start(wkv_, io["wkvbk"][:])
            wv_sb = b_w.tile([128, 4 * 512], BF16, name="wv_sb")
            wvv = wv_sb[:].rearrange("p (k c) -> p k c", k=4)
            nc.sync.dma_start(wvv, io["wkvbv"][:])
            ag1kv_in = dram.tile([LAT, TC], BF16, name="ag1kv_in")
            ag1kv_out = dram.tile([NCORES * LAT, TC], BF16, addr_space="Shared",
                                  name="ag1kv_out")
            agq = (
                dram.tile([QR // 2, TC], BF16, name="ag1qa_in"),
                dram.tile([NCORES * QR // 2, TC], BF16, addr_space="Shared",
                          name="ag1qa_out"),
                dram.tile([QR // 2, TC], BF16, name="ag1qb_in"),
                dram.tile([NCORES * QR // 2, TC], BF16, addr_space="Shared",
                          name="ag1qb_out"),
                dram.tile([1, TC], F32R, name="ag1rs_in"),
                dram.tile([NCORES, TC], F32R, addr_space="Shared",
                          name="ag1rs_out"),
            )
            ag2a_ins = [dram.tile([2 * VH, 512], BF16, name=f"ag2a_in_{qb}")
                        for qb in range(4)]
            ag2a_outs = [dram.tile([NCORES * 2 * VH, 512], BF16,
                                   addr_space="Shared",
                                   name=f"ag2a_out_{qb}") for qb in range(4)]
            ag2b_ins = [dram.tile([2 * VH, 512], BF16, name=f"ag2b_in_{qb}")
                        for qb in range(4)]
            ag2b_outs = [dram.tile([NCORES * 2 * VH, 512], BF16,
                                   addr_space="Shared",
                                   name=f"ag2b_out_{qb}") for qb in range(4)]

            consts_t = {}
            ones_sb = consts.tile([128, 128], F32R, name="ones_sb")
            nc.sync.dma_start(ones_sb[:], io["onesin"][:])
            consts_t["ones_c"] = ones_sb[:, 0:1]
            consts_t["ones_r"] = ones_sb
            ones_b = consts.tile([128, 1], BF16, name="ones_b")
            nc.vector.tensor_copy(ones_b[:], ones_sb[:, 0:1])
            consts_t["ones_cb"] = ones_b[:]
            trib = consts.tile([128, 128], BF16, name="trib")
            consts_t["tri_b"] = trib
            for nm, srcn, shp in (("tri_sb", "tri", [128, 128]),
                                  ("cosa_sb", "cosA", [128, TC]),
                                  ("sina_sb", "sinA", [128, TC]),
                                  ):
                consts_t[nm] = consts.tile(shp, F32R, name=nm)
                nc.sync.dma_start(consts_t[nm][:], io[srcn][:])
            consts_t["bias_sb"] = consts.tile([128, 5], F32, name="bias_sb")
            nc.sync.dma_start(consts_t["bias_sb"][:], io["biask"][:])
            nc.vector.tensor_copy(trib[:], consts_t["tri_sb"][:])
            pswap_b = consts.tile([128, 128], BF16, name="pswap_b")
            nc.sync.dma_start(pswap_b[:], io["pswap"][:])
            consts_t["pswap_b"] = pswap_b

            with nc.named_scope("phase_a"):
                _phase_a(nc, tc, io, consts_t, ag1kv_in, ag1kv_out, agq,
                         ht_sb)

            with tc.tile_pool(name="persist", bufs=1) as persist:
                kt_sb = persist.tile([128, HL * T], BF16, name="kt_sb")
                ktv = kt_sb[:].rearrange("p (j t) -> p j t", j=HL)
                v_sb = persist.tile([128, 16 * 512], BF16, name="v_sb")
                vv = v_sb[:].rearrange("p (mt c) -> p mt c", mt=16)
                kpe_sb = persist.tile([64, T], BF16, name="kpe_sb")
                qtn = [persist.tile([128, T], BF16, name=f"qtn_{m}")
                       for m in range(4)]
                qpe = [persist.tile([64, T], BF16, name=f"qpe_{j}")
                       for j in range(4)]
                with nc.named_scope("phase_b"):
                    _phase_b(nc, tc, ag1kv_out, ktv, vv, kpe_sb, wkv_, wvv)
                with nc.named_scope("phase_q"):
                    _phase_q(nc, tc, io, agq, qtn, qpe, consts_t)
                with tc.tile_pool(name="wo_pool", bufs=1) as wo_pool:
                    wo_sb = wo_pool.tile([128, 5 * 32 * 128], BF16,
                                         name="wo_sb")
                    wov = wo_sb[:].rearrange("p (d k c) -> p d k c", d=5, k=32)
                    for d in range(5):
                        nc.sync.dma_start(wov[:, d], io["wo"][d])
                    with nc.named_scope("phase_attn"):
                        _attn_out(nc, tc, io, ag2a_ins, ag2a_outs,
                                  ag2b_ins, ag2b_outs, qtn, qpe,
                                  ktv, vv, kpe_sb, consts_t, wov)
    return nc


def _get_nc():
    if "nc" not in _cache:
        _cache["nc"] = _build()
    return _cache["nc"]


def _prep(inputs):
    h = np.asarray(inputs["h"], np.float32)
    pos = np.asarray(inputs["position_ids"], np.int32)
    Wq_a = np.asarray(inputs["Wq_a"], np.float32)
    gq = np.asarray(inputs["gq"], np.float32)
    Wq_b = np.asarray(inputs["Wq_b"], np.float32)
    Wkv_a = np.asarray(inputs["Wkv_a"], np.float32)
    bkv_a = np.asarray(inputs["bkv_a"], np.float32)
    gkv = np.asarray(inputs["gkv"], np.float32)
    Wkv_b = np.asarray(inputs["Wkv_b"], np.float32)
    Wo = np.asarray(inputs["Wo"], np.float32)

    bf16 = ml_dtypes.bfloat16
    dperm = np.concatenate([np.arange(0, ROPE, 2), np.arange(1, ROPE, 2)])
    scale = np.float32(1.0 / math.sqrt(QK))

    hT = np.ascontiguousarray(h.T)                      # [D, T]
    wkva = Wkv_a.copy()
    wkva[:, KVR:] = Wkv_a[:, KVR + dperm]
    bias = bkv_a.copy()
    bias[KVR:] = bkv_a[KVR + dperm]
    bm = np.zeros((5, 128), np.float32)
    bm.reshape(-1)[:LAT] = bias
    biask = np.ascontiguousarray(bm.T)                  # [128, 5]

    wqb_eff = (Wq_b * gq[:, None]) * scale              # [QR, H*QK]
    wkvb_eff = Wkv_b * gkv[:, None]                     # [KVR, H*(NOPE+VH)]

    # A weights pre-tiled: wa[m, p, k, c] = Wcat[k*128+p, m*128+c]
    Wcat = np.concatenate([Wq_a, wkva], axis=1)          # [D, 2112]
    A = Wcat.reshape(40, 128, QR + LAT)
    wa = np.zeros((17, 128, 40, 128), np.float32)
    for m in range(17):
        cw = min(128, QR + LAT - m * 128)
        wa[m, :, :, :cw] = A[:, :, m * 128:m * 128 + cw].transpose(1, 0, 2)
    wa = wa.astype(bf16)

    inv = THETA ** (-np.arange(0, ROPE, 2, dtype=np.float32) / ROPE)
    fr = pos.astype(np.float32)[:, None] * inv[None, :]  # [T, 32]
    cosT = np.ascontiguousarray(np.tile(np.cos(fr).T, (4, 1)))  # [128, T]
    sinT = np.ascontiguousarray(np.tile(np.sin(fr).T, (4, 1)))
    sgn = np.repeat(np.array([-1.0, 1.0, -1.0, 1.0], np.float32), 32)[:, None]
    sinS = np.ascontiguousarray(sinT * sgn)
    tri = np.triu(np.ones((128, 128), np.float32))
    pswap = np.zeros((128, 128), np.float32)
    for i in range(128):
        pswap[i ^ 32, i] = 1.0
    pswap = np.ascontiguousarray(pswap).astype(bf16)

    in_maps = []
    for c in range(NCORES):
        heads = list(range(HL * c, HL * (c + 1)))
        qcols = [np.arange(hh * QK, hh * QK + NOPE) for hh in heads]
        for pair in range(2):
            for hh in heads[2 * pair:2 * pair + 2]:
                qcols.append(hh * QK + NOPE + dperm)
        kcols = np.concatenate(
            [np.arange(hh * (NOPE + VH), hh * (NOPE + VH) + NOPE)
             for hh in heads])
        vcols = np.concatenate(
            [np.arange(hh * (NOPE + VH) + NOPE, (hh + 1) * (NOPE + VH))
             for hh in heads])

        wqb_c = wqb_eff[:, np.concatenate(qcols)]        # [1536, 768]
        wqb_t = np.ascontiguousarray(
            wqb_c.reshape(12, 128, 6, 128).transpose(2, 1, 0, 3)).astype(bf16)
        wkvbk_t = np.ascontiguousarray(
            wkvb_eff[:, kcols].reshape(4, 128, 512).transpose(1, 0, 2)
        ).astype(bf16)
        wkvbv_t = np.ascontiguousarray(
            wkvb_eff[:, vcols].reshape(4, 128, 512).transpose(1, 0, 2)
        ).astype(bf16)
        wo_c = Wo[:, c * DCOL:(c + 1) * DCOL]            # [4096, 640]
        wo_t = np.ascontiguousarray(
            wo_c.reshape(32, 128, 5, 128).transpose(2, 1, 0, 3)).astype(bf16)

        in_maps.append({
            "hT": np.ascontiguousarray(hT[:, c * TC:(c + 1) * TC]).astype(bf16),
            "wa": wa,
            "biask": biask,
            "wqb": wqb_t,
            "wkvbk": wkvbk_t,
            "wkvbv": wkvbv_t,
            "wo": wo_t,
            "cosC": cosT,
            "sinS": sinS,
            "cosA": np.ascontiguousarray(cosT[:, c * TC:(c + 1) * TC]),
            "sinA": np.ascontiguousarray(sinT[:, c * TC:(c + 1) * TC]),
            "tri": tri,
            "onesin": np.ones((128, 128), np.float32),
            "pswap": pswap,
        })
    return in_maps


def kernel(**inputs):
    nc = _get_nc()
    in_maps = _prep(inputs)
    res = bass_utils.run_bass_kernel_spmd(
        nc, in_maps, core_ids=list(range(NCORES)), trace=TRACE[0])
    LAST_RESULT[0] = res
    out = np.empty((T, D), np.float32)
    for c in range(NCORES):
        out[:, c * DCOL:(c + 1) * DCOL] = res.results[c]["outT"].T
    return out


# revision 28
# speedup vs baseline: 1.2463x; 1.0041x over previous
"""DeepseekV2 MLA attention on 8 Trainium2 NeuronCores.

Sharding: token-split A projections -> AllGather(kv latents) early +
AllGather(q latents) -> head-split (4 heads/core) B projections + causal
attention -> per-qb AllGather(attn out) with the D-column-split output
projection interleaved one block behind the attention loop.

Layouts are d-major (feature dim on the SBUF partition axis); the host
pre-transposes h and pre-tiles every weight so each SBUF weight chunk is
one contiguous DMA. q tiles stay SBUF-resident (no DRAM spill).

Precision: bf16 matmul inputs with fp32 PSUM accumulation; rmsnorm
statistics and softmax run in fp32/f32r; softmax reciprocal via
reciprocal_approx_fast (~18 correct bits).
"""
import math

import numpy as np
import ml_dtypes

import concourse.bass as bass
import concourse.mybir as mybir
from concourse.tile import TileContext
from concourse import bass_utils

# ---------------------------------------------------------------------------
# Walrus workaround: this container's walrus accepts at most ONE sync-wait
# per TPB instruction, but Tile attaches several (tail Drain, LDWEIGHTS...).
# Split: keep the last wait, move the rest onto preceding same-engine NOPs.
# ---------------------------------------------------------------------------
import concourse.tile as _tile_mod

_orig_sched = _tile_mod.TileContext.schedule_and_allocate
_nopctr = [0]


def _split_multiwait(nc):
    for fn in nc.m.functions:
        for blk in fn.blocks:
            insts = blk.instructions
            if not any(
                i.sync_info and i.sync_info.on_wait and len(i.sync_info.on_wait) > 1
                for i in insts
            ):
                continue
            out = []
            for ins in insts:
                si = ins.sync_info
                if si and si.on_wait and len(si.on_wait) > 1:
                    waits = list(si.on_wait)
                    for w in waits[:-1]:
                        _nopctr[0] += 1
                        nop = mybir.InstNoOp(name=f"I-mws-{_nopctr[0]}", ins=[], outs=[])
                        nop.engine = ins.engine
                        nop.sync_info = mybir.SyncInfo(on_wait=[w], on_update=[])
                        out.append(nop)
                    ins.sync_info = mybir.SyncInfo(
                        on_wait=[waits[-1]], on_update=list(si.on_update or [])
                    )
                out.append(ins)
            blk.instructions = out


def _patched_sched(self, *a, **k):
    res = _orig_sched(self, *a, **k)
    _split_multiwait(self.nc)
    return res


if getattr(_tile_mod.TileContext.schedule_and_allocate, "__name__", "") != "_patched_sched":
    _tile_mod.TileContext.schedule_and_allocate = _patched_sched


# ---------------------------------------------------------------------------
T, D, H = 2048, 5120, 32
NOPE, ROPE, QK = 128, 64, 192
KVR, QR, VH = 512, 1536, 128
EPS, THETA = 1e-6, 10000.0
NCORES = 8
HL = H // NCORES          # 4 heads per core
TC = T // NCORES          # 256 tokens per core
LAT = KVR + ROPE          # 576
DCOL = D // NCORES        # 640 output columns per core

F32 = mybir.dt.float32
F32R = mybir.dt.float32r
BF16 = mybir.dt.bfloat16
AF = mybir.ActivationFunctionType
MUL = mybir.AluOpType.mult
ADD = mybir.AluOpType.add
SUB = mybir.AluOpType.subtract

TRACE = [False]          # test.py sets TRACE[0]=True to profile
LAST_RESULT = [None]     # BassKernelResults stashed here for test.py

_cache = {}


def _rms_scale(nc, a_tmp, a_ps, ss, nfeat, ones_r, key):
    """1/sqrt(mean(ss)+eps) broadcast to [128, TC] f32r."""
    ms = a_tmp.tile([1, TC], F32, name=f"ms_{key}", tag="ms")
    nc.vector.tensor_scalar(ms[:], ss[:], 1.0 / nfeat, EPS, op0=MUL, op1=ADD)
    sm = a_tmp.tile([1, TC], F32R, name=f"sm_{key}", tag="sm")
    nc.scalar.activation(sm[:], ms[:], AF.Sqrt)
    rs = a_tmp.tile([1, TC], F32R, name=f"rs_{key}", tag="rs")
    with nc.allow_low_precision(reason="f32r holds full fp32 bits"):
        nc.vector.reciprocal(rs[:], sm[:])
    bps = a_ps.tile([128, TC], F32, name=f"bps_{key}", tag="bps")
    nc.tensor.matmul(bps[:], ones_r[:1, :], rs[:], start=True, stop=True)
    bc = a_tmp.tile([128, TC], F32R, name=f"bc_{key}", tag=f"bc{key}")
    nc.vector.tensor_copy(bc[:], bps[:])
    return bc


def _phase_a(nc, tc, io, consts_t, ag1kv_in, ag1kv_out, agq, ht_sb):
    """Token-split A projections, kv-first so its allgather ships early.

    q latents ship RAW (unnormalized) in two halves as soon as computed;
    the per-token 1/rms factor follows in a tiny third allgather and is
    folded into phase_q's consumption of the gathered latents.
    """
    ag1qa_in, ag1qa_out, ag1qb_in, ag1qb_out, ag1rs_in, ag1rs_out = agq
    ones_c, ones_r = consts_t["ones_c"], consts_t["ones_r"]
    cosa_sb, sina_sb, bias_sb = (consts_t["cosa_sb"], consts_t["sina_sb"],
                                 consts_t["bias_sb"])
    with (
        tc.tile_pool(name="a_w", bufs=3) as a_w,
        tc.tile_pool(name="a_st", bufs=1) as a_st,
        tc.tile_pool(name="a_tmp", bufs=3) as a_tmp,
        tc.tile_pool(name="a_ps", bufs=2, space="PSUM") as a_ps,
        tc.tile_pool(name="a_ss", bufs=1, space="PSUM") as a_ss,
    ):
        htv = ht_sb[:].rearrange("p (k t) -> p k t", k=40)
        stage = a_st.tile([128, 17 * TC], F32R, name="stage")
        stg_q = a_st.tile([128, 12 * TC], BF16, name="stg_q")
        ss_q = a_ss.tile([1, TC], F32, name="ss_q")
        ss_kv = a_ss.tile([1, TC], F32, name="ss_kv")

        def mchunk(m):
            mrows = 64 if m == 16 else 128
            wt = a_w.tile([128, 40 * 128], BF16, name=f"a_w_{m}", tag="aw")
            wtv = wt[:].rearrange("p (k c) -> p k c", k=40)
            nc.sync.dma_start(wtv, io["wa"][m])
            ps = a_ps.tile([128, TC], F32, name=f"a_ps_{m}", tag="aps")
            for k in range(40):
                nc.tensor.matmul(ps[:mrows, :], wtv[:, k, :mrows], htv[:, k, :],
                                 start=(k == 0), stop=(k == 39))
            st = stage[:, m * TC:(m + 1) * TC]
            if m < 12:
                sq = a_tmp.tile([128, TC], F32R, name=f"sq_{m}", tag="sq")
                nc.scalar.activation(sq[:], ps[:], AF.Square)
                nc.tensor.matmul(ss_q[:], ones_c, sq[:],
                                 start=(m == 0), stop=(m == 11))
                nc.vector.tensor_copy(stg_q[:, m * TC:(m + 1) * TC], ps[:])
            elif m < 16:
                nc.vector.tensor_scalar(st, ps[:], bias_sb[:, m - 12:m - 11],
                                        None, op0=ADD)
                sq = a_tmp.tile([128, TC], F32R, name=f"sq_{m}", tag="sq")
                nc.scalar.activation(sq[:], st, AF.Square)
                nc.tensor.matmul(ss_kv[:], ones_c, sq[:],
                                 start=(m == 12), stop=(m == 15))
            else:
                nc.vector.tensor_scalar(st[:64, :], ps[:64, :],
                                        bias_sb[:64, 4:5], None, op0=ADD)

        # --- kv chunks first ---
        for m in range(12, 17):
            mchunk(m)
        bc_kv = _rms_scale(nc, a_tmp, a_ps, ss_kv, KVR, ones_r, "kv")
        stg_kv = a_st.tile([128, 4 * TC], BF16, name="stg_kv")
        for i in range(4):
            st = stage[:, (12 + i) * TC:(13 + i) * TC]
            nc.vector.tensor_tensor(stg_kv[:, i * TC:(i + 1) * TC], st,
                                    bc_kv[:], op=MUL)
        # k_pe rope (no norm) -> rows 512:576
        st = stage[:, 16 * TC:17 * TC]
        rp = a_tmp.tile([64, TC], BF16, name="rp_kpe")
        t1 = a_tmp.tile([32, TC], F32R, name="rt1", tag="rt1")
        t2 = a_tmp.tile([32, TC], F32R, name="rt2", tag="rt2")
        x1, x2 = st[0:32, :], st[32:64, :]
        nc.vector.tensor_tensor(t1[:], x1, cosa_sb[0:32, :], op=MUL)
        nc.vector.tensor_tensor(t2[:], x2, sina_sb[32:64, :], op=MUL)
        nc.vector.tensor_tensor(rp[0:32, :], t1[:], t2[:], op=SUB)
        nc.vector.tensor_tensor(t1[:], x1, sina_sb[0:32, :], op=MUL)
        nc.vector.tensor_tensor(t2[:], x2, cosa_sb[32:64, :], op=MUL)
        nc.vector.tensor_tensor(rp[32:64, :], t1[:], t2[:], op=ADD)
        nc.sync.dma_start(
            ag1kv_in[0:512, :].rearrange("(k p) t -> p k t", p=128),
            stg_kv[:].rearrange("p (k t) -> p k t", k=4))
        nc.sync.dma_start(ag1kv_in[512:576, :], rp[:])

        nc.gpsimd.collective_compute(
            "AllGather", mybir.AluOpType.bypass,
            ins=[ag1kv_in[:]], outs=[ag1kv_out[:]],
            replica_groups=[list(range(NCORES))],
        )

        # --- q chunks: raw latents ship in halves as soon as computed ---
        for m in range(6):
            mchunk(m)
        nc.sync.dma_start(
            ag1qa_in[:].rearrange("(k p) t -> p k t", p=128),
            stg_q[:, 0:6 * TC].rearrange("p (k t) -> p k t", k=6))
        nc.gpsimd.collective_compute(
            "AllGather", mybir.AluOpType.bypass,
            ins=[ag1qa_in[:]], outs=[ag1qa_out[:]],
            replica_groups=[list(range(NCORES))],
        )
        for m in range(6, 12):
            mchunk(m)
        nc.sync.dma_start(
            ag1qb_in[:].rearrange("(k p) t -> p k t", p=128),
            stg_q[:, 6 * TC:12 * TC].rearrange("p (k t) -> p k t", k=6))
        nc.gpsimd.collective_compute(
            "AllGather", mybir.AluOpType.bypass,
            ins=[ag1qb_in[:]], outs=[ag1qb_out[:]],
            replica_groups=[list(range(NCORES))],
        )
        # rs_q = 1/sqrt(mean(ss_q)+eps) row, gathered separately
        ms = a_tmp.tile([1, TC], F32, name="ms_q", tag="ms")
        nc.vector.tensor_scalar(ms[:], ss_q[:], 1.0 / QR, EPS,
                                op0=MUL, op1=ADD)
        sm = a_tmp.tile([1, TC], F32R, name="sm_q", tag="sm")
        nc.scalar.activation(sm[:], ms[:], AF.Sqrt)
        rsq = a_tmp.tile([1, TC], F32R, name="rs_q", tag="rs")
        with nc.allow_low_precision(reason="f32r holds full fp32 bits"):
            nc.vector.reciprocal(rsq[:], sm[:])
        nc.sync.dma_start(ag1rs_in[:], rsq[:])
        nc.gpsimd.collective_compute(
            "AllGather", mybir.AluOpType.bypass,
            ins=[ag1rs_in[:]], outs=[ag1rs_out[:]],
            replica_groups=[list(range(NCORES))],
        )


def _phase_b(nc, tc, ag1kv_out, ktv, vv, kpe_sb, wkv_, wvv):
    """Head-split k_nope^T and v projections from the gathered kv latents."""
    ag1kv_v = ag1kv_out[:].rearrange("(r a) t -> a r t", a=LAT)
    with (
        tc.tile_pool(name="b_kva", bufs=1) as b_kva,
        tc.tile_pool(name="b_ps", bufs=3, space="PSUM") as b_ps,
    ):
        kva_sb = b_kva.tile([128, 4 * T], BF16, name="kva_sb")
        kvav = kva_sb[:].rearrange("p (k t) -> p k t", k=4)
        for k in range(4):
            nc.sync.dma_start(
                kvav[:, k, :].rearrange("p (r t) -> p r t", r=NCORES),
                ag1kv_v[k * 128:(k + 1) * 128])
        nc.sync.dma_start(
            kpe_sb[:].rearrange("p (r t) -> p r t", r=NCORES),
            ag1kv_v[512:576])

        for j in range(HL):
            for qb in range(4):
                ps = b_ps.tile([128, 512], F32, name=f"psk_{j}_{qb}", tag="psk")
                for k in range(4):
                    nc.tensor.matmul(ps[:], wkv_[:, k, j * 128:(j + 1) * 128],
                                     kvav[:, k, qb * 512:(qb + 1) * 512],
                                     start=(k == 0), stop=(k == 3))
                nc.scalar.activation(ktv[:, j, qb * 512:(qb + 1) * 512], ps[:],
                                     AF.Copy)

        for mt in range(16):
            ps = b_ps.tile([128, 512], F32, name=f"psv_{mt}", tag="psv")
            for k in range(4):
                nc.tensor.matmul(ps[:], kvav[:, k, mt * 128:(mt + 1) * 128],
                                 wvv[:, k, :], start=(k == 0), stop=(k == 3))
            nc.scalar.activation(vv[:, mt, :], ps[:], AF.Copy)


def _phase_q(nc, tc, io, agq, qtn, qpe, consts_t):
    """Head-split q^T projection from raw gathered latents.

    The per-token rms factor (gathered separately) is folded into the
    PSUM evacuation for the nope chunks and pre-multiplied into the
    cos/sin tables for the rope chunks. Rope itself is full-tile ops
    plus a PE half-swap via a permutation matmul.
    """
    _, ag1qa_out, _, ag1qb_out, _, ag1rs_out = agq
    pswap_b, ones_r = consts_t["pswap_b"], consts_t["ones_r"]
    with (
        tc.tile_pool(name="c_qa", bufs=1) as c_qa,
        tc.tile_pool(name="c_tab", bufs=1) as c_tab,
        tc.tile_pool(name="c_w", bufs=1) as c_w,
        tc.tile_pool(name="c_tmp", bufs=2) as c_tmp,
        tc.tile_pool(name="c_ps", bufs=6, space="PSUM") as c_ps,
        tc.tile_pool(name="c_sw", bufs=2, space="PSUM") as c_sw,
    ):
        cos_sb = c_tab.tile([128, T], F32R, name="cos_sb")
        sin_sb = c_tab.tile([128, T], F32R, name="sin_sb")
        nc.sync.dma_start(cos_sb[:], io["cosC"][:])
        nc.sync.dma_start(sin_sb[:], io["sinS"][:])
        qa_sb = c_qa.tile([128, 12 * T], BF16, name="qa_sb")
        qav = qa_sb[:].rearrange("p (k t) -> p k t", k=12)
        qa_v = ag1qa_out[:].rearrange("(r a) t -> a r t", a=6 * 128)
        qb_v = ag1qb_out[:].rearrange("(r a) t -> a r t", a=6 * 128)
        for k in range(6):
            nc.sync.dma_start(
                qav[:, k, :].rearrange("p (r t) -> p r t", r=NCORES),
                qa_v[k * 128:(k + 1) * 128])
        for k in range(6, 12):
            nc.sync.dma_start(
                qav[:, k, :].rearrange("p (r t) -> p r t", r=NCORES),
                qb_v[(k - 6) * 128:(k - 5) * 128])
        wq_sb = c_w.tile([128, 6 * 12 * 128], BF16, name="wq_sb")
        wqv = wq_sb[:].rearrange("p (m k c) -> p m k c", m=6, k=12)
        for m in range(6):
            nc.sync.dma_start(wqv[:, m], io["wqb"][m])

        # rs broadcast [128, T] f32r, then fold into the trig tables
        rsg = c_tab.tile([1, T], F32R, name="rsg")
        nc.sync.dma_start(
            rsg[:], ag1rs_out[:].rearrange("(o r) t -> o (r t)", o=1))
        rsb = c_tab.tile([128, T], F32R, name="rsb")
        for r in range(0, NCORES, 2):
            bp = c_sw.tile([128, 2 * TC], F32, name=f"rsb_{r}", tag="sw")
            nc.tensor.matmul(bp[:], ones_r[:1, :],
                             rsg[:, r * TC:(r + 2) * TC],
                             start=True, stop=True)
            nc.scalar.activation(rsb[:, r * TC:(r + 2) * TC], bp[:], AF.Copy)
        nc.vector.tensor_tensor(cos_sb[:], cos_sb[:], rsb[:], op=MUL)
        nc.vector.tensor_tensor(sin_sb[:], sin_sb[:], rsb[:], op=MUL)

        for qb in range(4):
            cols = slice(qb * 512, (qb + 1) * 512)
            pss = []
            for m in range(6):
                ps = c_ps.tile([128, 512], F32, name=f"psq_{qb}_{m}",
                               tag="psq")
                for k in range(6):
                    nc.tensor.matmul(ps[:], wqv[:, m, k, :], qav[:, k, cols],
                                     start=(k == 0), stop=False)
                pss.append(ps)
            for m in range(6):
                ps = pss[m]
                for k in range(6, 12):
                    nc.tensor.matmul(ps[:], wqv[:, m, k, :], qav[:, k, cols],
                                     start=False, stop=(k == 11))
                if m < 4:
                    nc.vector.tensor_tensor(qtn[m][:, cols], ps[:],
                                            rsb[:, cols], op=MUL)
                else:
                    cs = c_tmp.tile([128, 512], BF16, name=f"cs_{qb}_{m}",
                                    tag="cs")
                    nc.scalar.activation(cs[:], ps[:], AF.Copy)
                    sw = c_sw.tile([128, 512], F32, name=f"sw_{qb}_{m}",
                                   tag="sw")
                    nc.tensor.matmul(sw[:], pswap_b[:], cs[:],
                                     start=True, stop=True)
                    t1 = c_tmp.tile([128, 512], F32R, name=f"t1_{qb}_{m}",
                                    tag="t1")
                    t2 = c_tmp.tile([128, 512], F32R, name=f"t2_{qb}_{m}",
                                    tag="t2")
                    nc.vector.tensor_tensor(t1[:], ps[:], cos_sb[:, cols],
                                            op=MUL)
                    nc.vector.tensor_tensor(t2[:], sw[:], sin_sb[:, cols],
                                            op=MUL)
                    jj = 2 * (m - 4)
                    nc.vector.tensor_tensor(qpe[jj][:, cols], t1[0:64, :],
                                            t2[0:64, :], op=ADD)
                    nc.vector.tensor_tensor(qpe[jj + 1][:, cols], t1[64:128, :],
                                            t2[64:128, :], op=ADD)


def _attn_out(nc, tc, io, ag2a_ins, ag2a_outs, ag2b_ins, ag2b_outs,
              qtn, qpe, ktv, vv, kpe_sb, consts_t, wov):
    """Causal attention per qb + interleaved output projection.

    Software-pipelined: den/PV matmuls trail the score matmuls by one
    kk-pair in the PE FIFO (so exp never stalls PE); per-head finals are
    split (reciprocal early, broadcast/rescale two pairs later). The
    attention-output allgather is split per head-pair (a: heads 0-1,
    b: heads 2-3) so the output projection of a token block can start
    on the a-half while the b-half is still in flight; each out half is
    accumulated in PSUM, evacuated, and the halves summed on the DVE.
    """
    ones_cb, ones_r, tri_b = (consts_t["ones_cb"], consts_t["ones_r"],
                              consts_t["tri_b"])
    with (
        tc.tile_pool(name="t_pt", bufs=3) as t_pt,
        tc.tile_pool(name="t_fin", bufs=2) as t_fin,
        tc.tile_pool(name="t_ring", bufs=1, space="PSUM") as t_ring,
        tc.tile_pool(name="t_ots", bufs=2, space="PSUM") as t_ots,
        tc.tile_pool(name="t_dpo", bufs=2, space="PSUM") as t_dpo,
        tc.tile_pool(name="o_oa", bufs=1) as o_oa,
        tc.tile_pool(name="o_part", bufs=5) as o_part,
        tc.tile_pool(name="o_st", bufs=2) as o_st,
    ):
        ring = t_ring.tile([128, 2048], F32, name="ring")
        oa_t = {}
        parts = {}

        def out_dma(tq):
            oaa = o_oa.tile([128, 16 * 512], BF16, name=f"oaa_{tq}", tag="oaa")
            nc.sync.dma_start(
                oaa[:].rearrange("p (k t) -> p k t", k=16),
                ag2a_outs[tq][:].rearrange("(k p) t -> p k t", p=128))
            oab = o_oa.tile([128, 16 * 512], BF16, name=f"oab_{tq}", tag="oab")
            nc.sync.dma_start(
                oab[:].rearrange("p (k t) -> p k t", k=16),
                ag2b_outs[tq][:].rearrange("(k p) t -> p k t", p=128))
            oa_t[tq] = (oaa[:].rearrange("p (k t) -> p k t", k=16),
                        oab[:].rearrange("p (k t) -> p k t", k=16))

        def out_half(tq, d, half):
            oav = oa_t[tq][half]
            ps = t_dpo.tile([128, 512], F32, name=f"ops_{tq}_{d}_{half}",
                            tag="dpo")
            for i in range(16):
                kg = 4 * (i // 2) + 2 * half + (i % 2)
                nc.tensor.matmul(ps[:], wov[:, d, kg, :], oav[:, i, :],
                                 start=(i == 0), stop=(i == 15))
            if half == 0:
                pt = o_part.tile([128, 512], F32, name=f"part_{tq}_{d}",
                                 tag="part")
                nc.vector.tensor_copy(pt[:], ps[:])
                parts[(tq, d)] = pt
            else:
                st = o_st.tile([128, 512], F32, name=f"ost_{tq}_{d}",
                               tag="ost")
                nc.vector.tensor_tensor(st[:], ps[:], parts.pop((tq, d))[:],
                                        op=ADD)
                nc.sync.dma_start(
                    io["outT"][d * 128:(d + 1) * 128,
                               tq * 512:(tq + 1) * 512], st[:])

        def out_rest(tq, d_from_a):
            for d in range(d_from_a, 5):
                out_half(tq, d, 0)
                if d >= 2:
                    out_half(tq, d - 2, 1)
            out_half(tq, 3, 1)
            out_half(tq, 4, 1)

        gctr = [0]

        def finals_a(qb, j, den):
            den_s = t_fin.tile([1, 512], F32R, name=f"dns_{qb}_{j}",
                               tag="dns")
            nc.scalar.activation(den_s[:], den[:], AF.Copy)
            rden = t_fin.tile([1, 512], F32R, name=f"rd_{qb}_{j}", tag="rd")
            with nc.allow_low_precision(reason="f32r = fp32 bits"):
                nc.vector.reciprocal(rden[:], den_s[:])
            return rden

        def finals_b(qb, j, ots, rden):
            bcp = t_dpo.tile([128, 512], F32, name=f"bcp_{qb}_{j}", tag="dpo")
            nc.tensor.matmul(bcp[:], ones_r[:1, :], rden[:],
                             start=True, stop=True)
            bcs = t_fin.tile([128, 512], F32R, name=f"bcs_{qb}_{j}", tag="bcs")
            nc.scalar.activation(bcs[:], bcp[:], AF.Copy)
            obf = t_fin.tile([128, 512], BF16, name=f"obf_{qb}_{j}", tag="obf")
            nc.vector.tensor_tensor(obf[:], ots[:], bcs[:], op=MUL)
            if j < 2:
                nc.sync.dma_start(ag2a_ins[qb][j * 128:(j + 1) * 128, :],
                                  obf[:])
            else:
                nc.sync.dma_start(ag2b_ins[qb][(j - 2) * 128:(j - 1) * 128, :],
                                  obf[:])
            if j == 1:
                nc.gpsimd.collective_compute(
                    "AllGather", mybir.AluOpType.bypass,
                    ins=[ag2a_ins[qb][:]], outs=[ag2a_outs[qb][:]],
                    replica_groups=[list(range(NCORES))],
                )

        for qb in range(4):
            kmax = 4 * qb + 4
            cols = slice(qb * 512, (qb + 1) * 512)
            state = {}           # j -> (ots, den)
            pend = []            # [(j, kk, pt)] depth-2 pipeline
            finq = []            # [(emit_iter, j, ots, rden)]
            it = [0]

            def denpv(j, kk, pt):
                if kk == 0:
                    state[j] = (
                        t_ots.tile([128, 512], F32, name=f"ot_{qb}_{j}",
                                   tag="ots"),
                        t_dpo.tile([1, 512], F32, name=f"den_{qb}_{j}",
                                   tag="dpo"),
                    )
                ots, den = state[j]
                c0 = max(0, kk - 4 * qb) * 128
                psl = pt[:, c0:512]
                nc.tensor.matmul(den[:, c0:512], ones_cb, psl,
                                 start=(kk == 0), stop=(kk == kmax - 1))
                nc.tensor.matmul(ots[:, c0:512],
                                 vv[:, kk, j * 128:(j + 1) * 128], psl,
                                 start=(kk == 0), stop=(kk == kmax - 1))
                if kk == kmax - 1:
                    finq.append((it[0], j, ots, finals_a(qb, j, den)))

            def flush_finals(min_age):
                while finq and it[0] - finq[0][0] >= min_age:
                    _, j, ots, rden = finq.pop(0)
                    finals_b(qb, j, ots, rden)

            for j in range(HL):
                qfn = qtn[j][:, cols]
                qfp = qpe[j][:, cols]
                for kk in range(kmax):
                    c0 = max(0, kk - 4 * qb) * 128
                    off = (gctr[0] % 4) * 512
                    gctr[0] += 1
                    sl = ring[:, off + c0:off + 512]
                    nc.tensor.matmul(sl,
                                     ktv[:, j, kk * 128:(kk + 1) * 128],
                                     qfn[:, c0:512], start=True, stop=False)
                    nc.tensor.matmul(sl,
                                     kpe_sb[:, kk * 128:(kk + 1) * 128],
                                     qfp[:, c0:512], start=False, stop=True)
                    pt = t_pt.tile([128, 512], BF16, name=f"pt_{qb}_{j}_{kk}",
                                   tag="pt")
                    nc.scalar.activation(pt[:, c0:512], ring[:, off + c0:
                                                             off + 512], AF.Exp)
                    o = kk - 4 * qb
                    if o >= 0:
                        nc.vector.tensor_tensor(pt[:, c0:c0 + 128],
                                                pt[:, c0:c0 + 128],
                                                tri_b[:], op=MUL)
                    flush_finals(4)
                    pend.append((j, kk, pt))
                    if len(pend) > 2:
                        denpv(*pend.pop(0))
                    it[0] += 1
            while pend:
                denpv(*pend.pop(0))
            # PE filler (a-halves of the previous out block) while the
            # last heads' reciprocals run
            if qb >= 1:
                out_half(qb - 1, 0, 0)
                out_half(qb - 1, 1, 0)
            flush_finals(0)
            nc.gpsimd.collective_compute(
                "AllGather", mybir.AluOpType.bypass,
                ins=[ag2b_ins[qb][:]], outs=[ag2b_outs[qb][:]],
                replica_groups=[list(range(NCORES))],
            )
            out_dma(qb)
            if qb >= 1:
                out_rest(qb - 1, 2)
        out_rest(3, 0)


def _build():
    nc = bass.Bass("TRN2", target_bir_lowering=False, debug=False,
                   num_devices=NCORES)
    io = {
        "hT": nc.dram_tensor("hT", [D, TC], BF16, kind="ExternalInput"),
        "wa": nc.dram_tensor("wa", [17, 128, 40, 128], BF16,
                             kind="ExternalInput"),
        "biask": nc.dram_tensor("biask", [128, 5], F32, kind="ExternalInput"),
        "wqb": nc.dram_tensor("wqb", [6, 128, 12, 128], BF16,
                              kind="ExternalInput"),
        "wkvbk": nc.dram_tensor("wkvbk", [128, 4, 512], BF16,
                                kind="ExternalInput"),
        "wkvbv": nc.dram_tensor("wkvbv", [128, 4, 512], BF16,
                                kind="ExternalInput"),
        "wo": nc.dram_tensor("wo", [5, 128, 32, 128], BF16,
                             kind="ExternalInput"),
        "cosC": nc.dram_tensor("cosC", [128, T], F32R, kind="ExternalInput"),
        "sinS": nc.dram_tensor("sinS", [128, T], F32R, kind="ExternalInput"),
        "cosA": nc.dram_tensor("cosA", [128, TC], F32R, kind="ExternalInput"),
        "sinA": nc.dram_tensor("sinA", [128, TC], F32R, kind="ExternalInput"),
        "tri": nc.dram_tensor("tri", [128, 128], F32R, kind="ExternalInput"),
        "onesin": nc.dram_tensor("onesin", [128, 128], F32R,
                                 kind="ExternalInput"),
        "pswap": nc.dram_tensor("pswap", [128, 128], BF16,
                                kind="ExternalInput"),
        "outT": nc.dram_tensor("outT", [DCOL, T], F32, kind="ExternalOutput"),
    }

    with TileContext(nc) as tc:
        with (
            tc.tile_pool(name="dram", bufs=1, space="DRAM") as dram,
            tc.tile_pool(name="consts", bufs=1) as consts,
            tc.tile_pool(name="a_ht", bufs=1) as a_ht,
            tc.tile_pool(name="b_w", bufs=1) as b_w,
        ):
            # earliest DMAs: h transpose chunks + phase_b weights (no deps)
            ht_sb = a_ht.tile([128, 40 * TC], BF16, name="ht_sb")
            htv = ht_sb[:].rearrange("p (k t) -> p k t", k=40)
            hsrc = io["hT"][:].rearrange("(k p) t -> p k t", p=128)
            for k0 in range(0, 40, 10):
                nc.sync.dma_start(htv[:, k0:k0 + 10, :], hsrc[:, k0:k0 + 10, :])
            wk_sb = b_w.tile([128, 4 * 512], BF16, name="wk_sb")
            wkv_ = wk_sb[:].rearrange("p (k c) -> p k c", k=4)
            nc.sync.dma_start(wkv_, io["wkvbk"][:])
            wv_sb = b_w.tile([128, 4 * 512], BF16, name="wv_sb")
            wvv = wv_sb[:].rearrange("p (k c) -> p k c", k=4)
            nc.sync.dma_start(wvv, io["wkvbv"][:])
            ag1kv_in = dram.tile([LAT, TC], BF16, name="ag1kv_in")
            ag1kv_out = dram.tile([NCORES * LAT, TC], BF16, addr_space="Shared",
                                  name="ag1kv_out")
            agq = (
                dram.tile([QR // 2, TC], BF16, name="ag1qa_in"),
                dram.tile([NCORES * QR // 2, TC], BF16, addr_space="Shared",
                          name="ag1qa_out"),
                dram.tile([QR // 2, TC], BF16, name="ag1qb_in"),
                dram.tile([NCORES * QR // 2, TC], BF16, addr_space="Shared",
                          name="ag1qb_out"),
                dram.tile([1, TC], F32R, name="ag1rs_in"),
                dram.tile([NCORES, TC], F32R, addr_space="Shared",
                          name="ag1rs_out"),
            )
            ag2a_ins = [dram.tile([2 * VH, 512], BF16, name=f"ag2a_in_{qb}")
                        for qb in range(4)]
            ag2a_outs = [dram.tile([NCORES * 2 * VH, 512], BF16,
                                   addr_space="Shared",
                                   name=f"ag2a_out_{qb}") for qb in range(4)]
            ag2b_ins = [dram.tile([2 * VH, 512], BF16, name=f"ag2b_in_{qb}")
                        for qb in range(4)]
            ag2b_outs = [dram.tile([NCORES * 2 * VH, 512], BF16,
                                   addr_space="Shared",
                                   name=f"ag2b_out_{qb}") for qb in range(4)]

            consts_t = {}
            ones_sb = consts.tile([128, 128], F32R, name="ones_sb")
            nc.sync.dma_start(ones_sb[:], io["onesin"][:])
            consts_t["ones_c"] = ones_sb[:, 0:1]
            consts_t["ones_r"] = ones_sb
            ones_b = consts.tile([128, 1], BF16, name="ones_b")
            nc.vector.tensor_copy(ones_b[:], ones_sb[:, 0:1])
            consts_t["ones_cb"] = ones_b[:]
            trib = consts.tile([128, 128], BF16, name="trib")
            consts_t["tri_b"] = trib
            for nm, srcn, shp in (("tri_sb", "tri", [128, 128]),
                                  ("cosa_sb", "cosA", [128, TC]),
                                  ("sina_sb", "sinA", [128, TC]),
                                  ):
                consts_t[nm] = consts.tile(shp, F32R, name=nm)
                nc.sync.dma_start(consts_t[nm][:], io[srcn][:])
            consts_t["bias_sb"] = consts.tile([128, 5], F32, name="bias_sb")
            nc.sync.dma_start(consts_t["bias_sb"][:], io["biask"][:])
            nc.vector.tensor_copy(trib[:], consts_t["tri_sb"][:])
            pswap_b = consts.tile([128, 128], BF16, name="pswap_b")
            nc.sync.dma_start(pswap_b[:], io["pswap"][:])
            consts_t["pswap_b"] = pswap_b

            with nc.named_scope("phase_a"):
                _phase_a(nc, tc, io, consts_t, ag1kv_in, ag1kv_out, agq,
                         ht_sb)

            with tc.tile_pool(name="persist", bufs=1) as persist:
                kt_sb = persist.tile([128, HL * T], BF16, name="kt_sb")
                ktv = kt_sb[:].rearrange("p (j t) -> p j t", j=HL)
                v_sb = persist.tile([128, 16 * 512], BF16, name="v_sb")
                vv = v_sb[:].rearrange("p (mt c) -> p mt c", mt=16)
                kpe_sb = persist.tile([64, T], BF16, name="kpe_sb")
                qtn = [persist.tile([128, T], BF16, name=f"qtn_{m}")
                       for m in range(4)]
                qpe = [persist.tile([64, T], BF16, name=f"qpe_{j}")
                       for j in range(4)]
                with nc.named_scope("phase_b"):
                    _phase_b(nc, tc, ag1kv_out, ktv, vv, kpe_sb, wkv_, wvv)
                with nc.named_scope("phase_q"):
                    _phase_q(nc, tc, io, agq, qtn, qpe, consts_t)
                with tc.tile_pool(name="wo_pool", bufs=1) as wo_pool:
                    wo_sb = wo_pool.tile([128, 5 * 32 * 128], BF16,
                                         name="wo_sb")
                    wov = wo_sb[:].rearrange("p (d k c) -> p d k c", d=5, k=32)
                    for d in range(5):
                        nc.sync.dma_start(wov[:, d], io["wo"][d])
                    with nc.named_scope("phase_attn"):
                        _attn_out(nc, tc, io, ag2a_ins, ag2a_outs,
                                  ag2b_ins, ag2b_outs, qtn, qpe,
                                  ktv, vv, kpe_sb, consts_t, wov)
    return nc


def _get_nc():
    if "nc" not in _cache:
        _cache["nc"] = _build()
    return _cache["nc"]


def _prep(inputs):
    h = np.asarray(inputs["h"], np.float32)
    pos = np.asarray(inputs["position_ids"], np.int32)
    Wq_a = np.asarray(inputs["Wq_a"], np.float32)
    gq = np.asarray(inputs["gq"], np.float32)
    Wq_b = np.asarray(inputs["Wq_b"], np.float32)
    Wkv_a = np.asarray(inputs["Wkv_a"], np.float32)
    bkv_a = np.asarray(inputs["bkv_a"], np.float32)
    gkv = np.asarray(inputs["gkv"], np.float32)
    Wkv_b = np.asarray(inputs["Wkv_b"], np.float32)
    Wo = np.asarray(inputs["Wo"], np.float32)

    bf16 = ml_dtypes.bfloat16
    dperm = np.concatenate([np.arange(0, ROPE, 2), np.arange(1, ROPE, 2)])
    scale = np.float32(1.0 / math.sqrt(QK))

    hT = np.ascontiguousarray(h.T)                      # [D, T]
    wkva = Wkv_a.copy()
    wkva[:, KVR:] = Wkv_a[:, KVR + dperm]
    bias = bkv_a.copy()
    bias[KVR:] = bkv_a[KVR + dperm]
    bm = np.zeros((5, 128), np.float32)
    bm.reshape(-1)[:LAT] = bias
    biask = np.ascontiguousarray(bm.T)                  # [128, 5]

    wqb_eff = (Wq_b * gq[:, None]) * scale              # [QR, H*QK]
    wkvb_eff = Wkv_b * gkv[:, None]                     # [KVR, H*(NOPE+VH)]

    # A weights pre-tiled: wa[m, p, k, c] = Wcat[k*128+p, m*128+c]
    Wcat = np.concatenate([Wq_a, wkva], axis=1)          # [D, 2112]
    A = Wcat.reshape(40, 128, QR + LAT)
    wa = np.zeros((17, 128, 40, 128), np.float32)
    for m in range(17):
        cw = min(128, QR + LAT - m * 128)
        wa[m, :, :, :cw] = A[:, :, m * 128:m * 128 + cw].transpose(1, 0, 2)
    wa = wa.astype(bf16)

    inv = THETA ** (-np.arange(0, ROPE, 2, dtype=np.float32) / ROPE)
    fr = pos.astype(np.float32)[:, None] * inv[None, :]  # [T, 32]
    cosT = np.ascontiguousarray(np.tile(np.cos(fr).T, (4, 1)))  # [128, T]
    sinT = np.ascontiguousarray(np.tile(np.sin(fr).T, (4, 1)))
    sgn = np.repeat(np.array([-1.0, 1.0, -1.0, 1.0], np.float32), 32)[:, None]
    sinS = np.ascontiguousarray(sinT * sgn)
    tri = np.triu(np.ones((128, 128), np.float32))
    pswap = np.zeros((128, 128), np.float32)
    for i in range(128):
        pswap[i ^ 32, i] = 1.0
    pswap = np.ascontiguousarray(pswap).astype(bf16)

    in_maps = []
    for c in range(NCORES):
        heads = list(range(HL * c, HL * (c + 1)))
        qcols = [np.arange(hh * QK, hh * QK + NOPE) for hh in heads]
        for pair in range(2):
            for hh in heads[2 * pair:2 * pair + 2]:
                qcols.append(hh * QK + NOPE + dperm)
        kcols = np.concatenate(
            [np.arange(hh * (NOPE + VH), hh * (NOPE + VH) + NOPE)
             for hh in heads])
        vcols = np.concatenate(
            [np.arange(hh * (NOPE + VH) + NOPE, (hh + 1) * (NOPE + VH))
             for hh in heads])

        wqb_c = wqb_eff[:, np.concatenate(qcols)]        # [1536, 768]
        wqb_t = np.ascontiguousarray(
            wqb_c.reshape(12, 128, 6, 128).transpose(2, 1, 0, 3)).astype(bf16)
        wkvbk_t = np.ascontiguousarray(
            wkvb_eff[:, kcols].reshape(4, 128, 512).transpose(1, 0, 2)
        ).astype(bf16)
        wkvbv_t = np.ascontiguousarray(
            wkvb_eff[:, vcols].reshape(4, 128, 512).transpose(1, 0, 2)
        ).astype(bf16)
        wo_c = Wo[:, c * DCOL:(c + 1) * DCOL]            # [4096, 640]
        wo_t = np.ascontiguousarray(
            wo_c.reshape(32, 128, 5, 128).transpose(2, 1, 0, 3)).astype(bf16)

        in_maps.append({
            "hT": np.ascontiguousarray(hT[:, c * TC:(c + 1) * TC]).astype(bf16),
            "wa": wa,
            "biask": biask,
            "wqb": wqb_t,
            "wkvbk": wkvbk_t,
            "wkvbv": wkvbv_t,
            "wo": wo_t,
            "cosC": cosT,
            "sinS": sinS,
            "cosA": np.ascontiguousarray(cosT[:, c * TC:(c + 1) * TC]),
            "sinA": np.ascontiguousarray(sinT[:, c * TC:(c + 1) * TC]),
            "tri": tri,
            "onesin": np.ones((128, 128), np.float32),
            "pswap": pswap,
        })
    return in_maps


def kernel(**inputs):
    nc = _get_nc()
    in_maps = _prep(inputs)
    res = bass_utils.run_bass_kernel_spmd(
        nc, in_maps, core_ids=list(range(NCORES)), trace=TRACE[0])
    LAST_RESULT[0] = res
    out = np.empty((T, D), np.float32)
    for c in range(NCORES):
        out[:, c * DCOL:(c + 1) * DCOL] = res.results[c]["outT"].T
    return out


# revision 30
# speedup vs baseline: 1.2571x; 1.0087x over previous
"""DeepseekV2 MLA attention on 8 Trainium2 NeuronCores.

Sharding: token-split A projections -> AllGather(kv latents) early +
AllGather(q latents) -> head-split (4 heads/core) B projections + causal
attention -> per-qb AllGather(attn out) with the D-column-split output
projection interleaved one block behind the attention loop.

Layouts are d-major (feature dim on the SBUF partition axis); the host
pre-transposes h and pre-tiles every weight so each SBUF weight chunk is
one contiguous DMA. q tiles stay SBUF-resident (no DRAM spill).

Precision: bf16 matmul inputs with fp32 PSUM accumulation; rmsnorm
statistics and softmax run in fp32/f32r.
"""
import math

import numpy as np
import ml_dtypes

import concourse.bass as bass
import concourse.mybir as mybir
from concourse.tile import TileContext
from concourse import bass_utils

# ---------------------------------------------------------------------------
# Walrus workaround: this container's walrus accepts at most ONE sync-wait
# per TPB instruction, but Tile attaches several (tail Drain, LDWEIGHTS...).
# Split: keep the last wait, move the rest onto preceding same-engine NOPs.
# ---------------------------------------------------------------------------
import concourse.tile as _tile_mod

_orig_sched = _tile_mod.TileContext.schedule_and_allocate
_nopctr = [0]


def _split_multiwait(nc):
    for fn in nc.m.functions:
        for blk in fn.blocks:
            insts = blk.instructions
            if not any(
                i.sync_info and i.sync_info.on_wait and len(i.sync_info.on_wait) > 1
                for i in insts
            ):
                continue
            out = []
            for ins in insts:
                si = ins.sync_info
                if si and si.on_wait and len(si.on_wait) > 1:
                    waits = list(si.on_wait)
                    for w in waits[:-1]:
                        _nopctr[0] += 1
                        nop = mybir.InstNoOp(name=f"I-mws-{_nopctr[0]}", ins=[], outs=[])
                        nop.engine = ins.engine
                        nop.sync_info = mybir.SyncInfo(on_wait=[w], on_update=[])
                        out.append(nop)
                    ins.sync_info = mybir.SyncInfo(
                        on_wait=[waits[-1]], on_update=list(si.on_update or [])
                    )
                out.append(ins)
            blk.instructions = out


def _patched_sched(self, *a, **k):
    res = _orig_sched(self, *a, **k)
    _split_multiwait(self.nc)
    return res


if getattr(_tile_mod.TileContext.schedule_and_allocate, "__name__", "") != "_patched_sched":
    _tile_mod.TileContext.schedule_and_allocate = _patched_sched


# ---------------------------------------------------------------------------
T, D, H = 2048, 5120, 32
NOPE, ROPE, QK = 128, 64, 192
KVR, QR, VH = 512, 1536, 128
EPS, THETA = 1e-6, 10000.0
NCORES = 8
HL = H // NCORES          # 4 heads per core
TC = T // NCORES          # 256 tokens per core
LAT = KVR + ROPE          # 576
DCOL = D // NCORES        # 640 output columns per core

F32 = mybir.dt.float32
F32R = mybir.dt.float32r
BF16 = mybir.dt.bfloat16
AF = mybir.ActivationFunctionType
MUL = mybir.AluOpType.mult
ADD = mybir.AluOpType.add
SUB = mybir.AluOpType.subtract

TRACE = [False]          # test.py sets TRACE[0]=True to profile
LAST_RESULT = [None]     # BassKernelResults stashed here for test.py

_cache = {}


def _rms_scale(nc, a_tmp, a_ps, ss, nfeat, ones_r, key):
    """1/sqrt(mean(ss)+eps) broadcast to [128, TC] f32r."""
    ms = a_tmp.tile([1, TC], F32, name=f"ms_{key}", tag="ms")
    nc.vector.tensor_scalar(ms[:], ss[:], 1.0 / nfeat, EPS, op0=MUL, op1=ADD)
    sm = a_tmp.tile([1, TC], F32R, name=f"sm_{key}", tag="sm")
    nc.scalar.activation(sm[:], ms[:], AF.Sqrt)
    rs = a_tmp.tile([1, TC], F32R, name=f"rs_{key}", tag="rs")
    with nc.allow_low_precision(reason="f32r holds full fp32 bits"):
        nc.vector.reciprocal(rs[:], sm[:])
    bps = a_ps.tile([128, TC], F32, name=f"bps_{key}", tag="bps")
    nc.tensor.matmul(bps[:], ones_r[:1, :], rs[:], start=True, stop=True)
    bc = a_tmp.tile([128, TC], F32R, name=f"bc_{key}", tag=f"bc{key}")
    nc.vector.tensor_copy(bc[:], bps[:])
    return bc


def _phase_a(nc, tc, io, consts_t, ag1kv_in, ag1kv_out, agq, ht_sb):
    """Token-split A projections, kv-first so its allgather ships early.

    q latents ship RAW (unnormalized) in two halves as soon as computed;
    the per-token 1/rms factor follows in a tiny third allgather and is
    folded into phase_q's consumption of the gathered latents.
    """
    ag1qa_in, ag1qa_out, ag1qb_in, ag1qb_out, ag1rs_in, ag1rs_out = agq
    ones_c, ones_r = consts_t["ones_c"], consts_t["ones_r"]
    cosa_sb, sina_sb, bias_sb = (consts_t["cosa_sb"], consts_t["sina_sb"],
                                 consts_t["bias_sb"])
    with (
        tc.tile_pool(name="a_w", bufs=3) as a_w,
        tc.tile_pool(name="a_st", bufs=1) as a_st,
        tc.tile_pool(name="a_tmp", bufs=3) as a_tmp,
        tc.tile_pool(name="a_ps", bufs=2, space="PSUM") as a_ps,
        tc.tile_pool(name="a_ss", bufs=1, space="PSUM") as a_ss,
    ):
        htv = ht_sb[:].rearrange("p (k t) -> p k t", k=40)
        stage = a_st.tile([128, 17 * TC], F32R, name="stage")
        stg_q = a_st.tile([128, 12 * TC], BF16, name="stg_q")
        ss_q = a_ss.tile([1, TC], F32, name="ss_q")
        ss_kv = a_ss.tile([1, TC], F32, name="ss_kv")

        def mchunk(m):
            mrows = 64 if m == 16 else 128
            wt = a_w.tile([128, 40 * 128], BF16, name=f"a_w_{m}", tag="aw")
            wtv = wt[:].rearrange("p (k c) -> p k c", k=40)
            nc.sync.dma_start(wtv, io["wa"][m])
            ps = a_ps.tile([128, TC], F32, name=f"a_ps_{m}", tag="aps")
            for k in range(40):
                nc.tensor.matmul(ps[:mrows, :], wtv[:, k, :mrows], htv[:, k, :],
                                 start=(k == 0), stop=(k == 39))
            st = stage[:, m * TC:(m + 1) * TC]
            if m < 12:
                sq = a_tmp.tile([128, TC], F32R, name=f"sq_{m}", tag="sq")
                nc.scalar.activation(sq[:], ps[:], AF.Square)
                nc.tensor.matmul(ss_q[:], ones_c, sq[:],
                                 start=(m == 0), stop=(m == 11))
                nc.vector.tensor_copy(stg_q[:, m * TC:(m + 1) * TC], ps[:])
            elif m < 16:
                nc.vector.tensor_scalar(st, ps[:], bias_sb[:, m - 12:m - 11],
                                        None, op0=ADD)
                sq = a_tmp.tile([128, TC], F32R, name=f"sq_{m}", tag="sq")
                nc.scalar.activation(sq[:], st, AF.Square)
                nc.tensor.matmul(ss_kv[:], ones_c, sq[:],
                                 start=(m == 12), stop=(m == 15))
            else:
                nc.vector.tensor_scalar(st[:64, :], ps[:64, :],
                                        bias_sb[:64, 4:5], None, op0=ADD)

        # --- kv chunks first ---
        for m in range(12, 17):
            mchunk(m)
        bc_kv = _rms_scale(nc, a_tmp, a_ps, ss_kv, KVR, ones_r, "kv")
        stg_kv = a_st.tile([128, 4 * TC], BF16, name="stg_kv")
        for i in range(4):
            st = stage[:, (12 + i) * TC:(13 + i) * TC]
            nc.vector.tensor_tensor(stg_kv[:, i * TC:(i + 1) * TC], st,
                                    bc_kv[:], op=MUL)
        # k_pe rope (no norm) -> rows 512:576
        st = stage[:, 16 * TC:17 * TC]
        rp = a_tmp.tile([64, TC], BF16, name="rp_kpe")
        t1 = a_tmp.tile([32, TC], F32R, name="rt1", tag="rt1")
        t2 = a_tmp.tile([32, TC], F32R, name="rt2", tag="rt2")
        x1, x2 = st[0:32, :], st[32:64, :]
        nc.vector.tensor_tensor(t1[:], x1, cosa_sb[0:32, :], op=MUL)
        nc.vector.tensor_tensor(t2[:], x2, sina_sb[32:64, :], op=MUL)
        nc.vector.tensor_tensor(rp[0:32, :], t1[:], t2[:], op=SUB)
        nc.vector.tensor_tensor(t1[:], x1, sina_sb[0:32, :], op=MUL)
        nc.vector.tensor_tensor(t2[:], x2, cosa_sb[32:64, :], op=MUL)
        nc.vector.tensor_tensor(rp[32:64, :], t1[:], t2[:], op=ADD)
        nc.sync.dma_start(
            ag1kv_in[0:512, :].rearrange("(k p) t -> p k t", p=128),
            stg_kv[:].rearrange("p (k t) -> p k t", k=4))
        nc.sync.dma_start(ag1kv_in[512:576, :], rp[:])

        nc.gpsimd.collective_compute(
            "AllGather", mybir.AluOpType.bypass,
            ins=[ag1kv_in[:]], outs=[ag1kv_out[:]],
            replica_groups=[list(range(NCORES))],
        )

        # --- q chunks: raw latents ship in halves as soon as computed ---
        for m in range(6):
            mchunk(m)
        nc.sync.dma_start(
            ag1qa_in[:].rearrange("(k p) t -> p k t", p=128),
            stg_q[:, 0:6 * TC].rearrange("p (k t) -> p k t", k=6))
        nc.gpsimd.collective_compute(
            "AllGather", mybir.AluOpType.bypass,
            ins=[ag1qa_in[:]], outs=[ag1qa_out[:]],
            replica_groups=[list(range(NCORES))],
        )
        for m in range(6, 12):
            mchunk(m)
        nc.sync.dma_start(
            ag1qb_in[:].rearrange("(k p) t -> p k t", p=128),
            stg_q[:, 6 * TC:12 * TC].rearrange("p (k t) -> p k t", k=6))
        nc.gpsimd.collective_compute(
            "AllGather", mybir.AluOpType.bypass,
            ins=[ag1qb_in[:]], outs=[ag1qb_out[:]],
            replica_groups=[list(range(NCORES))],
        )
        # rs_q = 1/sqrt(mean(ss_q)+eps) row, gathered separately
        ms = a_tmp.tile([1, TC], F32, name="ms_q", tag="ms")
        nc.vector.tensor_scalar(ms[:], ss_q[:], 1.0 / QR, EPS,
                                op0=MUL, op1=ADD)
        sm = a_tmp.tile([1, TC], F32R, name="sm_q", tag="sm")
        nc.scalar.activation(sm[:], ms[:], AF.Sqrt)
        rsq = a_tmp.tile([1, TC], F32R, name="rs_q", tag="rs")
        with nc.allow_low_precision(reason="f32r holds full fp32 bits"):
            nc.vector.reciprocal(rsq[:], sm[:])
        nc.sync.dma_start(ag1rs_in[:], rsq[:])
        nc.gpsimd.collective_compute(
            "AllGather", mybir.AluOpType.bypass,
            ins=[ag1rs_in[:]], outs=[ag1rs_out[:]],
            replica_groups=[list(range(NCORES))],
        )


def _phase_b(nc, tc, ag1kv_out, ktv, vv, kpe_sb, wkv_, wvv):
    """Head-split k_nope^T and v projections from the gathered kv latents."""
    ag1kv_v = ag1kv_out[:].rearrange("(r a) t -> a r t", a=LAT)
    with (
        tc.tile_pool(name="b_kva", bufs=1) as b_kva,
        tc.tile_pool(name="b_ps", bufs=3, space="PSUM") as b_ps,
    ):
        kva_sb = b_kva.tile([128, 4 * T], BF16, name="kva_sb")
        kvav = kva_sb[:].rearrange("p (k t) -> p k t", k=4)
        for k in range(4):
            nc.sync.dma_start(
                kvav[:, k, :].rearrange("p (r t) -> p r t", r=NCORES),
                ag1kv_v[k * 128:(k + 1) * 128])
        nc.sync.dma_start(
            kpe_sb[:].rearrange("p (r t) -> p r t", r=NCORES),
            ag1kv_v[512:576])

        for j in range(HL):
            for qb in range(4):
                ps = b_ps.tile([128, 512], F32, name=f"psk_{j}_{qb}", tag="psk")
                for k in range(4):
                    nc.tensor.matmul(ps[:], wkv_[:, k, j * 128:(j + 1) * 128],
                                     kvav[:, k, qb * 512:(qb + 1) * 512],
                                     start=(k == 0), stop=(k == 3))
                nc.scalar.activation(ktv[:, j, qb * 512:(qb + 1) * 512], ps[:],
                                     AF.Copy)

        for mt in range(16):
            ps = b_ps.tile([128, 512], F32, name=f"psv_{mt}", tag="psv")
            for k in range(4):
                nc.tensor.matmul(ps[:], kvav[:, k, mt * 128:(mt + 1) * 128],
                                 wvv[:, k, :], start=(k == 0), stop=(k == 3))
            nc.scalar.activation(vv[:, mt, :], ps[:], AF.Copy)


def _phase_q(nc, tc, io, agq, qtn, qpe, consts_t):
    """Head-split q^T projection from raw gathered latents.

    The per-token rms factor (gathered separately) is folded into the
    PSUM evacuation for the nope chunks and pre-multiplied into the
    cos/sin tables for the rope chunks. Rope itself is full-tile ops
    plus a PE half-swap via a permutation matmul.
    """
    _, ag1qa_out, _, ag1qb_out, _, ag1rs_out = agq
    pswap_b, ones_r = consts_t["pswap_b"], consts_t["ones_r"]
    with (
        tc.tile_pool(name="c_qa", bufs=1) as c_qa,
        tc.tile_pool(name="c_tab", bufs=1) as c_tab,
        tc.tile_pool(name="c_w", bufs=1) as c_w,
        tc.tile_pool(name="c_tmp", bufs=2) as c_tmp,
        tc.tile_pool(name="c_ps", bufs=6, space="PSUM") as c_ps,
        tc.tile_pool(name="c_sw", bufs=2, space="PSUM") as c_sw,
    ):
        cos_sb = c_tab.tile([128, T], F32R, name="cos_sb")
        sin_sb = c_tab.tile([128, T], F32R, name="sin_sb")
        nc.sync.dma_start(cos_sb[:], io["cosC"][:])
        nc.sync.dma_start(sin_sb[:], io["sinS"][:])
        qa_sb = c_qa.tile([128, 12 * T], BF16, name="qa_sb")
        qav = qa_sb[:].rearrange("p (k t) -> p k t", k=12)
        qa_v = ag1qa_out[:].rearrange("(r a) t -> a r t", a=6 * 128)
        qb_v = ag1qb_out[:].rearrange("(r a) t -> a r t", a=6 * 128)
        for k in range(6):
            nc.sync.dma_start(
                qav[:, k, :].rearrange("p (r t) -> p r t", r=NCORES),
                qa_v[k * 128:(k + 1) * 128])
        for k in range(6, 12):
            nc.sync.dma_start(
                qav[:, k, :].rearrange("p (r t) -> p r t", r=NCORES),
                qb_v[(k - 6) * 128:(k - 5) * 128])
        wq_sb = c_w.tile([128, 6 * 12 * 128], BF16, name="wq_sb")
        wqv = wq_sb[:].rearrange("p (m k c) -> p m k c", m=6, k=12)
        for m in range(6):
            nc.sync.dma_start(wqv[:, m], io["wqb"][m])

        # rs broadcast [128, T] f32r, then fold into the trig tables
        rsg = c_tab.tile([1, T], F32R, name="rsg")
        nc.sync.dma_start(
            rsg[:], ag1rs_out[:].rearrange("(o r) t -> o (r t)", o=1))
        rsb = c_tab.tile([128, T], F32R, name="rsb")
        for r in range(0, NCORES, 2):
            bp = c_sw.tile([128, 2 * TC], F32, name=f"rsb_{r}", tag="sw")
            nc.tensor.matmul(bp[:], ones_r[:1, :],
                             rsg[:, r * TC:(r + 2) * TC],
                             start=True, stop=True)
            nc.scalar.activation(rsb[:, r * TC:(r + 2) * TC], bp[:], AF.Copy)
        nc.vector.tensor_tensor(cos_sb[:], cos_sb[:], rsb[:], op=MUL)
        nc.vector.tensor_tensor(sin_sb[:], sin_sb[:], rsb[:], op=MUL)

        for qb in range(4):
            cols = slice(qb * 512, (qb + 1) * 512)
            pss = []
            for m in range(6):
                ps = c_ps.tile([128, 512], F32, name=f"psq_{qb}_{m}",
                               tag="psq")
                for k in range(6):
                    nc.tensor.matmul(ps[:], wqv[:, m, k, :], qav[:, k, cols],
                                     start=(k == 0), stop=False)
                pss.append(ps)
            for m in range(6):
                ps = pss[m]
                for k in range(6, 12):
                    nc.tensor.matmul(ps[:], wqv[:, m, k, :], qav[:, k, cols],
                                     start=False, stop=(k == 11))
                if m < 4:
                    nc.vector.tensor_tensor(qtn[m][:, cols], ps[:],
                                            rsb[:, cols], op=MUL)
                else:
                    cs = c_tmp.tile([128, 512], BF16, name=f"cs_{qb}_{m}",
                                    tag="cs")
                    nc.scalar.activation(cs[:], ps[:], AF.Copy)
                    sw = c_sw.tile([128, 512], F32, name=f"sw_{qb}_{m}",
                                   tag="sw")
                    nc.tensor.matmul(sw[:], pswap_b[:], cs[:],
                                     start=True, stop=True)
                    t1 = c_tmp.tile([128, 512], F32R, name=f"t1_{qb}_{m}",
                                    tag="t1")
                    t2 = c_tmp.tile([128, 512], F32R, name=f"t2_{qb}_{m}",
                                    tag="t2")
                    nc.vector.tensor_tensor(t1[:], ps[:], cos_sb[:, cols],
                                            op=MUL)
                    nc.vector.tensor_tensor(t2[:], sw[:], sin_sb[:, cols],
                                            op=MUL)
                    jj = 2 * (m - 4)
                    nc.vector.tensor_tensor(qpe[jj][:, cols], t1[0:64, :],
                                            t2[0:64, :], op=ADD)
                    nc.vector.tensor_tensor(qpe[jj + 1][:, cols], t1[64:128, :],
                                            t2[64:128, :], op=ADD)


def _attn_out(nc, tc, io, ag2a_ins, ag2a_outs, ag2b_ins, ag2b_outs,
              qtn, qpe, ktv, vv, kpe_sb, consts_t, wov):
    """Causal attention per qb + interleaved output projection.

    Software-pipelined: den/PV matmuls trail the score matmuls by one
    kk-pair in the PE FIFO (so exp never stalls PE); per-head finals are
    split (reciprocal early, broadcast/rescale two pairs later). The
    attention-output allgather is split per head-pair (a: heads 0-1,
    b: heads 2-3) so the output projection of a token block can start
    on the a-half while the b-half is still in flight; each out half is
    accumulated in PSUM, evacuated, and the halves summed on the DVE.
    """
    ones_cb, ones_r, tri_b = (consts_t["ones_cb"], consts_t["ones_r"],
                              consts_t["tri_b"])
    with (
        tc.tile_pool(name="t_pt", bufs=3) as t_pt,
        tc.tile_pool(name="t_fin", bufs=2) as t_fin,
        tc.tile_pool(name="t_ring", bufs=1, space="PSUM") as t_ring,
        tc.tile_pool(name="t_ots", bufs=2, space="PSUM") as t_ots,
        tc.tile_pool(name="t_dpo", bufs=2, space="PSUM") as t_dpo,
        tc.tile_pool(name="o_oa", bufs=1) as o_oa,
        tc.tile_pool(name="o_part", bufs=5) as o_part,
        tc.tile_pool(name="o_st", bufs=2) as o_st,
    ):
        ring = t_ring.tile([128, 2048], F32, name="ring")
        oa_t = {}
        parts = {}

        def out_dma(tq):
            oaa = o_oa.tile([128, 16 * 512], BF16, name=f"oaa_{tq}", tag="oaa")
            nc.sync.dma_start(
                oaa[:].rearrange("p (k t) -> p k t", k=16),
                ag2a_outs[tq][:].rearrange("(k p) t -> p k t", p=128))
            oab = o_oa.tile([128, 16 * 512], BF16, name=f"oab_{tq}", tag="oab")
            nc.sync.dma_start(
                oab[:].rearrange("p (k t) -> p k t", k=16),
                ag2b_outs[tq][:].rearrange("(k p) t -> p k t", p=128))
            oa_t[tq] = (oaa[:].rearrange("p (k t) -> p k t", k=16),
                        oab[:].rearrange("p (k t) -> p k t", k=16))

        def out_half(tq, d, half):
            oav = oa_t[tq][half]
            ps = t_dpo.tile([128, 512], F32, name=f"ops_{tq}_{d}_{half}",
                            tag="dpo")
            for i in range(16):
                kg = 4 * (i // 2) + 2 * half + (i % 2)
                nc.tensor.matmul(ps[:], wov[:, d, kg, :], oav[:, i, :],
                                 start=(i == 0), stop=(i == 15))
            if half == 0:
                pt = o_part.tile([128, 512], F32, name=f"part_{tq}_{d}",
                                 tag="part")
                nc.vector.tensor_copy(pt[:], ps[:])
                parts[(tq, d)] = pt
            else:
                st = o_st.tile([128, 512], F32, name=f"ost_{tq}_{d}",
                               tag="ost")
                nc.vector.tensor_tensor(st[:], ps[:], parts.pop((tq, d))[:],
                                        op=ADD)
                nc.sync.dma_start(
                    io["outT"][d * 128:(d + 1) * 128,
                               tq * 512:(tq + 1) * 512], st[:])

        def out_rest(tq, d_from_a):
            for d in range(d_from_a, 5):
                out_half(tq, d, 0)
                if d >= 2:
                    out_half(tq, d - 2, 1)
            out_half(tq, 3, 1)
            out_half(tq, 4, 1)

        gctr = [0]

        def finals_a(qb, j, den):
            den_s = t_fin.tile([1, 512], F32R, name=f"dns_{qb}_{j}",
                               tag="dns")
            nc.scalar.activation(den_s[:], den[:], AF.Copy)
            rden = t_fin.tile([1, 512], F32R, name=f"rd_{qb}_{j}", tag="rd")
            with nc.allow_low_precision(reason="f32r = fp32 bits"):
                nc.vector.reciprocal(rden[:], den_s[:])
            return rden

        def finals_b(qb, j, ots, rden):
            bcp = t_dpo.tile([128, 512], F32, name=f"bcp_{qb}_{j}", tag="dpo")
            nc.tensor.matmul(bcp[:], ones_r[:1, :], rden[:],
                             start=True, stop=True)
            bcs = t_fin.tile([128, 512], F32R, name=f"bcs_{qb}_{j}", tag="bcs")
            nc.scalar.activation(bcs[:], bcp[:], AF.Copy)
            obf = t_fin.tile([128, 512], BF16, name=f"obf_{qb}_{j}", tag="obf")
            nc.vector.tensor_tensor(obf[:], ots[:], bcs[:], op=MUL)
            if j < 2:
                nc.sync.dma_start(ag2a_ins[qb][j * 128:(j + 1) * 128, :],
                                  obf[:])
            else:
                nc.sync.dma_start(ag2b_ins[qb][(j - 2) * 128:(j - 1) * 128, :],
                                  obf[:])
            if j == 1:
                nc.gpsimd.collective_compute(
                    "AllGather", mybir.AluOpType.bypass,
                    ins=[ag2a_ins[qb][:]], outs=[ag2a_outs[qb][:]],
                    replica_groups=[list(range(NCORES))],
                )

        for qb in range(4):
            kmax = 4 * qb + 4
            npairs = kmax // 2
            cols = slice(qb * 512, (qb + 1) * 512)
            state = {}           # j -> (ots, den)
            pend = None          # (j, p, pt)
            finq = []            # [(emit_iter, j, ots, rden)]
            it = [0]

            def denpv(j, p, pt):
                if p == 0:
                    state[j] = (
                        t_ots.tile([128, 512], F32, name=f"ot_{qb}_{j}",
                                   tag="ots"),
                        t_dpo.tile([1, 512], F32, name=f"den_{qb}_{j}",
                                   tag="dpo"),
                    )
                ots, den = state[j]
                for kk in (2 * p, 2 * p + 1):
                    c0 = max(0, kk - 4 * qb) * 128
                    psl = pt[:, (kk % 2) * 512 + c0:(kk % 2) * 512 + 512]
                    nc.tensor.matmul(den[:, c0:512], ones_cb, psl,
                                     start=(kk == 0), stop=(kk == kmax - 1))
                    nc.tensor.matmul(ots[:, c0:512],
                                     vv[:, kk, j * 128:(j + 1) * 128], psl,
                                     start=(kk == 0), stop=(kk == kmax - 1))
                if p == npairs - 1:
                    finq.append((it[0], j, ots, finals_a(qb, j, den)))

            def flush_finals(min_age):
                while finq and it[0] - finq[0][0] >= min_age:
                    _, j, ots, rden = finq.pop(0)
                    finals_b(qb, j, ots, rden)

            for j in range(HL):
                qfn = qtn[j][:, cols]
                qfp = qpe[j][:, cols]
                for p in range(npairs):
                    kk0 = 2 * p
                    off = (gctr[0] % 2) * 1024
                    gctr[0] += 1
                    for kk in (kk0, kk0 + 1):
                        c0 = max(0, kk - 4 * qb) * 128
                        sl = ring[:, off + (kk % 2) * 512 + c0:
                                  off + (kk % 2) * 512 + 512]
                        nc.tensor.matmul(sl,
                                         ktv[:, j, kk * 128:(kk + 1) * 128],
                                         qfn[:, c0:512], start=True, stop=False)
                        nc.tensor.matmul(sl,
                                         kpe_sb[:, kk * 128:(kk + 1) * 128],
                                         qfp[:, c0:512], start=False, stop=True)
                    pt = t_pt.tile([128, 1024], BF16, name=f"pt_{qb}_{j}_{p}",
                                   tag="pt")
                    c0f = max(0, kk0 - 4 * qb) * 128
                    nc.scalar.activation(pt[:, c0f:1024],
                                         ring[:, off + c0f:off + 1024], AF.Exp)
                    for kk in (kk0, kk0 + 1):
                        o = kk - 4 * qb
                        if o >= 0:
                            d0 = (kk % 2) * 512 + o * 128
                            nc.vector.tensor_tensor(pt[:, d0:d0 + 128],
                                                    pt[:, d0:d0 + 128],
                                                    tri_b[:], op=MUL)
                    flush_finals(2)
                    if pend is not None:
                        denpv(*pend)
                    pend = (j, p, pt)
                    it[0] += 1
            denpv(*pend)
            # PE filler (a-halves of the previous out block) while the
            # last heads' reciprocals run
            if qb >= 1:
                out_half(qb - 1, 0, 0)
                out_half(qb - 1, 1, 0)
            flush_finals(0)
            nc.gpsimd.collective_compute(
                "AllGather", mybir.AluOpType.bypass,
                ins=[ag2b_ins[qb][:]], outs=[ag2b_outs[qb][:]],
                replica_groups=[list(range(NCORES))],
            )
            out_dma(qb)
            if qb >= 1:
                out_rest(qb - 1, 2)
        out_rest(3, 0)


def _build():
    nc = bass.Bass("TRN2", target_bir_lowering=False, debug=False,
                   num_devices=NCORES)
    io = {
        "hT": nc.dram_tensor("hT", [D, TC], BF16, kind="ExternalInput"),
        "wa": nc.dram_tensor("wa", [17, 128, 40, 128], BF16,
                             kind="ExternalInput"),
        "biask": nc.dram_tensor("biask", [128, 5], F32, kind="ExternalInput"),
        "wqb": nc.dram_tensor("wqb", [6, 128, 12, 128], BF16,
                              kind="ExternalInput"),
        "wkvbk": nc.dram_tensor("wkvbk", [128, 4, 512], BF16,
                                kind="ExternalInput"),
        "wkvbv": nc.dram_tensor("wkvbv", [128, 4, 512], BF16,
                                kind="ExternalInput"),
        "wo": nc.dram_tensor("wo", [5, 128, 32, 128], BF16,
                             kind="ExternalInput"),
        "cosC": nc.dram_tensor("cosC", [128, T], F32R, kind="ExternalInput"),
        "sinS": nc.dram_tensor("sinS", [128, T], F32R, kind="ExternalInput"),
        "cosA": nc.dram_tensor("cosA", [128, TC], F32R, kind="ExternalInput"),
        "sinA": nc.dram_tensor("sinA", [128, TC], F32R, kind="ExternalInput"),
        "tri": nc.dram_tensor("tri", [128, 128], F32R, kind="ExternalInput"),
        "onesin": nc.dram_tensor("onesin", [128, 128], F32R,
                                 kind="ExternalInput"),
        "pswap": nc.dram_tensor("pswap", [128, 128], BF16,
                                kind="ExternalInput"),
        "outT": nc.dram_tensor("outT", [DCOL, T], F32, kind="ExternalOutput"),
    }

    with TileContext(nc) as tc:
        with (
            tc.tile_pool(name="dram", bufs=1, space="DRAM") as dram,
            tc.tile_pool(name="consts", bufs=1) as consts,
            tc.tile_pool(name="a_ht", bufs=1) as a_ht,
            tc.tile_pool(name="b_w", bufs=1) as b_w,
        ):
            # earliest DMAs: h transpose chunks + phase_b weights (no deps)
            ht_sb = a_ht.tile([128, 40 * TC], BF16, name="ht_sb")
            htv = ht_sb[:].rearrange("p (k t) -> p k t", k=40)
            hsrc = io["hT"][:].rearrange("(k p) t -> p k t", p=128)
            for k0 in range(0, 40, 10):
                nc.sync.dma_start(htv[:, k0:k0 + 10, :], hsrc[:, k0:k0 + 10, :])
            wk_sb = b_w.tile([128, 4 * 512], BF16, name="wk_sb")
            wkv_ = wk_sb[:].rearrange("p (k c) -> p k c", k=4)
            nc.sync.dma_start(wkv_, io["wkvbk"][:])
            wv_sb = b_w.tile([128, 4 * 512], BF16, name="wv_sb")
            wvv = wv_sb[:].rearrange("p (k c) -> p k c", k=4)
            nc.sync.dma_start(wvv, io["wkvbv"][:])
            ag1kv_in = dram.tile([LAT, TC], BF16, name="ag1kv_in")
            ag1kv_out = dram.tile([NCORES * LAT, TC], BF16, addr_space="Shared",
                                  name="ag1kv_out")
            agq = (
                dram.tile([QR // 2, TC], BF16, name="ag1qa_in"),
                dram.tile([NCORES * QR // 2, TC], BF16, addr_space="Shared",
                          name="ag1qa_out"),
                dram.tile([QR // 2, TC], BF16, name="ag1qb_in"),
                dram.tile([NCORES * QR // 2, TC], BF16, addr_space="Shared",
                          name="ag1qb_out"),
                dram.tile([1, TC], F32R, name="ag1rs_in"),
                dram.tile([NCORES, TC], F32R, addr_space="Shared",
                          name="ag1rs_out"),
            )
            ag2a_ins = [dram.tile([2 * VH, 512], BF16, name=f"ag2a_in_{qb}")
                        for qb in range(4)]
            ag2a_outs = [dram.tile([NCORES * 2 * VH, 512], BF16,
                                   addr_space="Shared",
                                   name=f"ag2a_out_{qb}") for qb in range(4)]
            ag2b_ins = [dram.tile([2 * VH, 512], BF16, name=f"ag2b_in_{qb}")
                        for qb in range(4)]
            ag2b_outs = [dram.tile([NCORES * 2 * VH, 512], BF16,
                                   addr_space="Shared",
                                   name=f"ag2b_out_{qb}") for qb in range(4)]

            consts_t = {}
            ones_sb = consts.tile([128, 128], F32R, name="ones_sb")
            nc.sync.dma_start(ones_sb[:], io["onesin"][:])
            consts_t["ones_c"] = ones_sb[:, 0:1]
            consts_t["ones_r"] = ones_sb
            ones_b = consts.tile([128, 1], BF16, name="ones_b")
            nc.vector.tensor_copy(ones_b[:], ones_sb[:, 0:1])
            consts_t["ones_cb"] = ones_b[:]
            trib = consts.tile([128, 128], BF16, name="trib")
            consts_t["tri_b"] = trib
            for nm, srcn, shp in (("tri_sb", "tri", [128, 128]),
                                  ("cosa_sb", "cosA", [128, TC]),
                                  ("sina_sb", "sinA", [128, TC]),
                                  ):
                consts_t[nm] = consts.tile(shp, F32R, name=nm)
                nc.sync.dma_start(consts_t[nm][:], io[srcn][:])
            consts_t["bias_sb"] = consts.tile([128, 5], F32, name="bias_sb")
            nc.sync.dma_start(consts_t["bias_sb"][:], io["biask"][:])
            nc.vector.tensor_copy(trib[:], consts_t["tri_sb"][:])
            pswap_b = consts.tile([128, 128], BF16, name="pswap_b")
            nc.sync.dma_start(pswap_b[:], io["pswap"][:])
            consts_t["pswap_b"] = pswap_b

            with nc.named_scope("phase_a"):
                _phase_a(nc, tc, io, consts_t, ag1kv_in, ag1kv_out, agq,
                         ht_sb)

            with tc.tile_pool(name="persist", bufs=1) as persist:
                kt_sb = persist.tile([128, HL * T], BF16, name="kt_sb")
                ktv = kt_sb[:].rearrange("p (j t) -> p j t", j=HL)
                v_sb = persist.tile([128, 16 * 512], BF16, name="v_sb")
                vv = v_sb[:].rearrange("p (mt c) -> p mt c", mt=16)
                kpe_sb = persist.tile([64, T], BF16, name="kpe_sb")
                qtn = [persist.tile([128, T], BF16, name=f"qtn_{m}")
                       for m in range(4)]
                qpe = [persist.tile([64, T], BF16, name=f"qpe_{j}")
                       for j in range(4)]
                with nc.named_scope("phase_b"):
                    _phase_b(nc, tc, ag1kv_out, ktv, vv, kpe_sb, wkv_, wvv)
                with nc.named_scope("phase_q"):
                    _phase_q(nc, tc, io, agq, qtn, qpe, consts_t)
                with tc.tile_pool(name="wo_pool", bufs=1) as wo_pool:
                    wo_sb = wo_pool.tile([128, 5 * 32 * 128], BF16,
                                         name="wo_sb")
                    wov = wo_sb[:].rearrange("p (d k c) -> p d k c", d=5, k=32)
                    for d in range(5):
                        nc.sync.dma_start(wov[:, d], io["wo"][d])
                    with nc.named_scope("phase_attn"):
                        _attn_out(nc, tc, io, ag2a_ins, ag2a_outs,
                                  ag2b_ins, ag2b_outs, qtn, qpe,
                                  ktv, vv, kpe_sb, consts_t, wov)
    return nc


def _get_nc():
    if "nc" not in _cache:
        _cache["nc"] = _build()
    return _cache["nc"]


def _prep(inputs):
    h = np.asarray(inputs["h"], np.float32)
    pos = np.asarray(inputs["position_ids"], np.int32)
    Wq_a = np.asarray(inputs["Wq_a"], np.float32)
    gq = np.asarray(inputs["gq"], np.float32)
    Wq_b = np.asarray(inputs["Wq_b"], np.float32)
    Wkv_a = np.asarray(inputs["Wkv_a"], np.float32)
    bkv_a = np.asarray(inputs["bkv_a"], np.float32)
    gkv = np.asarray(inputs["gkv"], np.float32)
    Wkv_b = np.asarray(inputs["Wkv_b"], np.float32)
    Wo = np.asarray(inputs["Wo"], np.float32)

    bf16 = ml_dtypes.bfloat16
    dperm = np.concatenate([np.arange(0, ROPE, 2), np.arange(1, ROPE, 2)])
    scale = np.float32(1.0 / math.sqrt(QK))

    hT = np.ascontiguousarray(h.T)                      # [D, T]
    wkva = Wkv_a.copy()
    wkva[:, KVR:] = Wkv_a[:, KVR + dperm]
    bias = bkv_a.copy()
    bias[KVR:] = bkv_a[KVR + dperm]
    bm = np.zeros((5, 128), np.float32)
    bm.reshape(-1)[:LAT] = bias
    biask = np.ascontiguousarray(bm.T)                  # [128, 5]

    wqb_eff = (Wq_b * gq[:, None]) * scale              # [QR, H*QK]
    wkvb_eff = Wkv_b * gkv[:, None]                     # [KVR, H*(NOPE+VH)]

    # A weights pre-tiled: wa[m, p, k, c] = Wcat[k*128+p, m*128+c]
    Wcat = np.concatenate([Wq_a, wkva], axis=1)          # [D, 2112]
    A = Wcat.reshape(40, 128, QR + LAT)
    wa = np.zeros((17, 128, 40, 128), np.float32)
    for m in range(17):
        cw = min(128, QR + LAT - m * 128)
        wa[m, :, :, :cw] = A[:, :, m * 128:m * 128 + cw].transpose(1, 0, 2)
    wa = wa.astype(bf16)

    inv = THETA ** (-np.arange(0, ROPE, 2, dtype=np.float32) / ROPE)
    fr = pos.astype(np.float32)[:, None] * inv[None, :]  # [T, 32]
    cosT = np.ascontiguousarray(np.tile(np.cos(fr).T, (4, 1)))  # [128, T]
    sinT = np.ascontiguousarray(np.tile(np.sin(fr).T, (4, 1)))
    sgn = np.repeat(np.array([-1.0, 1.0, -1.0, 1.0], np.float32), 32)[:, None]
    sinS = np.ascontiguousarray(sinT * sgn)
    tri = np.triu(np.ones((128, 128), np.float32))
    pswap = np.zeros((128, 128), np.float32)
    for i in range(128):
        pswap[i ^ 32, i] = 1.0
    pswap = np.ascontiguousarray(pswap).astype(bf16)

    in_maps = []
    for c in range(NCORES):
        heads = list(range(HL * c, HL * (c + 1)))
        qcols = [np.arange(hh * QK, hh * QK + NOPE) for hh in heads]
        for pair in range(2):
            for hh in heads[2 * pair:2 * pair + 2]:
                qcols.append(hh * QK + NOPE + dperm)
        kcols = np.concatenate(
            [np.arange(hh * (NOPE + VH), hh * (NOPE + VH) + NOPE)
             for hh in heads])
        vcols = np.concatenate(
            [np.arange(hh * (NOPE + VH) + NOPE, (hh + 1) * (NOPE + VH))
             for hh in heads])

        wqb_c = wqb_eff[:, np.concatenate(qcols)]        # [1536, 768]
        wqb_t = np.ascontiguousarray(
            wqb_c.reshape(12, 128, 6, 128).transpose(2, 1, 0, 3)).astype(bf16)
        wkvbk_t = np.ascontiguousarray(
            wkvb_eff[:, kcols].reshape(4, 128, 512).transpose(1, 0, 2)
        ).astype(bf16)
        wkvbv_t = np.ascontiguousarray(
            wkvb_eff[:, vcols].reshape(4, 128, 512).transpose(1, 0, 2)
        ).astype(bf16)
        wo_c = Wo[:, c * DCOL:(c + 1) * DCOL]            # [4096, 640]
        wo_t = np.ascontiguousarray(
            wo_c.reshape(32, 128, 5, 128).transpose(2, 1, 0, 3)).astype(bf16)

        in_maps.append({
            "hT": np.ascontiguousarray(hT[:, c * TC:(c + 1) * TC]).astype(bf16),
            "wa": wa,
            "biask": biask,
            "wqb": wqb_t,
            "wkvbk": wkvbk_t,
            "wkvbv": wkvbv_t,
            "wo": wo_t,
            "cosC": cosT,
            "sinS": sinS,
            "cosA": np.ascontiguousarray(cosT[:, c * TC:(c + 1) * TC]),
            "sinA": np.ascontiguousarray(sinT[:, c * TC:(c + 1) * TC]),
            "tri": tri,
            "onesin": np.ones((128, 128), np.float32),
            "pswap": pswap,
        })
    return in_maps


def kernel(**inputs):
    nc = _get_nc()
    in_maps = _prep(inputs)
    res = bass_utils.run_bass_kernel_spmd(
        nc, in_maps, core_ids=list(range(NCORES)), trace=TRACE[0])
    LAST_RESULT[0] = res
    out = np.empty((T, D), np.float32)
    for c in range(NCORES):
        out[:, c * DCOL:(c + 1) * DCOL] = res.results[c]["outT"].T
    return out
